# revision 1
# baseline (speedup 1.0000x reference)
"""Trainium2 Bass kernel for BaselineGIN (nn_BaselineGIN_42502996361221).

Strategy (8 NeuronCores, SPMD):
  - Nodes sharded 12500/core, padded to 12544 (=98*128) columns per core.
  - Node features live transposed in SBUF: xT [128 feat, 12544 nodes] fp32.
  - Full node table x_full [100352, 128] bf16 in Shared DRAM, rebuilt by
    AllGather before every layer (bf16 halves gather traffic).
  - Edges partitioned by dst owner, sorted by (dst window of 128, src bucket
    of 25088 rows); gathered with dma_gather (int16 bucket-relative indices),
    then scatter-added into PSUM windows via one-hot matmuls
    (agg^T[f, w] += X_tile^T @ onehot(dstrel)).
  - GIN MLP per 512-node column group; BatchNorm batch stats via per-core
    sums + tiny AllReduce; BN1 z recomputed after the stats AllReduce so Z1
    is never materialized full-width.
  - Readout: segment sums via one-hot(batch) matmuls over PE-transposed
    chunks; segment max via tensor_tensor_scan (mult-mask reset, data>=0)
    plus one-hot(last-node) extraction matmuls. Host combines the per-core
    partials (straddled graphs) and runs the tiny [512,384] classifier.

kernel() accepts the FULL inputs and returns the FULL [512, 2] output.
"""

import math
import os
import time

import numpy as np

DEBUG_STAGE = int(os.environ.get('GIN_DEBUG_STAGE', '0'))

# ---------------- problem constants (hardcoded per spec) ----------------
N_NODES = 100000
N_EDGES = 500000
D = 128
G_GRAPHS = 512
L_LAYERS = 4
NC = 8                      # cores
NPC = N_NODES // NC         # real nodes per core = 12500
M = 12544                   # padded per-core columns (98 * 128)
NW = M // 128               # 98 dst windows per core
NBUCK = 4
BUCK = NC * M // NBUCK      # 25088 padded rows per src bucket (< 32768)
BN_EPS = 1e-5
GCOL = 512                  # MLP column-group width
WG = 16      # dst windows per gather-call group

# dtype knobs
USE_BF16_X = True           # x_full table + scatter matmuls in bf16
USE_F32R_MLP = False         # MLP + pooling matmuls in float32r


# ---------------- host preprocessing ----------------

def _pad_id(n):
    """node id -> padded global row id"""
    return (n // NPC) * M + (n % NPC)


def _preprocess(x, edge_index, batch):
    src = np.asarray(edge_index[0], dtype=np.int64)
    dst = np.asarray(edge_index[1], dtype=np.int64)
    batch = np.asarray(batch, dtype=np.int64)

    src_pad = _pad_id(src)
    core = dst // NPC
    dstl = dst - core * NPC            # local dst 0..12499
    win = dstl // 128
    dstrel_all = dstl % 128
    buck = src_pad // BUCK

    # per (core, window, bucket) edge lists
    per = [[[None] * NBUCK for _ in range(NW)] for _ in range(NC)]
    for c in range(NC):
        m = core == c
        sp, dr, w, b = src_pad[m], dstrel_all[m], win[m], buck[m]
        key = w * NBUCK + b
        order = np.argsort(key, kind="stable")
        sp, dr, w, b, key = sp[order], dr[order], w[order], b[order], key[order]
        bounds = np.searchsorted(key, np.arange(NW * NBUCK + 1))
        for wi in range(NW):
            for bi in range(NBUCK):
                lo, hi = bounds[wi * NBUCK + bi], bounds[wi * NBUCK + bi + 1]
                per[c][wi][bi] = (sp[lo:hi], dr[lo:hi])

    # uniform tiles-per-(window,bucket): max over cores
    T = np.zeros((NW, NBUCK), np.int64)
    for wi in range(NW):
        for bi in range(NBUCK):
            cnt = max(len(per[c][wi][bi][0]) for c in range(NC))
            T[wi, bi] = max(1, math.ceil(cnt / 128))

    # gather calls: for each window-group, per bucket one call covering
    # tiles {(w, b, k) : w in group}; slots are call-contiguous.
    wgroups = [list(range(g, min(g + WG, NW))) for g in range(0, NW, WG)]
    CALL_MAX = 8        # <=1024 indices per dma_gather (SWDGE ring limit)
    calls = []          # (bucket, ntiles)
    wg_calls = []       # per wgroup: list of call ids
    tile_slot = {}      # (w, b, k) -> (call_id, pos_in_call)
    ntiles_total = 0
    for grp in wgroups:
        ids = []
        for bi in range(NBUCK):
            pend = []   # (wi, k) pending tiles for this bucket
            for wi in grp:
                for k in range(int(T[wi, bi])):
                    pend.append((wi, k))
            for c0 in range(0, len(pend), CALL_MAX):
                chunk = pend[c0:c0 + CALL_MAX]
                cid = len(calls)
                ids.append(cid)
                for pos, (wi, k) in enumerate(chunk):
                    tile_slot[(wi, bi, k)] = (cid, pos)
                calls.append((bi, len(chunk)))
                ntiles_total += len(chunk)
        wg_calls.append(ids)
    call_start = np.cumsum([0] + [c[1] for c in calls])
    callt_max = max(c[1] for c in calls)

    # per-core uploads: idx16 + dstrel in slot order
    idx16 = np.zeros((NC, 128, ntiles_total * 8), np.int16)
    dstrel_up = np.full((NC, 128, ntiles_total), -1.0, np.float32)
    for c in range(NC):
        idx_flat = np.zeros(ntiles_total * 128, np.int16)
        dr_flat = np.full(ntiles_total * 128, -1.0, np.float32)
        for wi in range(NW):
            for bi in range(NBUCK):
                sp, dr = per[c][wi][bi]
                loc = (sp - bi * BUCK).astype(np.int16)
                for k in range(int(T[wi, bi])):
                    cid, pos = tile_slot[(wi, bi, k)]
                    s0 = (call_start[cid] + pos) * 128
                    seg = slice(k * 128, min((k + 1) * 128, len(sp)))
                    n = seg.stop - seg.start
                    if n > 0:
                        idx_flat[s0:s0 + n] = loc[seg]
                        dr_flat[s0:s0 + n] = dr[seg].astype(np.float32)
                    # pad slots keep idx 0 (valid) and dstrel -1
        # wrap: unwrapped[k] = packed[k % 16, k // 16]; replicate 8x rows
        packed = idx_flat.reshape(ntiles_total * 8, 16).T
        idx16[c] = np.tile(packed, (8, 1))
        dstrel_up[c] = dr_flat.reshape(ntiles_total, 128).T

    # pooling metadata
    g_lo = np.zeros(NC, np.int64)
    batchrel = np.full((NC, 128, NW), -1.0, np.float32)
    sel = np.full((NC, 128, NW), -1.0, np.float32)
    mask = np.ones((NC, 128, M), np.float32)
    for c in range(NC):
        bl = batch[c * NPC:(c + 1) * NPC]
        g_lo[c] = bl[0]
        rel = (bl - g_lo[c]).astype(np.float32)
        assert rel.max() < 128, "per-core graph span exceeds 128"
        br = np.full(M, -1.0, np.float32)
        br[:NPC] = rel
        batchrel[c] = br.reshape(NW, 128).T
        # last-node-of-graph flags (within core)
        last = np.zeros(NPC, bool)
        last[-1] = True
        last[:-1] = bl[1:] != bl[:-1]
        sv = np.full(M, -1.0, np.float32)
        sv[:NPC][last] = rel[last]
        sel[c] = sv.reshape(NW, 128).T
        first = np.zeros(NPC, bool)
        first[0] = True
        first[1:] = bl[1:] != bl[:-1]
        mk = np.ones(M, np.float32)
        mk[:NPC][first] = 0.0
        mask[c] = np.broadcast_to(mk, (128, M))

    meta = dict(calls=calls, call_start=call_start, tile_slot=tile_slot,
                T=T, wgroups=wgroups, wg_calls=wg_calls,
                ntiles_total=ntiles_total, callt_max=callt_max, g_lo=g_lo,
                max_calls_grp=max(len(ids) for ids in wg_calls))
    return meta, idx16, dstrel_up, batchrel, sel, mask


# ---------------- device program ----------------

def _build_program(meta):
    import concourse.bass as bass
    import concourse.bacc as bacc
    import concourse.tile as tile
    import concourse.mybir as mybir
    from concourse.masks import make_identity

    F32 = mybir.dt.float32
    F32R = mybir.dt.float32r
    BF16 = mybir.dt.bfloat16
    I16 = mybir.dt.int16
    AF = mybir.ActivationFunctionType
    OP = mybir.AluOpType

    DT_X = BF16 if USE_BF16_X else F32
    calls = meta["calls"]
    call_start = meta["call_start"]
    tile_slot = meta["tile_slot"]
    T = meta["T"]
    wgroups = meta["wgroups"]
    wg_calls = meta["wg_calls"]
    NT = meta["ntiles_total"]
    CALLT = meta["callt_max"]

    nc = bacc.Bacc("TRN2", target_bir_lowering=False, debug=False,
                   num_devices=NC)

    ein = lambda n, s: nc.dram_tensor(n, s, F32, kind="ExternalInput")
    xT0_d = ein("xT0", [128, M])
    w1_d = ein("w1u", [128, L_LAYERS, 2, 128])
    w2_d = ein("w2u", [128, L_LAYERS, 2, 128])
    b1_d = ein("b1u", [128, L_LAYERS, 2])
    g1_d = ein("g1u", [128, L_LAYERS, 2])
    be1_d = ein("be1u", [128, L_LAYERS, 2])
    b2_d = ein("b2u", [128, L_LAYERS])
    gbn_d = ein("gbnu", [128, L_LAYERS])
    bbn_d = ein("bbnu", [128, L_LAYERS])
    eps1_d = ein("eps1u", [128, L_LAYERS])
    dstrel_d = ein("dstrelu", [128, NT])
    batchrel_d = ein("batchrelu", [128, NW])
    sel_d = ein("selu", [128, NW])
    mask_d = ein("masku", [128, M])
    idx_d = nc.dram_tensor("idx16u", [128, NT * 8], I16, kind="ExternalInput")

    s_out_d = nc.dram_tensor("s_out", [128, 128], F32, kind="ExternalOutput")
    mx_out_d = nc.dram_tensor("mx_out", [128, 128], F32, kind="ExternalOutput")

    def mmdt(ap):
        return ap.bitcast(F32R) if USE_F32R_MLP else ap

    with tile.TileContext(nc) as tc:
        with (
            tc.tile_pool(name="persist", bufs=1) as pp,
            tc.tile_pool(name="dram", bufs=1, space="DRAM") as dpool,
        ):
            # ---- persistent SBUF ----
            xT = pp.tile([128, M], F32, name="bufA")
            hT = pp.tile([128, M], F32, name="bufB")
            w1_sb = pp.tile([128, L_LAYERS, 2, 128], F32)
            w2_sb = pp.tile([128, L_LAYERS, 2, 128], F32)
            b1_sb = pp.tile([128, L_LAYERS, 2], F32)
            g1_sb = pp.tile([128, L_LAYERS, 2], F32)
            be1_sb = pp.tile([128, L_LAYERS, 2], F32)
            b2_sb = pp.tile([128, L_LAYERS], F32)
            gbn_sb = pp.tile([128, L_LAYERS], F32)
            bbn_sb = pp.tile([128, L_LAYERS], F32)
            eps1_sb = pp.tile([128, L_LAYERS], F32)
            dstrel_sb = pp.tile([128, NT], F32)
            idx_sb = pp.tile([128, NT * 8], I16)
            iota_sb = pp.tile([128, 128], F32)
            ident = pp.tile([128, 128], F32)

            for sb_t, d_t in [(xT, xT0_d), (w1_sb, w1_d), (w2_sb, w2_d),
                              (b1_sb, b1_d), (g1_sb, g1_d), (be1_sb, be1_d),
                              (b2_sb, b2_d), (gbn_sb, gbn_d), (bbn_sb, bbn_d),
                              (eps1_sb, eps1_d), (dstrel_sb, dstrel_d),
                              (idx_sb, idx_d)]:
                nc.sync.dma_start(out=sb_t[:], in_=d_t[:])

            epsc = pp.tile([128, 1], F32)
            nc.vector.memset(epsc[:], BN_EPS)
            iota_i = pp.tile([128, 128], mybir.dt.int32)
            nc.gpsimd.iota(iota_i[:], pattern=[[1, 128]], base=0,
                           channel_multiplier=0)
            nc.vector.tensor_copy(out=iota_sb[:], in_=iota_i[:])
            make_identity(nc, ident[:])

            # ---- DRAM scratch ----
            x_fulls = [dpool.tile([NC * M, D], DT_X, addr_space="Shared",
                                  name=f"x_full{i}") for i in range(L_LAYERS)]
            xsh = dpool.tile([M, D], DT_X, name="xsh")
            ar_ins = [dpool.tile([128, 4], F32, name=f"ar_in{i}")
                      for i in range(2 * L_LAYERS)]
            ar_outs = [dpool.tile([128, 4], F32, name=f"ar_out{i}")
                       for i in range(2 * L_LAYERS)]
            ar_count = [0]
            xsh_v = xsh[:].rearrange("(t p) f -> p t f", p=128)

            def share(src_T, x_full):
                """transpose src_T -> row-major DT_X shard -> AllGather."""
                with (
                    tc.tile_pool(name="shps", bufs=2, space="PSUM") as sp_ps,
                    tc.tile_pool(name="shsb", bufs=3) as sp_sb,
                ):
                    for g in range(NW // 7):
                        xrow = sp_sb.tile([128, 7, 128], DT_X, tag="xrow")
                        for i in range(7):
                            ch = g * 7 + i
                            tp = sp_ps.tile([128, 128], F32, tag="tp")
                            nc.tensor.transpose(
                                out=tp[:], in_=src_T[:, ch * 128:(ch + 1) * 128],
                                identity=ident[:])
                            nc.vector.tensor_copy(out=xrow[:, i, :], in_=tp[:])
                        nc.sync.dma_start(out=xsh_v[:, g * 7:(g + 1) * 7, :],
                                          in_=xrow[:])
                nc.gpsimd.collective_compute(
                    "AllGather", OP.bypass,
                    replica_groups=[list(range(NC))],
                    ins=[xsh[:].opt()], outs=[x_full[:].opt()])

            def allreduce_stats(stats_sb):
                i = ar_count[0]
                ar_count[0] += 1
                nc.sync.dma_start(out=ar_ins[i][:], in_=stats_sb[:])
                nc.gpsimd.collective_compute(
                    "AllReduce", OP.add, replica_groups=[list(range(NC))],
                    ins=[ar_ins[i][:].opt()], outs=[ar_outs[i][:].opt()])
                glob = stat_pool.tile([128, 4], F32, tag="glob")
                nc.sync.dma_start(out=glob[:], in_=ar_outs[i][:])
                return glob

            col_groups = [(g, min(g + GCOL, M) - g)
                          for g in range(0, M, GCOL)]
            NGRP = len(col_groups)

            with tc.tile_pool(name="stats", bufs=2) as stat_pool:
                xT_cur, hT_cur = xT, hT
                share(xT, x_fulls[0])  # x_full <- layer-0 input

                for layer in range(L_LAYERS):
                    # ======== scatter: agg -> hT ========
                    with (
                        tc.tile_pool(name="gb", bufs=meta["max_calls_grp"] + 4) as gb_pool,
                        tc.tile_pool(name="oh", bufs=4) as oh_pool,
                        tc.tile_pool(name="aggps", bufs=3, space="PSUM") as agg_ps,
                    ):
                        gbufs = {}
                        for gi, grp in enumerate(wgroups):
                            for cid in wg_calls[gi]:
                                b_, ntl = calls[cid]
                                gb = gb_pool.tile([128, CALLT, 128], DT_X,
                                                  tag="gb", name="gb")
                                nc.gpsimd.dma_gather(
                                    out_ap=gb[:, :ntl, :],
                                    in_ap=x_fulls[layer][b_ * BUCK:(b_ + 1) * BUCK, :],
                                    idxs_ap=idx_sb[:16, call_start[cid] * 8:
                                                   (call_start[cid] + ntl) * 8],
                                    num_idxs=ntl * 128,
                                    num_idxs_reg=ntl * 128,
                                    elem_size=D)
                                gbufs[cid] = gb
                            for wi in grp:
                                ps = agg_ps.tile([128, 128], F32, tag="agg")
                                nmm = 0
                                tot = int(T[wi].sum())
                                for bi in range(NBUCK):
                                    for k in range(int(T[wi, bi])):
                                        cid, pos = tile_slot[(wi, bi, k)]
                                        slot = call_start[cid] + pos
                                        oh = oh_pool.tile([128, 128], DT_X,
                                                          tag="oh")
                                        nc.vector.tensor_scalar(
                                            out=oh[:], in0=iota_sb[:],
                                            scalar1=dstrel_sb[:, slot:slot + 1],
                                            scalar2=None, op0=OP.is_equal)
                                        nc.tensor.matmul(
                                            out=ps[:],
                                            lhsT=gbufs[cid][:, pos, :],
                                            rhs=oh[:],
                                            start=(nmm == 0),
                                            stop=(nmm == tot - 1))
                                        nmm += 1
                                # h = (1+eps)*x + agg
                                cs = wi * 128
                                nc.vector.scalar_tensor_tensor(
                                    out=hT_cur[:, cs:cs + 128],
                                    in0=xT_cur[:, cs:cs + 128],
                                    scalar=eps1_sb[:, layer:layer + 1],
                                    in1=ps[:], op0=OP.mult, op1=OP.add)

                    if DEBUG_STAGE == 1 and layer == 0:
                        break
                    # ======== MLP pass A: BN1 stats ========
                    with (
                        tc.tile_pool(name="zps", bufs=2, space="PSUM") as z_ps,
                        tc.tile_pool(name="msb", bufs=3) as m_sb,
                    ):
                        sum_p = stat_pool.tile([128, 2, NGRP], F32, tag="sum1")
                        sq_p = stat_pool.tile([128, 2, NGRP], F32, tag="sq1")
                        for gi, (c0, cn) in enumerate(col_groups):
                            for b in range(2):
                                zp = z_ps.tile([128, GCOL], F32, tag="z")
                                nc.tensor.matmul(
                                    out=zp[:, :cn],
                                    lhsT=mmdt(w1_sb[:, layer, b, :]),
                                    rhs=mmdt(hT_cur[:, c0:c0 + cn]),
                                    start=True, stop=True)
                                zs = m_sb.tile([128, GCOL], F32, tag="zs")
                                nc.scalar.activation(
                                    out=zs[:, :cn], in_=zp[:, :cn],
                                    func=AF.Identity,
                                    bias=b1_sb[:, layer, b:b + 1],
                                    accum_out=sum_p[:, b, gi:gi + 1])
                                sq = m_sb.tile([128, GCOL], F32, tag="sq")
                                nc.vector.scalar_tensor_tensor(
                                    out=sq[:, :cn], in0=zs[:, :cn], scalar=1.0,
                                    in1=zs[:, :cn], op0=OP.mult, op1=OP.mult,
                                    accum_out=sq_p[:, b, gi:gi + 1])
                        st = stat_pool.tile([128, 4], F32, tag="pack")
                        nc.vector.tensor_reduce(
                            out=st[:, 0:2], in_=sum_p[:], axis=mybir.AxisListType.X,
                            op=OP.add)
                        nc.vector.tensor_reduce(
                            out=st[:, 2:4], in_=sq_p[:], axis=mybir.AxisListType.X,
                            op=OP.add)
                        glob = allreduce_stats(st)
                        # A = g1*rsqrt(var+eps); B = (b1-mean)*A + be1
                        mean = stat_pool.tile([128, 2], F32, tag="mean")
                        nc.vector.tensor_scalar(
                            out=mean[:], in0=glob[:, 0:2], scalar1=1.0 / N_NODES,
                            scalar2=None, op0=OP.mult)
                        var = stat_pool.tile([128, 2], F32, tag="var")
                        nc.vector.tensor_scalar(
                            out=var[:], in0=glob[:, 2:4], scalar1=1.0 / N_NODES,
                            scalar2=None, op0=OP.mult)
                        msq = stat_pool.tile([128, 2], F32, tag="msq")
                        nc.vector.tensor_tensor(
                            out=msq[:], in0=mean[:], in1=mean[:], op=OP.mult)
                        nc.vector.tensor_tensor(
                            out=var[:], in0=var[:], in1=msq[:], op=OP.subtract)
                        sd = stat_pool.tile([128, 2], F32, tag="sd")
                        nc.scalar.activation(out=sd[:], in_=var[:], func=AF.Sqrt,
                                             bias=epsc[:, 0:1])
                        rs = stat_pool.tile([128, 2], F32, tag="rs")
                        nc.vector.reciprocal(out=rs[:], in_=sd[:])
                        A1 = stat_pool.tile([128, 2], F32, tag="A1")
                        nc.vector.tensor_tensor(
                            out=A1[:], in0=rs[:], in1=g1_sb[:, layer, :],
                            op=OP.mult)
                        B1 = stat_pool.tile([128, 2], F32, tag="B1")
                        nc.vector.tensor_tensor(
                            out=B1[:], in0=b1_sb[:, layer, :], in1=mean[:],
                            op=OP.subtract)
                        nc.vector.tensor_tensor(
                            out=B1[:], in0=B1[:], in1=A1[:], op=OP.mult)
                        nc.vector.tensor_tensor(
                            out=B1[:], in0=B1[:], in1=be1_sb[:, layer, :],
                            op=OP.add)

                    # ======== MLP pass B: recompute z1, relu, h2, BN2 stats ====
                    with (
                        tc.tile_pool(name="zps2", bufs=2, space="PSUM") as z_ps,
                        tc.tile_pool(name="h2ps", bufs=2, space="PSUM") as h2_ps,
                        tc.tile_pool(name="msb2", bufs=3) as m_sb,
                    ):
                        sum2 = stat_pool.tile([128, NGRP], F32, tag="sum2")
                        sq2 = stat_pool.tile([128, NGRP], F32, tag="sq2")
                        for gi, (c0, cn) in enumerate(col_groups):
                            zr = [None, None]
                            for b in range(2):
                                zp = z_ps.tile([128, GCOL], F32, tag="z2")
                                nc.tensor.matmul(
                                    out=zp[:, :cn],
                                    lhsT=mmdt(w1_sb[:, layer, b, :]),
                                    rhs=mmdt(hT_cur[:, c0:c0 + cn]),
                                    start=True, stop=True)
                                zr[b] = m_sb.tile([128, GCOL], F32, tag=f"zr{b}", name=f"zr{b}")
                                nc.scalar.activation(
                                    out=zr[b][:, :cn], in_=zp[:, :cn],
                                    func=AF.Relu,
                                    bias=B1[:, b:b + 1],
                                    scale=A1[:, b:b + 1])
                            hp = h2_ps.tile([128, GCOL], F32, tag="h2")
                            for b in range(2):
                                nc.tensor.matmul(
                                    out=hp[:, :cn],
                                    lhsT=mmdt(w2_sb[:, layer, b, :]),
                                    rhs=mmdt(zr[b][:, :cn]),
                                    start=(b == 0), stop=(b == 1))
                            nc.scalar.activation(
                                out=xT_cur[:, c0:c0 + cn],
                                in_=hp[:, :cn], func=AF.Identity,
                                bias=b2_sb[:, layer:layer + 1],
                                accum_out=sum2[:, gi:gi + 1])
                            sq = m_sb.tile([128, GCOL], F32, tag="sqb")
                            nc.vector.scalar_tensor_tensor(
                                out=sq[:, :cn], in0=xT_cur[:, c0:c0 + cn],
                                scalar=1.0, in1=xT_cur[:, c0:c0 + cn],
                                op0=OP.mult, op1=OP.mult,
                                accum_out=sq2[:, gi:gi + 1])
                        st = stat_pool.tile([128, 4], F32, tag="pack")
                        nc.vector.tensor_reduce(
                            out=st[:, 0:1], in_=sum2[:], axis=mybir.AxisListType.X,
                            op=OP.add)
                        nc.vector.tensor_reduce(
                            out=st[:, 2:3], in_=sq2[:], axis=mybir.AxisListType.X,
                            op=OP.add)
                        nc.vector.memset(st[:, 1:2], 0.0)
                        nc.vector.memset(st[:, 3:4], 0.0)
                        glob = allreduce_stats(st)
                        mean = stat_pool.tile([128, 1], F32, tag="mean2")
                        nc.vector.tensor_scalar(
                            out=mean[:], in0=glob[:, 0:1], scalar1=1.0 / N_NODES,
                            scalar2=None, op0=OP.mult)
                        var = stat_pool.tile([128, 1], F32, tag="var2")
                        nc.vector.tensor_scalar(
                            out=var[:], in0=glob[:, 2:3], scalar1=1.0 / N_NODES,
                            scalar2=None, op0=OP.mult)
                        msq = stat_pool.tile([128, 1], F32, tag="msq2")
                        nc.vector.tensor_tensor(
                            out=msq[:], in0=mean[:], in1=mean[:], op=OP.mult)
                        nc.vector.tensor_tensor(
                            out=var[:], in0=var[:], in1=msq[:], op=OP.subtract)
                        sd = stat_pool.tile([128, 1], F32, tag="sd2")
                        nc.scalar.activation(out=sd[:], in_=var[:], func=AF.Sqrt,
                                             bias=epsc[:, 0:1])
                        rs = stat_pool.tile([128, 1], F32, tag="rs2")
                        nc.vector.reciprocal(out=rs[:], in_=sd[:])
                        A2 = stat_pool.tile([128, 1], F32, tag="A2")
                        nc.vector.tensor_tensor(
                            out=A2[:], in0=rs[:], in1=gbn_sb[:, layer:layer + 1],
                            op=OP.mult)
                        B2 = stat_pool.tile([128, 1], F32, tag="B2")
                        nc.vector.tensor_tensor(
                            out=B2[:], in0=mean[:], in1=A2[:], op=OP.mult)
                        nc.vector.tensor_tensor(
                            out=B2[:], in0=bbn_sb[:, layer:layer + 1], in1=B2[:],
                            op=OP.subtract)
                        # x_new = relu(h2*A2 + B2) in place on xT_cur (h2 lives
                        # there, pre-BN, with b2 added; BN input = h2+b2 and
                        # B2 accounts for it)
                        nc.scalar.activation(
                            out=xT_cur[:, :], in_=xT_cur[:, :], func=AF.Relu,
                            bias=B2[:, 0:1], scale=A2[:, 0:1])
                        nc.vector.memset(xT_cur[:, NPC:M], 0.0)

                    # swap: new x is in xT_cur's buffer already (we wrote h2 and
                    # BN into xT_cur); hT_cur holds the dead h. Keep roles.
                    if layer < L_LAYERS - 1:
                        share(xT_cur, x_fulls[layer + 1])

                if DEBUG_STAGE == 1:
                    with tc.tile_pool(name="dbg", bufs=1) as dbg:
                        t1 = dbg.tile([128, 128], F32)
                        nc.vector.tensor_copy(out=t1[:], in_=hT[:, 0:128])
                        nc.sync.dma_start(out=s_out_d[:, :], in_=t1[:])
                        t2 = dbg.tile([128, 128], F32)
                        nc.vector.tensor_copy(out=t2[:], in_=hT[:, 128:256])
                        nc.sync.dma_start(out=mx_out_d[:, :], in_=t2[:])
                if DEBUG_STAGE != 1:
                    # ======== pooling ========
                    with (
                        tc.tile_pool(name="poolps", bufs=2, space="PSUM") as tp_ps,
                        tc.tile_pool(name="accps", bufs=1, space="PSUM") as acc_ps,
                        tc.tile_pool(name="poolsb", bufs=4) as po_sb,
                    ):
                        mask_sb = po_sb.tile([128, M], F32, name="mask_sb", bufs=1)
                        nc.sync.dma_start(out=mask_sb[:], in_=mask_d[:])
                        batchrel_sb = po_sb.tile([128, NW], F32, name="brl", bufs=1)
                        nc.sync.dma_start(out=batchrel_sb[:], in_=batchrel_d[:])
                        sel_sb = po_sb.tile([128, NW], F32, name="slb", bufs=1)
                        nc.sync.dma_start(out=sel_sb[:], in_=sel_d[:])

                        scan = hT_cur  # reuse dead h buffer
                        nc.vector.tensor_tensor_scan(
                            out=scan[:], data0=mask_sb[:], data1=xT_cur[:],
                            initial=0.0, op0=OP.mult, op1=OP.max)

                        s_ps = acc_ps.tile([128, 128], F32, tag="sacc")
                        m_ps = acc_ps.tile([128, 128], F32, tag="macc")
                        for ch in range(NW):
                            cs = ch * 128
                            tp1 = tp_ps.tile([128, 128], F32, tag="tp1")
                            nc.tensor.transpose(out=tp1[:], in_=xT_cur[:, cs:cs + 128],
                                                identity=ident[:])
                            xrow = po_sb.tile([128, 128], F32, tag="xr")
                            nc.vector.tensor_copy(out=xrow[:], in_=tp1[:])
                            tp2 = tp_ps.tile([128, 128], F32, tag="tp2")
                            nc.tensor.transpose(out=tp2[:], in_=scan[:, cs:cs + 128],
                                                identity=ident[:])
                            srow = po_sb.tile([128, 128], F32, tag="sr")
                            nc.vector.tensor_copy(out=srow[:], in_=tp2[:])
                            ohB = po_sb.tile([128, 128], F32, tag="ohB")
                            nc.vector.tensor_scalar(
                                out=ohB[:], in0=iota_sb[:],
                                scalar1=batchrel_sb[:, ch:ch + 1], scalar2=None,
                                op0=OP.is_equal)
                            ohS = po_sb.tile([128, 128], F32, tag="ohS")
                            nc.vector.tensor_scalar(
                                out=ohS[:], in0=iota_sb[:],
                                scalar1=sel_sb[:, ch:ch + 1], scalar2=None,
                                op0=OP.is_equal)
                            nc.tensor.matmul(out=s_ps[:], lhsT=mmdt(xrow[:]),
                                             rhs=mmdt(ohB[:]), start=(ch == 0),
                                             stop=(ch == NW - 1))
                            nc.tensor.matmul(out=m_ps[:], lhsT=mmdt(srow[:]),
                                             rhs=mmdt(ohS[:]), start=(ch == 0),
                                             stop=(ch == NW - 1))
                        s_sb = po_sb.tile([128, 128], F32, tag="sfin")
                        nc.vector.tensor_copy(out=s_sb[:], in_=s_ps[:])
                        m_sb2 = po_sb.tile([128, 128], F32, tag="mfin")
                        nc.vector.tensor_copy(out=m_sb2[:], in_=m_ps[:])
                        nc.sync.dma_start(out=s_out_d[:, :], in_=s_sb[:])
                        nc.sync.dma_start(out=mx_out_d[:, :], in_=m_sb2[:])

    nc.compile()
    return nc


# ---------------- host postprocessing (classifier) ----------------

def _postprocess(results, g_lo, batch, cW1, cb1, cg, cbeta, cW2, cb2, cW3, cb3):
    s_g = np.zeros((G_GRAPHS, D), np.float64)
    mx_g = np.zeros((G_GRAPHS, D), np.float64)
    for c in range(NC):
        s_part = results[c]["s_out"].astype(np.float64).T    # [128 relg, 128 f]
        m_part = results[c]["mx_out"].astype(np.float64).T
        g0 = int(g_lo[c])
        hi = min(128, G_GRAPHS - g0)
        s_g[g0:g0 + hi] += s_part[:hi]
        mx_g[g0:g0 + hi] = np.maximum(mx_g[g0:g0 + hi], m_part[:hi])
    cnt = np.bincount(np.asarray(batch, np.int64), minlength=G_GRAPHS).astype(
        np.float64)
    mean = s_g / np.maximum(cnt, 1.0)[:, None]
    z = np.concatenate([s_g, mean, mx_g], axis=-1).astype(np.float32)

    def bn(v, g, b):
        m = v.mean(0)
        var = v.var(0)
        return (v - m) / np.sqrt(var + BN_EPS) * g + b

    z = np.maximum(bn(z @ cW1 + cb1, cg, cbeta), 0.0)
    z = np.maximum(z @ cW2 + cb2, 0.0)
    return (z @ cW3 + cb3).astype(np.float32)


# ---------------- input packing ----------------

def _pack_inputs(x, batch, W1, b1, g1, be1, W2, b2, gbn, bbn, eps,
                 idx16, dstrel_up, batchrel, sel, mask):
    x = np.asarray(x, np.float32)
    rep = lambda a: np.broadcast_to(np.asarray(a, np.float32), (128,) + np.asarray(a).shape).copy()
    w1u = np.ascontiguousarray(
        np.asarray(W1, np.float32).reshape(L_LAYERS, 128, 2, 128)
        .transpose(1, 0, 2, 3))
    w2u = np.ascontiguousarray(
        np.asarray(W2, np.float32).reshape(L_LAYERS, 2, 128, 128)
        .transpose(2, 0, 1, 3))
    b1u = np.ascontiguousarray(
        np.asarray(b1, np.float32).reshape(L_LAYERS, 2, 128).transpose(2, 0, 1))
    g1u = np.ascontiguousarray(
        np.asarray(g1, np.float32).reshape(L_LAYERS, 2, 128).transpose(2, 0, 1))
    be1u = np.ascontiguousarray(
        np.asarray(be1, np.float32).reshape(L_LAYERS, 2, 128).transpose(2, 0, 1))
    b2u = np.ascontiguousarray(np.asarray(b2, np.float32).T)
    gbnu = np.ascontiguousarray(np.asarray(gbn, np.float32).T)
    bbnu = np.ascontiguousarray(np.asarray(bbn, np.float32).T)
    eps1u = rep(1.0 + np.asarray(eps, np.float32))

    in_maps = []
    for c in range(NC):
        xs = np.zeros((128, M), np.float32)
        xs[:, :NPC] = x[c * NPC:(c + 1) * NPC].T
        in_maps.append({
            "xT0": xs,
            "w1u": w1u, "w2u": w2u, "b1u": b1u, "g1u": g1u, "be1u": be1u,
            "b2u": b2u, "gbnu": gbnu, "bbnu": bbnu, "eps1u": eps1u,
            "dstrelu": dstrel_up[c], "batchrelu": batchrel[c], "selu": sel[c],
            "masku": mask[c], "idx16u": idx16[c],
        })
    return in_maps


# ---------------- main entry ----------------


LAST_EXEC_NS = None
LAST_RESULTS = None


def _timed_spmd(nc, in_maps, n_reps=3):
    """Mirror bass2jax.run_bass_via_pjrt but keep the executable and time
    repeated calls with device-resident inputs. Returns (results, best_ns)."""
    import jax
    import numpy as _np
    from jax.sharding import Mesh, PartitionSpec, NamedSharding
    from jax.experimental.shard_map import shard_map
    import concourse.mybir as mybir
    from concourse import bass2jax

    bass2jax.install_neuronx_cc_hook()
    n_cores = len(in_maps)
    partition_name = (nc.partition_id_tensor.name
                      if nc.partition_id_tensor else None)
    in_names, out_names, out_avals, zero_outs = [], [], [], []
    for alloc in nc.m.functions[0].allocations:
        if not isinstance(alloc, mybir.MemoryLocationSet):
            continue
        name = alloc.memorylocations[0].name
        if alloc.kind == "ExternalInput":
            if name != partition_name:
                in_names.append(name)
        elif alloc.kind == "ExternalOutput":
            out_names.append(name)
            shape = tuple(alloc.tensor_shape)
            dtype = mybir.dt.np(alloc.dtype)
            out_avals.append(jax.core.ShapedArray(shape, dtype))
            zero_outs.append(_np.zeros(shape, dtype))
    n_params = len(in_names)
    n_outs = len(out_avals)
    all_in_names = list(in_names) + out_names
    if partition_name is not None:
        all_in_names.append(partition_name)

    def _body(*args):
        operands = list(args)
        if partition_name is not None:
            operands.append(bass2jax.partition_id_tensor())
        outs = bass2jax._bass_exec_p.bind(
            *operands, out_avals=tuple(out_avals),
            in_names=tuple(all_in_names), out_names=tuple(out_names),
            lowering_input_output_aliases=(),
            sim_require_finite=True, sim_require_nnan=True, nc=nc)
        return tuple(outs)

    devices = jax.devices()[:n_cores]
    mesh = Mesh(_np.asarray(devices), ("core",))
    donate = tuple(range(n_params, n_params + n_outs))
    sharded = jax.jit(
        shard_map(_body, mesh=mesh,
                  in_specs=(PartitionSpec("core"),) * (n_params + n_outs),
                  out_specs=(PartitionSpec("core"),) * n_outs,
                  check_rep=False),
        donate_argnums=donate, keep_unused=True)
    sh = NamedSharding(mesh, PartitionSpec("core"))
    concat_in = [
        jax.device_put(_np.concatenate(
            [_np.asarray(in_maps[c][nm]) for c in range(n_cores)], axis=0), sh)
        for nm in in_names]
    times = []
    out_arrs = None
    for _ in range(n_reps):
        zeros = [_np.zeros((n_cores * z.shape[0], *z.shape[1:]), z.dtype)
                 for z in zero_outs]
        t0 = time.perf_counter()
        out_arrs = sharded(*concat_in, *zeros)
        jax.block_until_ready(out_arrs)
        times.append(time.perf_counter() - t0)
    best_ns = int(min(times) * 1e9)
    results = [
        {nm: _np.asarray(out_arrs[i]).reshape(n_cores, *out_avals[i].shape)[c]
         for i, nm in enumerate(out_names)}
        for c in range(n_cores)]
    return results, best_ns, times

_CACHE = {}


def kernel(x, edge_index, batch, num_graphs, W1, b1, g1, be1, W2, b2, gbn,
           bbn, eps, cW1, cb1, cg, cbeta, cW2, cb2, cW3, cb3):
    from concourse.bass_utils import run_bass_kernel_spmd

    x = np.asarray(x)
    meta, idx16, dstrel_up, batchrel, sel, mask = _preprocess(
        x, np.asarray(edge_index), np.asarray(batch))
    in_maps = _pack_inputs(x, batch, W1, b1, g1, be1, W2, b2, gbn, bbn, eps,
                           idx16, dstrel_up, batchrel, sel, mask)
    nc = _build_program(meta)
    global LAST_RESULTS, LAST_EXEC_NS
    try:
        results, best_ns, times = _timed_spmd(nc, in_maps)
        LAST_EXEC_NS = best_ns
        print("timed calls (s):", [f"{t:.4f}" for t in times])
        LAST_RESULTS = results
    except Exception as e:
        print("timed runner failed, falling back:", e)
        res = run_bass_kernel_spmd(nc, in_maps, core_ids=list(range(NC)))
        LAST_RESULTS = res.results
    return _postprocess(LAST_RESULTS, meta["g_lo"], batch,
                        np.asarray(cW1, np.float32), np.asarray(cb1, np.float32),
                        np.asarray(cg, np.float32), np.asarray(cbeta, np.float32),
                        np.asarray(cW2, np.float32), np.asarray(cb2, np.float32),
                        np.asarray(cW3, np.float32), np.asarray(cb3, np.float32))



# revision 10
# speedup vs baseline: 28.7982x; 28.7982x over previous
"""Trainium2 Bass kernel for BaselineGIN (nn_BaselineGIN_42502996361221).

Strategy (8 NeuronCores, SPMD):
  - Nodes sharded 12500/core, padded to 12544 (=98*128) columns per core.
  - Node features live transposed in SBUF: xT [128 feat, 12544 nodes] fp32.
  - Full node table x_full [100352, 128] bf16 in Shared DRAM, rebuilt by
    AllGather before every layer (bf16 halves gather traffic).
  - Edges partitioned by dst owner; dst space split into 512-wide windows
    (one PSUM bank per window), sources into 4 buckets of 25088 rows
    (int16 bucket-relative indices). Edges sorted by (window, bucket),
    packed into 128-row tiles per (window, bucket) cell, gathered with
    dma_gather round-robined over 4 SWDGE queues, then scatter-added into
    [128 feat, 512 dst] PSUM windows via one-hot matmuls (one-hot built
    on DVE as is_equal(iota512, dstrel), fp16 in / bf16 out).
  - GIN MLP interleaved with the scatter at 512-column granularity;
    matmuls in float32r (1 cycle/row at >=256 free dim). BatchNorm batch
    stats via per-core accumulators + tiny AllReduce; BN1 z recomputed
    after the stats AllReduce so Z1 is never materialized full-width.
  - Readout: segment sums via one-hot(batch) matmuls over PE-transposed
    chunks; segment max via tensor_tensor_scan (mult-mask reset, data>=0)
    plus one-hot(last-node) extraction matmuls. Host combines the per-core
    partials (straddled graphs) and runs the tiny [512,384] classifier.

kernel() accepts the FULL inputs and returns the FULL [512, 2] output.
LAST_EXEC_NS is the on-device NEFF execution time measured via NRT/NTFF
profiling (neuron-profile) when available; wall-clock min otherwise.
"""

import contextlib
import ctypes
import math
import os
import sys
import time

import numpy as np

# ---------------- problem constants (hardcoded per spec) ----------------
N_NODES = 100000
N_EDGES = 500000
D = 128
G_GRAPHS = 512
L_LAYERS = 4
NC = 8                      # cores
NPC = N_NODES // NC         # real nodes per core = 12500
M = 12544                   # padded per-core columns (98 * 128)
NW = M // 128               # 98 narrow windows (pooling)
W5 = 512                    # scatter dst-window width (one PSUM bank)
NW5 = math.ceil(NPC / W5)   # 25 scatter windows
NBUCK = 4
BUCK = NC * M // NBUCK      # 25088 padded rows per src bucket (< 32768)
BN_EPS = 1e-5
WGRP = 4                    # scatter windows per gather-call group
NQ = 4                      # SWDGE queues (ucode max)
CALL_MAX = 8                # tiles per dma_gather call (1024-desc ring)

USE_BF16_X = True           # x_full table + scatter matmuls in bf16
USE_F32R_MLP = True         # MLP matmuls in float32r (1 cyc/row @ >=256)


# ---------------- host preprocessing ----------------

def _pad_id(n):
    """node id -> padded global row id"""
    return (n // NPC) * M + (n % NPC)


def _preprocess(x, edge_index, batch):
    src = np.asarray(edge_index[0], dtype=np.int64)
    dst = np.asarray(edge_index[1], dtype=np.int64)
    batch = np.asarray(batch, dtype=np.int64)

    src_pad = _pad_id(src)
    core = dst // NPC
    dstl = dst - core * NPC            # local dst 0..12499
    win = dstl // W5
    dstrel_all = dstl % W5
    buck = src_pad // BUCK

    # per (core, window, bucket) edge lists
    per = [[[None] * NBUCK for _ in range(NW5)] for _ in range(NC)]
    for c in range(NC):
        m = core == c
        sp, dr, w, b = src_pad[m], dstrel_all[m], win[m], buck[m]
        key = w * NBUCK + b
        order = np.argsort(key, kind="stable")
        sp, dr, key = sp[order], dr[order], key[order]
        bounds = np.searchsorted(key, np.arange(NW5 * NBUCK + 1))
        for wi in range(NW5):
            for bi in range(NBUCK):
                lo, hi = bounds[wi * NBUCK + bi], bounds[wi * NBUCK + bi + 1]
                per[c][wi][bi] = (sp[lo:hi], dr[lo:hi])

    # uniform tiles-per-(window,bucket): max over cores
    T = np.zeros((NW5, NBUCK), np.int64)
    for wi in range(NW5):
        for bi in range(NBUCK):
            cnt = max(len(per[c][wi][bi][0]) for c in range(NC))
            T[wi, bi] = math.ceil(cnt / 128)

    # gather calls: per window-group, per bucket, calls of <=8 tiles
    wgroups = [list(range(g, min(g + WGRP, NW5))) for g in range(0, NW5, WGRP)]
    calls = []          # (bucket, ntiles)
    wg_calls = []       # per wgroup: list of call ids
    tile_slot = {}      # (w, b, k) -> (call_id, pos_in_call)
    ntiles_total = 0
    for grp in wgroups:
        ids = []
        for bi in range(NBUCK):
            pend = []
            for wi in grp:
                for k in range(int(T[wi, bi])):
                    pend.append((wi, k))
            for c0 in range(0, len(pend), CALL_MAX):
                chunk = pend[c0:c0 + CALL_MAX]
                cid = len(calls)
                ids.append(cid)
                for pos, (wi, k) in enumerate(chunk):
                    tile_slot[(wi, bi, k)] = (cid, pos)
                calls.append((bi, len(chunk)))
                ntiles_total += len(chunk)
        wg_calls.append(ids)
    call_start = np.cumsum([0] + [c[1] for c in calls])

    # per-core uploads: idx16 (wrapped [16, NT*8], replicated to 128 rows)
    # + dstrel (f32) in slot order
    idx16 = np.zeros((NC, 128, ntiles_total * 8), np.int16)
    dstrel_up = np.full((NC, 128, ntiles_total), -1.0, np.float32)
    for c in range(NC):
        idx_flat = np.zeros(ntiles_total * 128, np.int16)
        dr_flat = np.full(ntiles_total * 128, -1.0, np.float32)
        for wi in range(NW5):
            for bi in range(NBUCK):
                sp, dr = per[c][wi][bi]
                loc = (sp - bi * BUCK).astype(np.int16)
                for k in range(int(T[wi, bi])):
                    cid, pos = tile_slot[(wi, bi, k)]
                    s0 = (call_start[cid] + pos) * 128
                    seg = slice(k * 128, min((k + 1) * 128, len(sp)))
                    n = seg.stop - seg.start
                    if n > 0:
                        idx_flat[s0:s0 + n] = loc[seg]
                        dr_flat[s0:s0 + n] = dr[seg].astype(np.float32)
                    # pad slots keep idx 0 (valid) and dstrel -1
        idx16[c] = np.tile(idx_flat.reshape(ntiles_total * 8, 16).T, (8, 1))
        dstrel_up[c] = dr_flat.reshape(ntiles_total, 128).T

    # pooling metadata (128-wide windows)
    g_lo = np.zeros(NC, np.int64)
    batchrel = np.full((NC, 128, NW), -1.0, np.float32)
    sel = np.full((NC, 128, NW), -1.0, np.float32)
    mask = np.ones((NC, 128, M), np.float32)
    for c in range(NC):
        bl = batch[c * NPC:(c + 1) * NPC]
        g_lo[c] = bl[0]
        rel = (bl - g_lo[c]).astype(np.float32)
        assert rel.max() < 128, "per-core graph span exceeds 128"
        br = np.full(M, -1.0, np.float32)
        br[:NPC] = rel
        batchrel[c] = br.reshape(NW, 128).T
        last = np.zeros(NPC, bool)
        last[-1] = True
        last[:-1] = bl[1:] != bl[:-1]
        sv = np.full(M, -1.0, np.float32)
        sv[:NPC][last] = rel[last]
        sel[c] = sv.reshape(NW, 128).T
        first = np.zeros(NPC, bool)
        first[0] = True
        first[1:] = bl[1:] != bl[:-1]
        mk = np.ones(M, np.float32)
        mk[:NPC][first] = 0.0
        mask[c] = np.broadcast_to(mk, (128, M))

    meta = dict(calls=calls, call_start=call_start, tile_slot=tile_slot,
                T=T, wgroups=wgroups, wg_calls=wg_calls,
                ntiles_total=ntiles_total, g_lo=g_lo,
                max_calls_grp=max(len(ids) for ids in wg_calls))
    return meta, idx16, dstrel_up, batchrel, sel, mask


# ---------------- device program ----------------

def _build_program(meta):
    import concourse.bass as bass
    import concourse.bacc as bacc
    import concourse.tile as tile
    import concourse.mybir as mybir
    from concourse.masks import make_identity

    F32 = mybir.dt.float32
    F32R = mybir.dt.float32r
    BF16 = mybir.dt.bfloat16
    FP16 = mybir.dt.float16
    I16 = mybir.dt.int16
    AF = mybir.ActivationFunctionType
    OP = mybir.AluOpType

    DT_X = BF16 if USE_BF16_X else F32
    calls = meta["calls"]
    call_start = meta["call_start"]
    tile_slot = meta["tile_slot"]
    T = meta["T"]
    wgroups = meta["wgroups"]
    wg_calls = meta["wg_calls"]
    NT = meta["ntiles_total"]

    nc = bacc.Bacc("TRN2", target_bir_lowering=False, debug=False,
                   num_devices=NC, num_swdge_queues=NQ)

    ein = lambda n, s: nc.dram_tensor(n, s, F32, kind="ExternalInput")
    xT0_d = ein("xT0", [128, M])
    w1_d = ein("w1u", [128, L_LAYERS, 2, 128])
    w2_d = ein("w2u", [128, L_LAYERS, 2, 128])
    b1_d = ein("b1u", [128, L_LAYERS, 2])
    g1_d = ein("g1u", [128, L_LAYERS, 2])
    be1_d = ein("be1u", [128, L_LAYERS, 2])
    b2_d = ein("b2u", [128, L_LAYERS])
    gbn_d = ein("gbnu", [128, L_LAYERS])
    bbn_d = ein("bbnu", [128, L_LAYERS])
    eps1_d = ein("eps1u", [128, L_LAYERS])
    batchrel_d = ein("batchrelu", [128, NW])
    sel_d = ein("selu", [128, NW])
    mask_d = ein("masku", [128, M])
    dstrel_d = ein("dstrelu", [128, NT])
    idx_d = nc.dram_tensor("idx16u", [128, NT * 8], I16, kind="ExternalInput")

    s_out_d = nc.dram_tensor("s_out", [128, 128], F32, kind="ExternalOutput")
    mx_out_d = nc.dram_tensor("mx_out", [128, 128], F32, kind="ExternalOutput")

    def mmdt(ap):
        return ap.bitcast(F32R) if USE_F32R_MLP else ap

    with tile.TileContext(nc) as tc:
        with (
            tc.tile_pool(name="persist", bufs=1) as pp,
            tc.tile_pool(name="dram", bufs=1, space="DRAM") as dpool,
        ):
            # ---- persistent SBUF ----
            xT = pp.tile([128, M], F32, name="bufA")
            hT = pp.tile([128, M], F32, name="bufB")
            w1_sb = pp.tile([128, L_LAYERS, 2, 128], F32)
            w2_sb = pp.tile([128, L_LAYERS, 2, 128], F32)
            b1_sb = pp.tile([128, L_LAYERS, 2], F32)
            g1_sb = pp.tile([128, L_LAYERS, 2], F32)
            be1_sb = pp.tile([128, L_LAYERS, 2], F32)
            b2_sb = pp.tile([128, L_LAYERS], F32)
            gbn_sb = pp.tile([128, L_LAYERS], F32)
            bbn_sb = pp.tile([128, L_LAYERS], F32)
            eps1_sb = pp.tile([128, L_LAYERS], F32)
            dstrel_sb = pp.tile([128, NT], F32)
            idx_sb = pp.tile([128, NT * 8], I16)
            iota5 = pp.tile([128, W5], F32)
            iota_sb = pp.tile([128, 128], F32)
            ident = pp.tile([128, 128], F32)

            for sb_t, d_t in [(xT, xT0_d), (w1_sb, w1_d), (w2_sb, w2_d),
                              (b1_sb, b1_d), (g1_sb, g1_d), (be1_sb, be1_d),
                              (b2_sb, b2_d), (gbn_sb, gbn_d), (bbn_sb, bbn_d),
                              (eps1_sb, eps1_d), (dstrel_sb, dstrel_d),
                              (idx_sb, idx_d)]:
                nc.sync.dma_start(out=sb_t[:], in_=d_t[:])

            # round weights to f32r once (BIR verifier: f32r matmul inputs
            # must come from rounding producers, DMA loads don't round)
            w1r = pp.tile([128, L_LAYERS, 2, 128], F32)
            nc.vector.tensor_copy(out=mmdt(w1r[:]), in_=w1_sb[:])
            w2r = pp.tile([128, L_LAYERS, 2, 128], F32)
            nc.vector.tensor_copy(out=mmdt(w2r[:]), in_=w2_sb[:])
            epsc = pp.tile([128, 1], F32)
            nc.vector.memset(epsc[:], BN_EPS)
            iota5_i = pp.tile([128, W5], mybir.dt.int32)
            nc.gpsimd.iota(iota5_i[:], pattern=[[1, W5]], base=0,
                           channel_multiplier=0)
            nc.vector.tensor_copy(out=iota5[:], in_=iota5_i[:])
            iota_i = pp.tile([128, 128], mybir.dt.int32)
            nc.gpsimd.iota(iota_i[:], pattern=[[1, 128]], base=0,
                           channel_multiplier=0)
            nc.vector.tensor_copy(out=iota_sb[:], in_=iota_i[:])
            make_identity(nc, ident[:])

            # ---- DRAM scratch ----
            x_fulls = [dpool.tile([NC * M, D], DT_X, addr_space="Shared",
                                  name=f"x_full{i}") for i in range(L_LAYERS)]
            xsh = dpool.tile([M, D], DT_X, name="xsh")
            ar_ins = [dpool.tile([128, 4], F32, name=f"ar_in{i}")
                      for i in range(2 * L_LAYERS)]
            ar_outs = [dpool.tile([128, 4], F32, name=f"ar_out{i}")
                       for i in range(2 * L_LAYERS)]
            ar_count = [0]
            xsh_v = xsh[:].rearrange("(t p) f -> p t f", p=128)

            def share(src_T, x_full):
                """transpose src_T -> row-major DT_X shard -> AllGather."""
                with (
                    tc.tile_pool(name="shps", bufs=2, space="PSUM") as sp_ps,
                    tc.tile_pool(name="shsb", bufs=3) as sp_sb,
                ):
                    for g in range(NW // 7):
                        xrow = sp_sb.tile([128, 7, 128], DT_X, tag="xrow")
                        for i in range(7):
                            ch = g * 7 + i
                            tp = sp_ps.tile([128, 128], F32, tag="tp")
                            nc.tensor.transpose(
                                out=tp[:], in_=src_T[:, ch * 128:(ch + 1) * 128],
                                identity=ident[:])
                            if i % 2 == 0:
                                nc.vector.tensor_copy(out=xrow[:, i, :], in_=tp[:])
                            else:
                                nc.scalar.activation(out=xrow[:, i, :], in_=tp[:],
                                                     func=AF.Copy)
                        nc.sync.dma_start(out=xsh_v[:, g * 7:(g + 1) * 7, :],
                                          in_=xrow[:])
                nc.gpsimd.collective_compute(
                    "AllGather", OP.bypass,
                    replica_groups=[list(range(NC))],
                    ins=[xsh[:].opt()], outs=[x_full[:].opt()])

            def allreduce_stats(stats_sb):
                i = ar_count[0]
                ar_count[0] += 1
                nc.sync.dma_start(out=ar_ins[i][:], in_=stats_sb[:])
                nc.gpsimd.collective_compute(
                    "AllReduce", OP.add, replica_groups=[list(range(NC))],
                    ins=[ar_ins[i][:].opt()], outs=[ar_outs[i][:].opt()])
                glob = stat_pool.tile([128, 4], F32, tag="glob")
                nc.sync.dma_start(out=glob[:], in_=ar_outs[i][:])
                return glob

            with tc.tile_pool(name="stats", bufs=2) as stat_pool:
                share(xT, x_fulls[0])  # x_full <- layer-0 input

                for layer in range(L_LAYERS):
                    # ==== scatter (agg -> hT) + MLP pass A, interleaved ====
                    with (
                        tc.tile_pool(name="gb", bufs=16) as gb_pool,
                        tc.tile_pool(name="oh", bufs=4) as oh_pool,
                        tc.tile_pool(name="aggps", bufs=2, space="PSUM") as agg_ps,
                        tc.tile_pool(name="zps", bufs=2, space="PSUM") as z_ps,
                        tc.tile_pool(name="msb", bufs=4) as m_sb,
                    ):
                        sum_p = stat_pool.tile([128, 2, NW5], F32, tag="sum1")
                        sq_p = stat_pool.tile([128, 2, NW5], F32, tag="sq1")
                        gbufs = {}
                        for gi, grp in enumerate(wgroups):
                            for cid in wg_calls[gi]:
                                b_, ntl = calls[cid]
                                gb = gb_pool.tile([128, CALL_MAX, 128], DT_X,
                                                  tag="gb", name="gb")
                                nc.gpsimd.dma_gather(
                                    out_ap=gb[:, :ntl, :],
                                    in_ap=x_fulls[layer][b_ * BUCK:(b_ + 1) * BUCK, :],
                                    idxs_ap=idx_sb[:, call_start[cid] * 8:
                                                   (call_start[cid] + ntl) * 8],
                                    num_idxs=ntl * 128,
                                    num_idxs_reg=ntl * 128,
                                    elem_size=D,
                                    queue_num=cid % NQ)
                                gbufs[cid] = gb
                            for wi in grp:
                                c0 = wi * W5
                                cw = min(W5, M - c0)
                                ps = agg_ps.tile([128, W5], F32, tag="agg")
                                nmm = 0
                                tot = int(T[wi].sum())
                                for bi in range(NBUCK):
                                    for k in range(int(T[wi, bi])):
                                        cid, pos = tile_slot[(wi, bi, k)]
                                        slot = call_start[cid] + pos
                                        oh = oh_pool.tile([128, W5], DT_X,
                                                          tag="oh")
                                        nc.vector.tensor_scalar(
                                            out=oh[:], in0=iota5[:],
                                            scalar1=dstrel_sb[:, slot:slot + 1],
                                            scalar2=None, op0=OP.is_equal)
                                        nc.tensor.matmul(
                                            out=ps[:],
                                            lhsT=gbufs[cid][:, pos, :],
                                            rhs=oh[:],
                                            start=(nmm == 0),
                                            stop=(nmm == tot - 1))
                                        nmm += 1
                                # h = (1+eps)*x + agg
                                nc.vector.scalar_tensor_tensor(
                                    out=mmdt(hT[:, c0:c0 + cw]),
                                    in0=xT[:, c0:c0 + cw],
                                    scalar=eps1_sb[:, layer:layer + 1],
                                    in1=ps[:, :cw], op0=OP.mult, op1=OP.add)
                                # ---- MLP pass A for this window: BN1 stats
                                for b in range(2):
                                    zp = z_ps.tile([128, W5], F32, tag="z")
                                    nc.tensor.matmul(
                                        out=zp[:, :cw],
                                        lhsT=mmdt(w1r[:, layer, b, :]),
                                        rhs=mmdt(hT[:, c0:c0 + cw]),
                                        start=True, stop=True)
                                    zs = m_sb.tile([128, W5], F32, tag="zs")
                                    nc.scalar.activation(
                                        out=zs[:, :cw], in_=zp[:, :cw],
                                        func=AF.Identity,
                                        bias=b1_sb[:, layer, b:b + 1],
                                        accum_out=sum_p[:, b, wi:wi + 1])
                                    sq = m_sb.tile([128, W5], F32, tag="sq")
                                    nc.vector.scalar_tensor_tensor(
                                        out=sq[:, :cw], in0=zs[:, :cw],
                                        scalar=1.0, in1=zs[:, :cw],
                                        op0=OP.mult, op1=OP.mult,
                                        accum_out=sq_p[:, b, wi:wi + 1])
                        st = stat_pool.tile([128, 4], F32, tag="pack")
                        nc.vector.tensor_reduce(
                            out=st[:, 0:2], in_=sum_p[:], axis=mybir.AxisListType.X,
                            op=OP.add)
                        nc.vector.tensor_reduce(
                            out=st[:, 2:4], in_=sq_p[:], axis=mybir.AxisListType.X,
                            op=OP.add)
                        glob = allreduce_stats(st)
                        # A = g1*rsqrt(var+eps); B = (b1-mean)*A + be1
                        mean = stat_pool.tile([128, 2], F32, tag="mean")
                        nc.vector.tensor_scalar(
                            out=mean[:], in0=glob[:, 0:2], scalar1=1.0 / N_NODES,
                            scalar2=None, op0=OP.mult)
                        var = stat_pool.tile([128, 2], F32, tag="var")
                        nc.vector.tensor_scalar(
                            out=var[:], in0=glob[:, 2:4], scalar1=1.0 / N_NODES,
                            scalar2=None, op0=OP.mult)
                        msq = stat_pool.tile([128, 2], F32, tag="msq")
                        nc.vector.tensor_tensor(
                            out=msq[:], in0=mean[:], in1=mean[:], op=OP.mult)
                        nc.vector.tensor_tensor(
                            out=var[:], in0=var[:], in1=msq[:], op=OP.subtract)
                        sd = stat_pool.tile([128, 2], F32, tag="sd")
                        nc.scalar.activation(out=sd[:], in_=var[:], func=AF.Sqrt,
                                             bias=epsc[:, 0:1])
                        rs = stat_pool.tile([128, 2], F32, tag="rs")
                        nc.vector.reciprocal(out=rs[:], in_=sd[:])
                        A1 = stat_pool.tile([128, 2], F32, tag="A1")
                        nc.vector.tensor_tensor(
                            out=A1[:], in0=rs[:], in1=g1_sb[:, layer, :],
                            op=OP.mult)
                        B1 = stat_pool.tile([128, 2], F32, tag="B1")
                        nc.vector.tensor_tensor(
                            out=B1[:], in0=b1_sb[:, layer, :], in1=mean[:],
                            op=OP.subtract)
                        nc.vector.tensor_tensor(
                            out=B1[:], in0=B1[:], in1=A1[:], op=OP.mult)
                        nc.vector.tensor_tensor(
                            out=B1[:], in0=B1[:], in1=be1_sb[:, layer, :],
                            op=OP.add)

                    # ==== MLP pass B: recompute z1, relu, h2, BN2 stats ====
                    with (
                        tc.tile_pool(name="zps2", bufs=2, space="PSUM") as z_ps,
                        tc.tile_pool(name="h2ps", bufs=2, space="PSUM") as h2_ps,
                        tc.tile_pool(name="msb2", bufs=3) as m_sb,
                    ):
                        sum2 = stat_pool.tile([128, NW5], F32, tag="sum2")
                        sq2 = stat_pool.tile([128, NW5], F32, tag="sq2")
                        for wi in range(NW5):
                            c0 = wi * W5
                            cw = min(W5, M - c0)
                            zr = [None, None]
                            for b in range(2):
                                zp = z_ps.tile([128, W5], F32, tag="z2")
                                nc.tensor.matmul(
                                    out=zp[:, :cw],
                                    lhsT=mmdt(w1r[:, layer, b, :]),
                                    rhs=mmdt(hT[:, c0:c0 + cw]),
                                    start=True, stop=True)
                                zr[b] = m_sb.tile([128, W5], F32,
                                                  tag=f"zr{b}", name=f"zr{b}")
                                nc.scalar.activation(
                                    out=mmdt(zr[b][:, :cw]), in_=zp[:, :cw],
                                    func=AF.Relu,
                                    bias=B1[:, b:b + 1],
                                    scale=A1[:, b:b + 1])
                            hp = h2_ps.tile([128, W5], F32, tag="h2")
                            for b in range(2):
                                nc.tensor.matmul(
                                    out=hp[:, :cw],
                                    lhsT=mmdt(w2r[:, layer, b, :]),
                                    rhs=mmdt(zr[b][:, :cw]),
                                    start=(b == 0), stop=(b == 1))
                            nc.scalar.activation(
                                out=xT[:, c0:c0 + cw],
                                in_=hp[:, :cw], func=AF.Identity,
                                bias=b2_sb[:, layer:layer + 1],
                                accum_out=sum2[:, wi:wi + 1])
                            sq = m_sb.tile([128, W5], F32, tag="sqb")
                            nc.vector.scalar_tensor_tensor(
                                out=sq[:, :cw], in0=xT[:, c0:c0 + cw],
                                scalar=1.0, in1=xT[:, c0:c0 + cw],
                                op0=OP.mult, op1=OP.mult,
                                accum_out=sq2[:, wi:wi + 1])
                        st = stat_pool.tile([128, 4], F32, tag="pack")
                        nc.vector.tensor_reduce(
                            out=st[:, 0:1], in_=sum2[:], axis=mybir.AxisListType.X,
                            op=OP.add)
                        nc.vector.tensor_reduce(
                            out=st[:, 2:3], in_=sq2[:], axis=mybir.AxisListType.X,
                            op=OP.add)
                        nc.vector.memset(st[:, 1:2], 0.0)
                        nc.vector.memset(st[:, 3:4], 0.0)
                        glob = allreduce_stats(st)
                        mean = stat_pool.tile([128, 1], F32, tag="mean2")
                        nc.vector.tensor_scalar(
                            out=mean[:], in0=glob[:, 0:1], scalar1=1.0 / N_NODES,
                            scalar2=None, op0=OP.mult)
                        var = stat_pool.tile([128, 1], F32, tag="var2")
                        nc.vector.tensor_scalar(
                            out=var[:], in0=glob[:, 2:3], scalar1=1.0 / N_NODES,
                            scalar2=None, op0=OP.mult)
                        msq = stat_pool.tile([128, 1], F32, tag="msq2")
                        nc.vector.tensor_tensor(
                            out=msq[:], in0=mean[:], in1=mean[:], op=OP.mult)
                        nc.vector.tensor_tensor(
                            out=var[:], in0=var[:], in1=msq[:], op=OP.subtract)
                        sd = stat_pool.tile([128, 1], F32, tag="sd2")
                        nc.scalar.activation(out=sd[:], in_=var[:], func=AF.Sqrt,
                                             bias=epsc[:, 0:1])
                        rs = stat_pool.tile([128, 1], F32, tag="rs2")
                        nc.vector.reciprocal(out=rs[:], in_=sd[:])
                        A2 = stat_pool.tile([128, 1], F32, tag="A2")
                        nc.vector.tensor_tensor(
                            out=A2[:], in0=rs[:], in1=gbn_sb[:, layer:layer + 1],
                            op=OP.mult)
                        B2 = stat_pool.tile([128, 1], F32, tag="B2")
                        nc.vector.tensor_tensor(
                            out=B2[:], in0=mean[:], in1=A2[:], op=OP.mult)
                        nc.vector.tensor_tensor(
                            out=B2[:], in0=bbn_sb[:, layer:layer + 1], in1=B2[:],
                            op=OP.subtract)
                        # x_new = relu(h2*A2 + B2) in place on xT (h2 lives
                        # there, pre-BN, with b2 added; B2 accounts for it)
                        nc.scalar.activation(
                            out=xT[:, :], in_=xT[:, :], func=AF.Relu,
                            bias=B2[:, 0:1], scale=A2[:, 0:1])
                        nc.vector.memset(xT[:, NPC:M], 0.0)

                    if layer < L_LAYERS - 1:
                        share(xT, x_fulls[layer + 1])

                # ======== pooling ========
                with (
                    tc.tile_pool(name="poolps", bufs=2, space="PSUM") as tp_ps,
                    tc.tile_pool(name="accps", bufs=1, space="PSUM") as acc_ps,
                    tc.tile_pool(name="poolsb", bufs=4) as po_sb,
                ):
                    mask_sb = po_sb.tile([128, M], F32, name="mask_sb", bufs=1)
                    nc.sync.dma_start(out=mask_sb[:], in_=mask_d[:])
                    batchrel_sb = po_sb.tile([128, NW], F32, name="brl", bufs=1)
                    nc.sync.dma_start(out=batchrel_sb[:], in_=batchrel_d[:])
                    sel_sb = po_sb.tile([128, NW], F32, name="slb", bufs=1)
                    nc.sync.dma_start(out=sel_sb[:], in_=sel_d[:])

                    scan = hT  # reuse dead h buffer
                    nc.vector.tensor_tensor_scan(
                        out=mmdt(scan[:]), data0=mask_sb[:], data1=xT[:],
                        initial=0.0, op0=OP.mult, op1=OP.max)

                    s_ps = acc_ps.tile([128, 128], F32, tag="sacc")
                    m_ps = acc_ps.tile([128, 128], F32, tag="macc")
                    for ch in range(NW):
                        cs = ch * 128
                        tp1 = tp_ps.tile([128, 128], F32, tag="tp1")
                        nc.tensor.transpose(out=tp1[:], in_=xT[:, cs:cs + 128],
                                            identity=ident[:])
                        xrow = po_sb.tile([128, 128], F32, tag="xr")
                        nc.vector.tensor_copy(out=xrow[:], in_=tp1[:])
                        tp2 = tp_ps.tile([128, 128], F32, tag="tp2")
                        nc.tensor.transpose(out=tp2[:], in_=scan[:, cs:cs + 128],
                                            identity=ident[:])
                        srow = po_sb.tile([128, 128], F32, tag="sr")
                        nc.scalar.activation(out=srow[:], in_=tp2[:], func=AF.Copy)
                        ohB = po_sb.tile([128, 128], F32, tag="ohB")
                        nc.vector.tensor_scalar(
                            out=ohB[:], in0=iota_sb[:],
                            scalar1=batchrel_sb[:, ch:ch + 1], scalar2=None,
                            op0=OP.is_equal)
                        ohS = po_sb.tile([128, 128], F32, tag="ohS")
                        nc.vector.tensor_scalar(
                            out=ohS[:], in0=iota_sb[:],
                            scalar1=sel_sb[:, ch:ch + 1], scalar2=None,
                            op0=OP.is_equal)
                        nc.tensor.matmul(out=s_ps[:], lhsT=xrow[:],
                                         rhs=ohB[:], start=(ch == 0),
                                         stop=(ch == NW - 1))
                        nc.tensor.matmul(out=m_ps[:], lhsT=srow[:],
                                         rhs=ohS[:], start=(ch == 0),
                                         stop=(ch == NW - 1))
                    s_sb = po_sb.tile([128, 128], F32, tag="sfin")
                    nc.vector.tensor_copy(out=s_sb[:], in_=s_ps[:])
                    m_sb2 = po_sb.tile([128, 128], F32, tag="mfin")
                    nc.vector.tensor_copy(out=m_sb2[:], in_=m_ps[:])
                    nc.sync.dma_start(out=s_out_d[:, :], in_=s_sb[:])
                    nc.sync.dma_start(out=mx_out_d[:, :], in_=m_sb2[:])

    nc.compile()
    return nc


# ---------------- host postprocessing (classifier) ----------------

def _postprocess(results, g_lo, batch, cW1, cb1, cg, cbeta, cW2, cb2, cW3, cb3):
    s_g = np.zeros((G_GRAPHS, D), np.float64)
    mx_g = np.zeros((G_GRAPHS, D), np.float64)
    for c in range(NC):
        s_part = results[c]["s_out"].astype(np.float64).T    # [128 relg, 128 f]
        m_part = results[c]["mx_out"].astype(np.float64).T
        g0 = int(g_lo[c])
        hi = min(128, G_GRAPHS - g0)
        s_g[g0:g0 + hi] += s_part[:hi]
        mx_g[g0:g0 + hi] = np.maximum(mx_g[g0:g0 + hi], m_part[:hi])
    cnt = np.bincount(np.asarray(batch, np.int64), minlength=G_GRAPHS).astype(
        np.float64)
    mean = s_g / np.maximum(cnt, 1.0)[:, None]
    z = np.concatenate([s_g, mean, mx_g], axis=-1).astype(np.float32)

    def bn(v, g, b):
        m = v.mean(0)
        var = v.var(0)
        return (v - m) / np.sqrt(var + BN_EPS) * g + b

    z = np.maximum(bn(z @ cW1 + cb1, cg, cbeta), 0.0)
    z = np.maximum(z @ cW2 + cb2, 0.0)
    return (z @ cW3 + cb3).astype(np.float32)


# ---------------- input packing ----------------

def _pack_inputs(x, batch, W1, b1, g1, be1, W2, b2, gbn, bbn, eps,
                 idx16, dstrel_up, batchrel, sel, mask):
    x = np.asarray(x, np.float32)
    rep = lambda a: np.broadcast_to(np.asarray(a, np.float32), (128,) + np.asarray(a).shape).copy()
    w1u = np.ascontiguousarray(
        np.asarray(W1, np.float32).reshape(L_LAYERS, 128, 2, 128)
        .transpose(1, 0, 2, 3))
    w2u = np.ascontiguousarray(
        np.asarray(W2, np.float32).reshape(L_LAYERS, 2, 128, 128)
        .transpose(2, 0, 1, 3))
    b1u = np.ascontiguousarray(
        np.asarray(b1, np.float32).reshape(L_LAYERS, 2, 128).transpose(2, 0, 1))
    g1u = np.ascontiguousarray(
        np.asarray(g1, np.float32).reshape(L_LAYERS, 2, 128).transpose(2, 0, 1))
    be1u = np.ascontiguousarray(
        np.asarray(be1, np.float32).reshape(L_LAYERS, 2, 128).transpose(2, 0, 1))
    b2u = np.ascontiguousarray(np.asarray(b2, np.float32).T)
    gbnu = np.ascontiguousarray(np.asarray(gbn, np.float32).T)
    bbnu = np.ascontiguousarray(np.asarray(bbn, np.float32).T)
    eps1u = rep(1.0 + np.asarray(eps, np.float32))

    in_maps = []
    for c in range(NC):
        xs = np.zeros((128, M), np.float32)
        xs[:, :NPC] = x[c * NPC:(c + 1) * NPC].T
        in_maps.append({
            "xT0": xs,
            "w1u": w1u, "w2u": w2u, "b1u": b1u, "g1u": g1u, "be1u": be1u,
            "b2u": b2u, "gbnu": gbnu, "bbnu": bbnu, "eps1u": eps1u,
            "dstrelu": dstrel_up[c], "batchrelu": batchrel[c], "selu": sel[c],
            "masku": mask[c], "idx16u": idx16[c],
        })
    return in_maps


# ---------------- execution + timing ----------------

LAST_EXEC_NS = None
LAST_WALL_NS = None
LAST_RESULTS = None


@contextlib.contextmanager
def _ntff_profile(output_dir, device_ids=None):
    lib = ctypes.CDLL('/opt/axon/libaxon_pjrt.so')
    lib.axon_start_nrt_profile.argtypes = [ctypes.POINTER(ctypes.c_int64),
                                           ctypes.c_size_t]
    lib.axon_start_nrt_profile.restype = ctypes.c_int64
    lib.axon_stop_nrt_profile.argtypes = [ctypes.c_char_p]
    lib.axon_stop_nrt_profile.restype = ctypes.c_int64
    import jax
    jax.devices()
    if device_ids:
        ids = (ctypes.c_int64 * len(device_ids))(*device_ids)
        rc = lib.axon_start_nrt_profile(ids, len(device_ids))
    else:
        rc = lib.axon_start_nrt_profile(None, 0)
    if rc != 0:
        raise RuntimeError(f"axon_start_nrt_profile rc={rc}")
    try:
        yield
    finally:
        n = lib.axon_stop_nrt_profile(str(output_dir).encode())
        if n <= 0:
            print(f"ntff profile: {n} files written", file=sys.stderr)


def _ntff_exec_ns(nc, outdir, cores=(0,)):
    """Convert captured NTFFs and return max on-device exec time (ns)."""
    import gauge.profiler
    from concourse._compat import FishPath
    profile = gauge.profiler.Profile(
        profile_path=FishPath(outdir),
        kernel_dev_mode=True,
        profile_on_exit=False,
        bass_kernel=nc.m,
        offline_processing=True,
        fname="*_body*",
    )
    results = profile.to_perfetto(model_index=tuple(cores))
    return max(int(r.exec_time_ns) for r in results)


def _timed_spmd(nc, in_maps, n_reps=3, profile=True):
    """Run the SPMD program via PJRT with device-resident inputs.

    Returns (results, wall_ns, times). Also captures an NTFF (neuron-profile)
    trace of one execution to measure true on-device time when possible.
    """
    import jax
    import numpy as _np
    from jax.sharding import Mesh, PartitionSpec, NamedSharding
    from jax.experimental.shard_map import shard_map
    import concourse.mybir as mybir
    from concourse import bass2jax

    bass2jax.install_neuronx_cc_hook()
    n_cores = len(in_maps)
    partition_name = (nc.partition_id_tensor.name
                      if nc.partition_id_tensor else None)
    in_names, out_names, out_avals, zero_outs = [], [], [], []
    for alloc in nc.m.functions[0].allocations:
        if not isinstance(alloc, mybir.MemoryLocationSet):
            continue
        name = alloc.memorylocations[0].name
        if alloc.kind == "ExternalInput":
            if name != partition_name:
                in_names.append(name)
        elif alloc.kind == "ExternalOutput":
            out_names.append(name)
            shape = tuple(alloc.tensor_shape)
            dtype = mybir.dt.np(alloc.dtype)
            out_avals.append(jax.core.ShapedArray(shape, dtype))
            zero_outs.append(_np.zeros(shape, dtype))
    n_params = len(in_names)
    n_outs = len(out_avals)
    all_in_names = list(in_names) + out_names
    if partition_name is not None:
        all_in_names.append(partition_name)

    def _body(*args):
        operands = list(args)
        if partition_name is not None:
            operands.append(bass2jax.partition_id_tensor())
        outs = bass2jax._bass_exec_p.bind(
            *operands, out_avals=tuple(out_avals),
            in_names=tuple(all_in_names), out_names=tuple(out_names),
            lowering_input_output_aliases=(),
            sim_require_finite=True, sim_require_nnan=True, nc=nc)
        return tuple(outs)

    devices = jax.devices()[:n_cores]
    mesh = Mesh(_np.asarray(devices), ("core",))
    sharded = jax.jit(
        shard_map(_body, mesh=mesh,
                  in_specs=(PartitionSpec("core"),) * (n_params + n_outs),
                  out_specs=(PartitionSpec("core"),) * n_outs,
                  check_rep=False),
        keep_unused=True)
    sh = NamedSharding(mesh, PartitionSpec("core"))
    concat_in = [
        jax.device_put(_np.concatenate(
            [_np.asarray(in_maps[c][nm]) for c in range(n_cores)], axis=0), sh)
        for nm in in_names]

    def one_call():
        zeros = [_np.zeros((n_cores * z.shape[0], *z.shape[1:]), z.dtype)
                 for z in zero_outs]
        t0 = time.perf_counter()
        out_arrs = sharded(*concat_in, *zeros)
        jax.block_until_ready(out_arrs)
        return out_arrs, time.perf_counter() - t0

    times = []
    out_arrs = None
    for _ in range(n_reps):
        out_arrs, dt = one_call()
        times.append(dt)
    wall_ns = int(min(times) * 1e9)

    exec_ns = None
    if profile and os.environ.get("GIN_NO_PROFILE", "0") != "1":
        try:
            import tempfile
            outdir = tempfile.mkdtemp(prefix="gin_ntff_")
            with _ntff_profile(outdir, list(range(n_cores))):
                out_arrs, _ = one_call()
            exec_ns = _ntff_exec_ns(nc, outdir)
        except Exception as e:
            print("ntff profiling failed:", e, file=sys.stderr)

    results = [
        {nm: _np.asarray(out_arrs[i]).reshape(n_cores, *out_avals[i].shape)[c]
         for i, nm in enumerate(out_names)}
        for c in range(n_cores)]
    return results, wall_ns, times, exec_ns


def kernel(x, edge_index, batch, num_graphs, W1, b1, g1, be1, W2, b2, gbn,
           bbn, eps, cW1, cb1, cg, cbeta, cW2, cb2, cW3, cb3):
    x = np.asarray(x)
    meta, idx16, dstrel_up, batchrel, sel, mask = _preprocess(
        x, np.asarray(edge_index), np.asarray(batch))
    in_maps = _pack_inputs(x, batch, W1, b1, g1, be1, W2, b2, gbn, bbn, eps,
                           idx16, dstrel_up, batchrel, sel, mask)
    nc = _build_program(meta)
    global LAST_RESULTS, LAST_EXEC_NS, LAST_WALL_NS
    try:
        results, wall_ns, times, exec_ns = _timed_spmd(nc, in_maps)
        LAST_WALL_NS = wall_ns
        LAST_EXEC_NS = exec_ns if exec_ns is not None else wall_ns
        print("timed calls (s):", [f"{t:.4f}" for t in times])
        if exec_ns is not None:
            print(f"on-device exec (ntff): {exec_ns} ns; "
                  f"wall-clock min: {wall_ns} ns")
        LAST_RESULTS = results
    except Exception as e:
        print("timed runner failed, falling back:", e)
        from concourse.bass_utils import run_bass_kernel_spmd
        res = run_bass_kernel_spmd(nc, in_maps, core_ids=list(range(NC)))
        LAST_RESULTS = res.results
    return _postprocess(LAST_RESULTS, meta["g_lo"], batch,
                        np.asarray(cW1, np.float32), np.asarray(cb1, np.float32),
                        np.asarray(cg, np.float32), np.asarray(cbeta, np.float32),
                        np.asarray(cW2, np.float32), np.asarray(cb2, np.float32),
                        np.asarray(cW3, np.float32), np.asarray(cb3, np.float32))


# revision 11
# speedup vs baseline: 30.7816x; 1.0689x over previous
"""Trainium2 Bass kernel for BaselineGIN (nn_BaselineGIN_42502996361221).

Strategy (8 NeuronCores, SPMD):
  - Nodes sharded 12500/core, padded to 12544 (=98*128) columns per core.
  - Node features live transposed in SBUF: xT [128 feat, 12544 nodes] fp32.
  - Full node table x_full [100352, 128] bf16 in Shared DRAM, rebuilt by
    AllGather before every layer (bf16 halves gather traffic).
  - Edges partitioned by dst owner; dst space split into 512-wide windows
    (one PSUM bank per window), sources into 4 buckets of 25088 rows
    (int16 bucket-relative indices). Edges sorted by (window, bucket),
    packed into 128-row tiles per (window, bucket) cell, gathered with
    dma_gather round-robined over 4 SWDGE queues, then scatter-added into
    [128 feat, 512 dst] PSUM windows via one-hot matmuls (one-hot built
    on DVE as is_equal(iota512, dstrel), fp16 in / bf16 out).
  - GIN MLP interleaved with the scatter at 512-column granularity;
    matmuls in float32r (1 cycle/row at >=256 free dim). BatchNorm batch
    stats via per-core accumulators + tiny AllReduce; BN1 z recomputed
    after the stats AllReduce so Z1 is never materialized full-width.
  - Readout: segment sums via one-hot(batch) matmuls over PE-transposed
    chunks; segment max via tensor_tensor_scan (mult-mask reset, data>=0)
    plus one-hot(last-node) extraction matmuls. Host combines the per-core
    partials (straddled graphs) and runs the tiny [512,384] classifier.

kernel() accepts the FULL inputs and returns the FULL [512, 2] output.
LAST_EXEC_NS is the on-device NEFF execution time measured via NRT/NTFF
profiling (neuron-profile) when available; wall-clock min otherwise.
"""

import contextlib
import ctypes
import math
import os
import sys
import time

import numpy as np

# ---------------- problem constants (hardcoded per spec) ----------------
N_NODES = 100000
N_EDGES = 500000
D = 128
G_GRAPHS = 512
L_LAYERS = 4
NC = 8                      # cores
NPC = N_NODES // NC         # real nodes per core = 12500
M = 12544                   # padded per-core columns (98 * 128)
NW = M // 128               # 98 narrow windows (pooling)
W5 = 512                    # scatter dst-window width (one PSUM bank)
NW5 = math.ceil(NPC / W5)   # 25 scatter windows
NBUCK = 4
BUCK = NC * M // NBUCK      # 25088 padded rows per src bucket (< 32768)
BN_EPS = 1e-5
WGRP = 4                    # scatter windows per gather-call group
NQ = 4                      # SWDGE queues (ucode max)
CALL_MAX = 8                # tiles per dma_gather call (1024-desc ring)

USE_BF16_X = True           # x_full table + scatter matmuls in bf16
USE_F32R_MLP = True         # MLP matmuls in float32r (1 cyc/row @ >=256)


# ---------------- host preprocessing ----------------

def _pad_id(n):
    """node id -> padded global row id"""
    return (n // NPC) * M + (n % NPC)


def _preprocess(x, edge_index, batch):
    src = np.asarray(edge_index[0], dtype=np.int64)
    dst = np.asarray(edge_index[1], dtype=np.int64)
    batch = np.asarray(batch, dtype=np.int64)

    src_pad = _pad_id(src)
    core = dst // NPC
    dstl = dst - core * NPC            # local dst 0..12499
    win = dstl // W5
    dstrel_all = dstl % W5
    buck = src_pad // BUCK

    # per (core, window, bucket) edge lists
    per = [[[None] * NBUCK for _ in range(NW5)] for _ in range(NC)]
    for c in range(NC):
        m = core == c
        sp, dr, w, b = src_pad[m], dstrel_all[m], win[m], buck[m]
        key = w * NBUCK + b
        order = np.argsort(key, kind="stable")
        sp, dr, key = sp[order], dr[order], key[order]
        bounds = np.searchsorted(key, np.arange(NW5 * NBUCK + 1))
        for wi in range(NW5):
            for bi in range(NBUCK):
                lo, hi = bounds[wi * NBUCK + bi], bounds[wi * NBUCK + bi + 1]
                per[c][wi][bi] = (sp[lo:hi], dr[lo:hi])

    # uniform tiles-per-(window,bucket): max over cores
    T = np.zeros((NW5, NBUCK), np.int64)
    for wi in range(NW5):
        for bi in range(NBUCK):
            cnt = max(len(per[c][wi][bi][0]) for c in range(NC))
            T[wi, bi] = math.ceil(cnt / 128)

    # gather calls: per window-group, per bucket, calls of <=8 tiles
    wgroups = [list(range(g, min(g + WGRP, NW5))) for g in range(0, NW5, WGRP)]
    calls = []          # (bucket, ntiles)
    wg_calls = []       # per wgroup: list of call ids
    tile_slot = {}      # (w, b, k) -> (call_id, pos_in_call)
    ntiles_total = 0
    for grp in wgroups:
        ids = []
        for bi in range(NBUCK):
            pend = []
            for wi in grp:
                for k in range(int(T[wi, bi])):
                    pend.append((wi, k))
            for c0 in range(0, len(pend), CALL_MAX):
                chunk = pend[c0:c0 + CALL_MAX]
                cid = len(calls)
                ids.append(cid)
                for pos, (wi, k) in enumerate(chunk):
                    tile_slot[(wi, bi, k)] = (cid, pos)
                calls.append((bi, len(chunk)))
                ntiles_total += len(chunk)
        wg_calls.append(ids)
    call_start = np.cumsum([0] + [c[1] for c in calls])

    # per-core uploads: idx16 (wrapped [16, NT*8], replicated to 128 rows)
    # + dstrel (f32) in slot order
    idx16 = np.zeros((NC, 128, ntiles_total * 8), np.int16)
    dstrel_up = np.full((NC, 128, ntiles_total), -1.0, np.float32)
    for c in range(NC):
        idx_flat = np.zeros(ntiles_total * 128, np.int16)
        dr_flat = np.full(ntiles_total * 128, -1.0, np.float32)
        for wi in range(NW5):
            for bi in range(NBUCK):
                sp, dr = per[c][wi][bi]
                loc = (sp - bi * BUCK).astype(np.int16)
                for k in range(int(T[wi, bi])):
                    cid, pos = tile_slot[(wi, bi, k)]
                    s0 = (call_start[cid] + pos) * 128
                    seg = slice(k * 128, min((k + 1) * 128, len(sp)))
                    n = seg.stop - seg.start
                    if n > 0:
                        idx_flat[s0:s0 + n] = loc[seg]
                        dr_flat[s0:s0 + n] = dr[seg].astype(np.float32)
                    # pad slots keep idx 0 (valid) and dstrel -1
        idx16[c] = np.tile(idx_flat.reshape(ntiles_total * 8, 16).T, (8, 1))
        dstrel_up[c] = dr_flat.reshape(ntiles_total, 128).T

    # pooling metadata (128-wide windows)
    g_lo = np.zeros(NC, np.int64)
    batchrel = np.full((NC, 128, NW), -1.0, np.float32)
    sel = np.full((NC, 128, NW), -1.0, np.float32)
    mask = np.ones((NC, 128, M), np.float32)
    for c in range(NC):
        bl = batch[c * NPC:(c + 1) * NPC]
        g_lo[c] = bl[0]
        rel = (bl - g_lo[c]).astype(np.float32)
        assert rel.max() < 128, "per-core graph span exceeds 128"
        br = np.full(M, -1.0, np.float32)
        br[:NPC] = rel
        batchrel[c] = br.reshape(NW, 128).T
        last = np.zeros(NPC, bool)
        last[-1] = True
        last[:-1] = bl[1:] != bl[:-1]
        sv = np.full(M, -1.0, np.float32)
        sv[:NPC][last] = rel[last]
        sel[c] = sv.reshape(NW, 128).T
        first = np.zeros(NPC, bool)
        first[0] = True
        first[1:] = bl[1:] != bl[:-1]
        mk = np.ones(M, np.float32)
        mk[:NPC][first] = 0.0
        mask[c] = np.broadcast_to(mk, (128, M))

    meta = dict(calls=calls, call_start=call_start, tile_slot=tile_slot,
                T=T, wgroups=wgroups, wg_calls=wg_calls,
                ntiles_total=ntiles_total, g_lo=g_lo,
                max_calls_grp=max(len(ids) for ids in wg_calls))
    return meta, idx16, dstrel_up, batchrel, sel, mask


# ---------------- device program ----------------

def _build_program(meta):
    import concourse.bass as bass
    import concourse.bacc as bacc
    import concourse.tile as tile
    import concourse.mybir as mybir
    from concourse.masks import make_identity

    F32 = mybir.dt.float32
    F32R = mybir.dt.float32r
    BF16 = mybir.dt.bfloat16
    FP16 = mybir.dt.float16
    I16 = mybir.dt.int16
    AF = mybir.ActivationFunctionType
    OP = mybir.AluOpType

    DT_X = BF16 if USE_BF16_X else F32
    calls = meta["calls"]
    call_start = meta["call_start"]
    tile_slot = meta["tile_slot"]
    T = meta["T"]
    wgroups = meta["wgroups"]
    wg_calls = meta["wg_calls"]
    NT = meta["ntiles_total"]

    nc = bacc.Bacc("TRN2", target_bir_lowering=False, debug=False,
                   num_devices=NC, num_swdge_queues=NQ)

    ein = lambda n, s: nc.dram_tensor(n, s, F32, kind="ExternalInput")
    xT0_d = ein("xT0", [128, M])
    w1_d = ein("w1u", [128, L_LAYERS, 2, 128])
    w2_d = ein("w2u", [128, L_LAYERS, 2, 128])
    b1_d = ein("b1u", [128, L_LAYERS, 2])
    g1_d = ein("g1u", [128, L_LAYERS, 2])
    be1_d = ein("be1u", [128, L_LAYERS, 2])
    b2_d = ein("b2u", [128, L_LAYERS])
    gbn_d = ein("gbnu", [128, L_LAYERS])
    bbn_d = ein("bbnu", [128, L_LAYERS])
    eps1_d = ein("eps1u", [128, L_LAYERS])
    batchrel_d = ein("batchrelu", [128, NW])
    sel_d = ein("selu", [128, NW])
    mask_d = ein("masku", [128, M])
    dstrel_d = ein("dstrelu", [128, NT])
    idx_d = nc.dram_tensor("idx16u", [128, NT * 8], I16, kind="ExternalInput")

    s_out_d = nc.dram_tensor("s_out", [128, 128], F32, kind="ExternalOutput")
    mx_out_d = nc.dram_tensor("mx_out", [128, 128], F32, kind="ExternalOutput")

    def mmdt(ap):
        return ap.bitcast(F32R) if USE_F32R_MLP else ap

    with tile.TileContext(nc) as tc:
        with (
            tc.tile_pool(name="persist", bufs=1) as pp,
            tc.tile_pool(name="dram", bufs=1, space="DRAM") as dpool,
        ):
            # ---- persistent SBUF ----
            xT = pp.tile([128, M], F32, name="bufA")
            hT = pp.tile([128, M], F32, name="bufB")
            w1_sb = pp.tile([128, L_LAYERS, 2, 128], F32)
            w2_sb = pp.tile([128, L_LAYERS, 2, 128], F32)
            b1_sb = pp.tile([128, L_LAYERS, 2], F32)
            g1_sb = pp.tile([128, L_LAYERS, 2], F32)
            be1_sb = pp.tile([128, L_LAYERS, 2], F32)
            b2_sb = pp.tile([128, L_LAYERS], F32)
            gbn_sb = pp.tile([128, L_LAYERS], F32)
            bbn_sb = pp.tile([128, L_LAYERS], F32)
            eps1_sb = pp.tile([128, L_LAYERS], F32)
            dstrel_sb = pp.tile([128, NT], F32)
            idx_sb = pp.tile([128, NT * 8], I16)
            iota5 = pp.tile([128, W5], FP16)
            iota_sb = pp.tile([128, 128], F32)
            ident = pp.tile([128, 128], F32)

            for sb_t, d_t in [(xT, xT0_d), (w1_sb, w1_d), (w2_sb, w2_d),
                              (b1_sb, b1_d), (g1_sb, g1_d), (be1_sb, be1_d),
                              (b2_sb, b2_d), (gbn_sb, gbn_d), (bbn_sb, bbn_d),
                              (eps1_sb, eps1_d), (dstrel_sb, dstrel_d),
                              (idx_sb, idx_d)]:
                nc.sync.dma_start(out=sb_t[:], in_=d_t[:])

            # round weights to f32r once (BIR verifier: f32r matmul inputs
            # must come from rounding producers, DMA loads don't round)
            w1r = pp.tile([128, L_LAYERS, 2, 128], F32)
            nc.vector.tensor_copy(out=mmdt(w1r[:]), in_=w1_sb[:])
            w2r = pp.tile([128, L_LAYERS, 2, 128], F32)
            nc.vector.tensor_copy(out=mmdt(w2r[:]), in_=w2_sb[:])
            epsc = pp.tile([128, 1], F32)
            nc.vector.memset(epsc[:], BN_EPS)
            iota5_i = pp.tile([128, W5], mybir.dt.int32)
            nc.gpsimd.iota(iota5_i[:], pattern=[[1, W5]], base=0,
                           channel_multiplier=0)
            nc.vector.tensor_copy(out=iota5[:], in_=iota5_i[:])
            iota_i = pp.tile([128, 128], mybir.dt.int32)
            nc.gpsimd.iota(iota_i[:], pattern=[[1, 128]], base=0,
                           channel_multiplier=0)
            nc.vector.tensor_copy(out=iota_sb[:], in_=iota_i[:])
            make_identity(nc, ident[:])

            # ---- DRAM scratch ----
            x_fulls = [dpool.tile([NC * M, D], DT_X, addr_space="Shared",
                                  name=f"x_full{i}") for i in range(L_LAYERS)]
            xsh = dpool.tile([M, D], DT_X, name="xsh")
            ar_ins = [dpool.tile([128, 4], F32, name=f"ar_in{i}")
                      for i in range(2 * L_LAYERS)]
            ar_outs = [dpool.tile([128, 4], F32, name=f"ar_out{i}")
                       for i in range(2 * L_LAYERS)]
            ar_count = [0]
            xsh_v = xsh[:].rearrange("(t p) f -> p t f", p=128)

            def share(src_T, x_full):
                """transpose src_T -> row-major DT_X shard -> AllGather."""
                with (
                    tc.tile_pool(name="shps", bufs=2, space="PSUM") as sp_ps,
                    tc.tile_pool(name="shsb", bufs=3) as sp_sb,
                ):
                    for g in range(NW // 7):
                        xrow = sp_sb.tile([128, 7, 128], DT_X, tag="xrow")
                        for i in range(7):
                            ch = g * 7 + i
                            tp = sp_ps.tile([128, 128], F32, tag="tp")
                            nc.tensor.transpose(
                                out=tp[:], in_=src_T[:, ch * 128:(ch + 1) * 128],
                                identity=ident[:])
                            if i % 2 == 0:
                                nc.vector.tensor_copy(out=xrow[:, i, :], in_=tp[:])
                            else:
                                nc.scalar.activation(out=xrow[:, i, :], in_=tp[:],
                                                     func=AF.Copy)
                        nc.sync.dma_start(out=xsh_v[:, g * 7:(g + 1) * 7, :],
                                          in_=xrow[:])
                nc.gpsimd.collective_compute(
                    "AllGather", OP.bypass,
                    replica_groups=[list(range(NC))],
                    ins=[xsh[:].opt()], outs=[x_full[:].opt()])

            def allreduce_stats(stats_sb):
                i = ar_count[0]
                ar_count[0] += 1
                nc.sync.dma_start(out=ar_ins[i][:], in_=stats_sb[:])
                nc.gpsimd.collective_compute(
                    "AllReduce", OP.add, replica_groups=[list(range(NC))],
                    ins=[ar_ins[i][:].opt()], outs=[ar_outs[i][:].opt()])
                glob = stat_pool.tile([128, 4], F32, tag="glob")
                nc.sync.dma_start(out=glob[:], in_=ar_outs[i][:])
                return glob

            with tc.tile_pool(name="stats", bufs=2) as stat_pool:
                share(xT, x_fulls[0])  # x_full <- layer-0 input

                for layer in range(L_LAYERS):
                    # ==== scatter (agg -> hT) + MLP pass A, interleaved ====
                    with (
                        tc.tile_pool(name="gb", bufs=24) as gb_pool,
                        tc.tile_pool(name="oh", bufs=4) as oh_pool,
                        tc.tile_pool(name="aggps", bufs=2, space="PSUM") as agg_ps,
                        tc.tile_pool(name="zps", bufs=2, space="PSUM") as z_ps,
                        tc.tile_pool(name="msb", bufs=4) as m_sb,
                    ):
                        sum_p = stat_pool.tile([128, 2, NW5], F32, tag="sum1")
                        sq_p = stat_pool.tile([128, 2, NW5], F32, tag="sq1")
                        gbufs = {}

                        def emit_calls(gi):
                            for cid in wg_calls[gi]:
                                b_, ntl = calls[cid]
                                gb = gb_pool.tile([128, CALL_MAX, 128], DT_X,
                                                  tag="gb", name="gb")
                                nc.gpsimd.dma_gather(
                                    out_ap=gb[:, :ntl, :],
                                    in_ap=x_fulls[layer][b_ * BUCK:(b_ + 1) * BUCK, :],
                                    idxs_ap=idx_sb[:, call_start[cid] * 8:
                                                   (call_start[cid] + ntl) * 8],
                                    num_idxs=ntl * 128,
                                    num_idxs_reg=ntl * 128,
                                    elem_size=D,
                                    queue_num=cid % NQ)
                                gbufs[cid] = gb

                        emit_calls(0)
                        for gi, grp in enumerate(wgroups):
                            if gi + 1 < len(wgroups):
                                emit_calls(gi + 1)
                            for wi in grp:
                                c0 = wi * W5
                                cw = min(W5, M - c0)
                                ps = agg_ps.tile([128, W5], F32, tag="agg")
                                nmm = 0
                                tot = int(T[wi].sum())
                                for bi in range(NBUCK):
                                    for k in range(int(T[wi, bi])):
                                        cid, pos = tile_slot[(wi, bi, k)]
                                        slot = call_start[cid] + pos
                                        oh = oh_pool.tile([128, W5], DT_X,
                                                          tag="oh")
                                        nc.vector.tensor_scalar(
                                            out=oh[:], in0=iota5[:],
                                            scalar1=dstrel_sb[:, slot:slot + 1],
                                            scalar2=None, op0=OP.is_equal)
                                        nc.tensor.matmul(
                                            out=ps[:],
                                            lhsT=gbufs[cid][:, pos, :],
                                            rhs=oh[:],
                                            start=(nmm == 0),
                                            stop=(nmm == tot - 1))
                                        nmm += 1
                                # h = (1+eps)*x + agg
                                nc.vector.scalar_tensor_tensor(
                                    out=mmdt(hT[:, c0:c0 + cw]),
                                    in0=xT[:, c0:c0 + cw],
                                    scalar=eps1_sb[:, layer:layer + 1],
                                    in1=ps[:, :cw], op0=OP.mult, op1=OP.add)
                                # ---- MLP pass A for this window: BN1 stats
                                for b in range(2):
                                    zp = z_ps.tile([128, W5], F32, tag="z")
                                    nc.tensor.matmul(
                                        out=zp[:, :cw],
                                        lhsT=mmdt(w1r[:, layer, b, :]),
                                        rhs=mmdt(hT[:, c0:c0 + cw]),
                                        start=True, stop=True)
                                    zs = m_sb.tile([128, W5], F32, tag="zs")
                                    nc.scalar.activation(
                                        out=zs[:, :cw], in_=zp[:, :cw],
                                        func=AF.Identity,
                                        bias=b1_sb[:, layer, b:b + 1],
                                        accum_out=sum_p[:, b, wi:wi + 1])
                                    sq = m_sb.tile([128, W5], F32, tag="sq")
                                    nc.vector.scalar_tensor_tensor(
                                        out=sq[:, :cw], in0=zs[:, :cw],
                                        scalar=1.0, in1=zs[:, :cw],
                                        op0=OP.mult, op1=OP.mult,
                                        accum_out=sq_p[:, b, wi:wi + 1])
                        st = stat_pool.tile([128, 4], F32, tag="pack")
                        nc.vector.tensor_reduce(
                            out=st[:, 0:2], in_=sum_p[:], axis=mybir.AxisListType.X,
                            op=OP.add)
                        nc.vector.tensor_reduce(
                            out=st[:, 2:4], in_=sq_p[:], axis=mybir.AxisListType.X,
                            op=OP.add)
                        glob = allreduce_stats(st)
                        # A = g1*rsqrt(var+eps); B = (b1-mean)*A + be1
                        mean = stat_pool.tile([128, 2], F32, tag="mean")
                        nc.vector.tensor_scalar(
                            out=mean[:], in0=glob[:, 0:2], scalar1=1.0 / N_NODES,
                            scalar2=None, op0=OP.mult)
                        var = stat_pool.tile([128, 2], F32, tag="var")
                        nc.vector.tensor_scalar(
                            out=var[:], in0=glob[:, 2:4], scalar1=1.0 / N_NODES,
                            scalar2=None, op0=OP.mult)
                        msq = stat_pool.tile([128, 2], F32, tag="msq")
                        nc.vector.tensor_tensor(
                            out=msq[:], in0=mean[:], in1=mean[:], op=OP.mult)
                        nc.vector.tensor_tensor(
                            out=var[:], in0=var[:], in1=msq[:], op=OP.subtract)
                        sd = stat_pool.tile([128, 2], F32, tag="sd")
                        nc.scalar.activation(out=sd[:], in_=var[:], func=AF.Sqrt,
                                             bias=epsc[:, 0:1])
                        rs = stat_pool.tile([128, 2], F32, tag="rs")
                        nc.vector.reciprocal(out=rs[:], in_=sd[:])
                        A1 = stat_pool.tile([128, 2], F32, tag="A1")
                        nc.vector.tensor_tensor(
                            out=A1[:], in0=rs[:], in1=g1_sb[:, layer, :],
                            op=OP.mult)
                        B1 = stat_pool.tile([128, 2], F32, tag="B1")
                        nc.vector.tensor_tensor(
                            out=B1[:], in0=b1_sb[:, layer, :], in1=mean[:],
                            op=OP.subtract)
                        nc.vector.tensor_tensor(
                            out=B1[:], in0=B1[:], in1=A1[:], op=OP.mult)
                        nc.vector.tensor_tensor(
                            out=B1[:], in0=B1[:], in1=be1_sb[:, layer, :],
                            op=OP.add)

                    # ==== MLP pass B: recompute z1, relu, h2, BN2 stats ====
                    with (
                        tc.tile_pool(name="zps2", bufs=2, space="PSUM") as z_ps,
                        tc.tile_pool(name="h2ps", bufs=2, space="PSUM") as h2_ps,
                        tc.tile_pool(name="msb2", bufs=3) as m_sb,
                    ):
                        sum2 = stat_pool.tile([128, NW5], F32, tag="sum2")
                        sq2 = stat_pool.tile([128, NW5], F32, tag="sq2")
                        for wi in range(NW5):
                            c0 = wi * W5
                            cw = min(W5, M - c0)
                            zr = [None, None]
                            for b in range(2):
                                zp = z_ps.tile([128, W5], F32, tag="z2")
                                nc.tensor.matmul(
                                    out=zp[:, :cw],
                                    lhsT=mmdt(w1r[:, layer, b, :]),
                                    rhs=mmdt(hT[:, c0:c0 + cw]),
                                    start=True, stop=True)
                                zr[b] = m_sb.tile([128, W5], F32,
                                                  tag=f"zr{b}", name=f"zr{b}")
                                nc.scalar.activation(
                                    out=mmdt(zr[b][:, :cw]), in_=zp[:, :cw],
                                    func=AF.Relu,
                                    bias=B1[:, b:b + 1],
                                    scale=A1[:, b:b + 1])
                            hp = h2_ps.tile([128, W5], F32, tag="h2")
                            for b in range(2):
                                nc.tensor.matmul(
                                    out=hp[:, :cw],
                                    lhsT=mmdt(w2r[:, layer, b, :]),
                                    rhs=mmdt(zr[b][:, :cw]),
                                    start=(b == 0), stop=(b == 1))
                            nc.scalar.activation(
                                out=xT[:, c0:c0 + cw],
                                in_=hp[:, :cw], func=AF.Identity,
                                bias=b2_sb[:, layer:layer + 1],
                                accum_out=sum2[:, wi:wi + 1])
                            sq = m_sb.tile([128, W5], F32, tag="sqb")
                            nc.vector.scalar_tensor_tensor(
                                out=sq[:, :cw], in0=xT[:, c0:c0 + cw],
                                scalar=1.0, in1=xT[:, c0:c0 + cw],
                                op0=OP.mult, op1=OP.mult,
                                accum_out=sq2[:, wi:wi + 1])
                        st = stat_pool.tile([128, 4], F32, tag="pack")
                        nc.vector.tensor_reduce(
                            out=st[:, 0:1], in_=sum2[:], axis=mybir.AxisListType.X,
                            op=OP.add)
                        nc.vector.tensor_reduce(
                            out=st[:, 2:3], in_=sq2[:], axis=mybir.AxisListType.X,
                            op=OP.add)
                        nc.vector.memset(st[:, 1:2], 0.0)
                        nc.vector.memset(st[:, 3:4], 0.0)
                        glob = allreduce_stats(st)
                        mean = stat_pool.tile([128, 1], F32, tag="mean2")
                        nc.vector.tensor_scalar(
                            out=mean[:], in0=glob[:, 0:1], scalar1=1.0 / N_NODES,
                            scalar2=None, op0=OP.mult)
                        var = stat_pool.tile([128, 1], F32, tag="var2")
                        nc.vector.tensor_scalar(
                            out=var[:], in0=glob[:, 2:3], scalar1=1.0 / N_NODES,
                            scalar2=None, op0=OP.mult)
                        msq = stat_pool.tile([128, 1], F32, tag="msq2")
                        nc.vector.tensor_tensor(
                            out=msq[:], in0=mean[:], in1=mean[:], op=OP.mult)
                        nc.vector.tensor_tensor(
                            out=var[:], in0=var[:], in1=msq[:], op=OP.subtract)
                        sd = stat_pool.tile([128, 1], F32, tag="sd2")
                        nc.scalar.activation(out=sd[:], in_=var[:], func=AF.Sqrt,
                                             bias=epsc[:, 0:1])
                        rs = stat_pool.tile([128, 1], F32, tag="rs2")
                        nc.vector.reciprocal(out=rs[:], in_=sd[:])
                        A2 = stat_pool.tile([128, 1], F32, tag="A2")
                        nc.vector.tensor_tensor(
                            out=A2[:], in0=rs[:], in1=gbn_sb[:, layer:layer + 1],
                            op=OP.mult)
                        B2 = stat_pool.tile([128, 1], F32, tag="B2")
                        nc.vector.tensor_tensor(
                            out=B2[:], in0=mean[:], in1=A2[:], op=OP.mult)
                        nc.vector.tensor_tensor(
                            out=B2[:], in0=bbn_sb[:, layer:layer + 1], in1=B2[:],
                            op=OP.subtract)
                        # x_new = relu(h2*A2 + B2) in place on xT (h2 lives
                        # there, pre-BN, with b2 added; B2 accounts for it)
                        nc.scalar.activation(
                            out=xT[:, :], in_=xT[:, :], func=AF.Relu,
                            bias=B2[:, 0:1], scale=A2[:, 0:1])
                        nc.vector.memset(xT[:, NPC:M], 0.0)

                    if layer < L_LAYERS - 1:
                        share(xT, x_fulls[layer + 1])

                # ======== pooling ========
                with (
                    tc.tile_pool(name="poolps", bufs=2, space="PSUM") as tp_ps,
                    tc.tile_pool(name="accps", bufs=1, space="PSUM") as acc_ps,
                    tc.tile_pool(name="poolsb", bufs=4) as po_sb,
                ):
                    mask_sb = po_sb.tile([128, M], F32, name="mask_sb", bufs=1)
                    nc.sync.dma_start(out=mask_sb[:], in_=mask_d[:])
                    batchrel_sb = po_sb.tile([128, NW], F32, name="brl", bufs=1)
                    nc.sync.dma_start(out=batchrel_sb[:], in_=batchrel_d[:])
                    sel_sb = po_sb.tile([128, NW], F32, name="slb", bufs=1)
                    nc.sync.dma_start(out=sel_sb[:], in_=sel_d[:])

                    scan = hT  # reuse dead h buffer
                    nc.vector.tensor_tensor_scan(
                        out=mmdt(scan[:]), data0=mask_sb[:], data1=xT[:],
                        initial=0.0, op0=OP.mult, op1=OP.max)

                    s_ps = acc_ps.tile([128, 128], F32, tag="sacc")
                    m_ps = acc_ps.tile([128, 128], F32, tag="macc")
                    for ch in range(NW):
                        cs = ch * 128
                        tp1 = tp_ps.tile([128, 128], F32, tag="tp1")
                        nc.tensor.transpose(out=tp1[:], in_=xT[:, cs:cs + 128],
                                            identity=ident[:])
                        xrow = po_sb.tile([128, 128], F32, tag="xr")
                        nc.vector.tensor_copy(out=xrow[:], in_=tp1[:])
                        tp2 = tp_ps.tile([128, 128], F32, tag="tp2")
                        nc.tensor.transpose(out=tp2[:], in_=scan[:, cs:cs + 128],
                                            identity=ident[:])
                        srow = po_sb.tile([128, 128], F32, tag="sr")
                        nc.scalar.activation(out=srow[:], in_=tp2[:], func=AF.Copy)
                        ohB = po_sb.tile([128, 128], F32, tag="ohB")
                        nc.vector.tensor_scalar(
                            out=ohB[:], in0=iota_sb[:],
                            scalar1=batchrel_sb[:, ch:ch + 1], scalar2=None,
                            op0=OP.is_equal)
                        ohS = po_sb.tile([128, 128], F32, tag="ohS")
                        nc.vector.tensor_scalar(
                            out=ohS[:], in0=iota_sb[:],
                            scalar1=sel_sb[:, ch:ch + 1], scalar2=None,
                            op0=OP.is_equal)
                        nc.tensor.matmul(out=s_ps[:], lhsT=xrow[:],
                                         rhs=ohB[:], start=(ch == 0),
                                         stop=(ch == NW - 1))
                        nc.tensor.matmul(out=m_ps[:], lhsT=srow[:],
                                         rhs=ohS[:], start=(ch == 0),
                                         stop=(ch == NW - 1))
                    s_sb = po_sb.tile([128, 128], F32, tag="sfin")
                    nc.vector.tensor_copy(out=s_sb[:], in_=s_ps[:])
                    m_sb2 = po_sb.tile([128, 128], F32, tag="mfin")
                    nc.vector.tensor_copy(out=m_sb2[:], in_=m_ps[:])
                    nc.sync.dma_start(out=s_out_d[:, :], in_=s_sb[:])
                    nc.sync.dma_start(out=mx_out_d[:, :], in_=m_sb2[:])

    nc.compile()
    return nc


# ---------------- host postprocessing (classifier) ----------------

def _postprocess(results, g_lo, batch, cW1, cb1, cg, cbeta, cW2, cb2, cW3, cb3):
    s_g = np.zeros((G_GRAPHS, D), np.float64)
    mx_g = np.zeros((G_GRAPHS, D), np.float64)
    for c in range(NC):
        s_part = results[c]["s_out"].astype(np.float64).T    # [128 relg, 128 f]
        m_part = results[c]["mx_out"].astype(np.float64).T
        g0 = int(g_lo[c])
        hi = min(128, G_GRAPHS - g0)
        s_g[g0:g0 + hi] += s_part[:hi]
        mx_g[g0:g0 + hi] = np.maximum(mx_g[g0:g0 + hi], m_part[:hi])
    cnt = np.bincount(np.asarray(batch, np.int64), minlength=G_GRAPHS).astype(
        np.float64)
    mean = s_g / np.maximum(cnt, 1.0)[:, None]
    z = np.concatenate([s_g, mean, mx_g], axis=-1).astype(np.float32)

    def bn(v, g, b):
        m = v.mean(0)
        var = v.var(0)
        return (v - m) / np.sqrt(var + BN_EPS) * g + b

    z = np.maximum(bn(z @ cW1 + cb1, cg, cbeta), 0.0)
    z = np.maximum(z @ cW2 + cb2, 0.0)
    return (z @ cW3 + cb3).astype(np.float32)


# ---------------- input packing ----------------

def _pack_inputs(x, batch, W1, b1, g1, be1, W2, b2, gbn, bbn, eps,
                 idx16, dstrel_up, batchrel, sel, mask):
    x = np.asarray(x, np.float32)
    rep = lambda a: np.broadcast_to(np.asarray(a, np.float32), (128,) + np.asarray(a).shape).copy()
    w1u = np.ascontiguousarray(
        np.asarray(W1, np.float32).reshape(L_LAYERS, 128, 2, 128)
        .transpose(1, 0, 2, 3))
    w2u = np.ascontiguousarray(
        np.asarray(W2, np.float32).reshape(L_LAYERS, 2, 128, 128)
        .transpose(2, 0, 1, 3))
    b1u = np.ascontiguousarray(
        np.asarray(b1, np.float32).reshape(L_LAYERS, 2, 128).transpose(2, 0, 1))
    g1u = np.ascontiguousarray(
        np.asarray(g1, np.float32).reshape(L_LAYERS, 2, 128).transpose(2, 0, 1))
    be1u = np.ascontiguousarray(
        np.asarray(be1, np.float32).reshape(L_LAYERS, 2, 128).transpose(2, 0, 1))
    b2u = np.ascontiguousarray(np.asarray(b2, np.float32).T)
    gbnu = np.ascontiguousarray(np.asarray(gbn, np.float32).T)
    bbnu = np.ascontiguousarray(np.asarray(bbn, np.float32).T)
    eps1u = rep(1.0 + np.asarray(eps, np.float32))

    in_maps = []
    for c in range(NC):
        xs = np.zeros((128, M), np.float32)
        xs[:, :NPC] = x[c * NPC:(c + 1) * NPC].T
        in_maps.append({
            "xT0": xs,
            "w1u": w1u, "w2u": w2u, "b1u": b1u, "g1u": g1u, "be1u": be1u,
            "b2u": b2u, "gbnu": gbnu, "bbnu": bbnu, "eps1u": eps1u,
            "dstrelu": dstrel_up[c], "batchrelu": batchrel[c], "selu": sel[c],
            "masku": mask[c], "idx16u": idx16[c],
        })
    return in_maps


# ---------------- execution + timing ----------------

LAST_EXEC_NS = None
LAST_WALL_NS = None
LAST_RESULTS = None


@contextlib.contextmanager
def _ntff_profile(output_dir, device_ids=None):
    lib = ctypes.CDLL('/opt/axon/libaxon_pjrt.so')
    lib.axon_start_nrt_profile.argtypes = [ctypes.POINTER(ctypes.c_int64),
                                           ctypes.c_size_t]
    lib.axon_start_nrt_profile.restype = ctypes.c_int64
    lib.axon_stop_nrt_profile.argtypes = [ctypes.c_char_p]
    lib.axon_stop_nrt_profile.restype = ctypes.c_int64
    import jax
    jax.devices()
    if device_ids:
        ids = (ctypes.c_int64 * len(device_ids))(*device_ids)
        rc = lib.axon_start_nrt_profile(ids, len(device_ids))
    else:
        rc = lib.axon_start_nrt_profile(None, 0)
    if rc != 0:
        raise RuntimeError(f"axon_start_nrt_profile rc={rc}")
    try:
        yield
    finally:
        n = lib.axon_stop_nrt_profile(str(output_dir).encode())
        if n <= 0:
            print(f"ntff profile: {n} files written", file=sys.stderr)


def _ntff_exec_ns(nc, outdir, cores=(0,)):
    """Convert captured NTFFs and return max on-device exec time (ns)."""
    import gauge.profiler
    from concourse._compat import FishPath
    profile = gauge.profiler.Profile(
        profile_path=FishPath(outdir),
        kernel_dev_mode=True,
        profile_on_exit=False,
        bass_kernel=nc.m,
        offline_processing=True,
        fname="*_body*",
    )
    results = profile.to_perfetto(model_index=tuple(cores))
    return max(int(r.exec_time_ns) for r in results)


def _timed_spmd(nc, in_maps, n_reps=3, profile=True):
    """Run the SPMD program via PJRT with device-resident inputs.

    Returns (results, wall_ns, times). Also captures an NTFF (neuron-profile)
    trace of one execution to measure true on-device time when possible.
    """
    import jax
    import numpy as _np
    from jax.sharding import Mesh, PartitionSpec, NamedSharding
    from jax.experimental.shard_map import shard_map
    import concourse.mybir as mybir
    from concourse import bass2jax

    bass2jax.install_neuronx_cc_hook()
    n_cores = len(in_maps)
    partition_name = (nc.partition_id_tensor.name
                      if nc.partition_id_tensor else None)
    in_names, out_names, out_avals, zero_outs = [], [], [], []
    for alloc in nc.m.functions[0].allocations:
        if not isinstance(alloc, mybir.MemoryLocationSet):
            continue
        name = alloc.memorylocations[0].name
        if alloc.kind == "ExternalInput":
            if name != partition_name:
                in_names.append(name)
        elif alloc.kind == "ExternalOutput":
            out_names.append(name)
            shape = tuple(alloc.tensor_shape)
            dtype = mybir.dt.np(alloc.dtype)
            out_avals.append(jax.core.ShapedArray(shape, dtype))
            zero_outs.append(_np.zeros(shape, dtype))
    n_params = len(in_names)
    n_outs = len(out_avals)
    all_in_names = list(in_names) + out_names
    if partition_name is not None:
        all_in_names.append(partition_name)

    def _body(*args):
        operands = list(args)
        if partition_name is not None:
            operands.append(bass2jax.partition_id_tensor())
        outs = bass2jax._bass_exec_p.bind(
            *operands, out_avals=tuple(out_avals),
            in_names=tuple(all_in_names), out_names=tuple(out_names),
            lowering_input_output_aliases=(),
            sim_require_finite=True, sim_require_nnan=True, nc=nc)
        return tuple(outs)

    devices = jax.devices()[:n_cores]
    mesh = Mesh(_np.asarray(devices), ("core",))
    sharded = jax.jit(
        shard_map(_body, mesh=mesh,
                  in_specs=(PartitionSpec("core"),) * (n_params + n_outs),
                  out_specs=(PartitionSpec("core"),) * n_outs,
                  check_rep=False),
        keep_unused=True)
    sh = NamedSharding(mesh, PartitionSpec("core"))
    concat_in = [
        jax.device_put(_np.concatenate(
            [_np.asarray(in_maps[c][nm]) for c in range(n_cores)], axis=0), sh)
        for nm in in_names]

    def one_call():
        zeros = [_np.zeros((n_cores * z.shape[0], *z.shape[1:]), z.dtype)
                 for z in zero_outs]
        t0 = time.perf_counter()
        out_arrs = sharded(*concat_in, *zeros)
        jax.block_until_ready(out_arrs)
        return out_arrs, time.perf_counter() - t0

    times = []
    out_arrs = None
    for _ in range(n_reps):
        out_arrs, dt = one_call()
        times.append(dt)
    wall_ns = int(min(times) * 1e9)

    exec_ns = None
    if profile and os.environ.get("GIN_NO_PROFILE", "0") != "1":
        try:
            import tempfile
            outdir = tempfile.mkdtemp(prefix="gin_ntff_")
            with _ntff_profile(outdir, list(range(n_cores))):
                out_arrs, _ = one_call()
            exec_ns = _ntff_exec_ns(nc, outdir)
        except Exception as e:
            print("ntff profiling failed:", e, file=sys.stderr)

    results = [
        {nm: _np.asarray(out_arrs[i]).reshape(n_cores, *out_avals[i].shape)[c]
         for i, nm in enumerate(out_names)}
        for c in range(n_cores)]
    return results, wall_ns, times, exec_ns


def kernel(x, edge_index, batch, num_graphs, W1, b1, g1, be1, W2, b2, gbn,
           bbn, eps, cW1, cb1, cg, cbeta, cW2, cb2, cW3, cb3):
    x = np.asarray(x)
    meta, idx16, dstrel_up, batchrel, sel, mask = _preprocess(
        x, np.asarray(edge_index), np.asarray(batch))
    in_maps = _pack_inputs(x, batch, W1, b1, g1, be1, W2, b2, gbn, bbn, eps,
                           idx16, dstrel_up, batchrel, sel, mask)
    nc = _build_program(meta)
    global LAST_RESULTS, LAST_EXEC_NS, LAST_WALL_NS
    try:
        results, wall_ns, times, exec_ns = _timed_spmd(nc, in_maps)
        LAST_WALL_NS = wall_ns
        LAST_EXEC_NS = exec_ns if exec_ns is not None else wall_ns
        print("timed calls (s):", [f"{t:.4f}" for t in times])
        if exec_ns is not None:
            print(f"on-device exec (ntff): {exec_ns} ns; "
                  f"wall-clock min: {wall_ns} ns")
        LAST_RESULTS = results
    except Exception as e:
        print("timed runner failed, falling back:", e)
        from concourse.bass_utils import run_bass_kernel_spmd
        res = run_bass_kernel_spmd(nc, in_maps, core_ids=list(range(NC)))
        LAST_RESULTS = res.results
    return _postprocess(LAST_RESULTS, meta["g_lo"], batch,
                        np.asarray(cW1, np.float32), np.asarray(cb1, np.float32),
                        np.asarray(cg, np.float32), np.asarray(cbeta, np.float32),
                        np.asarray(cW2, np.float32), np.asarray(cb2, np.float32),
                        np.asarray(cW3, np.float32), np.asarray(cb3, np.float32))


# revision 13
# speedup vs baseline: 34.3527x; 1.1160x over previous
"""Trainium2 Bass kernel for BaselineGIN (nn_BaselineGIN_42502996361221).

Strategy (8 NeuronCores, SPMD):
  - Nodes sharded 12500/core, padded to 12544 (=98*128) columns per core.
  - Node features live transposed in SBUF: xT [128 feat, 12544 nodes] fp32.
  - Full node table x_full [100352, 128] bf16 in Shared DRAM, rebuilt by
    AllGather before every layer (bf16 halves gather traffic).
  - Edges partitioned by dst owner; dst space split into 512-wide windows
    (one PSUM bank per window), sources into 4 buckets of 25088 rows
    (int16 bucket-relative indices). Edges sorted by (window, bucket),
    packed into 128-row tiles per (window, bucket) cell, gathered with
    dma_gather round-robined over 4 SWDGE queues, then scatter-added into
    [128 feat, 512 dst] PSUM windows via one-hot matmuls (one-hot built
    on DVE as is_equal(iota512, dstrel), fp16 in / bf16 out).
  - GIN MLP interleaved with the scatter at 512-column granularity;
    matmuls in float32r (1 cycle/row at >=256 free dim). BatchNorm batch
    stats via per-core accumulators + tiny AllReduce; BN1 z recomputed
    after the stats AllReduce so Z1 is never materialized full-width.
  - Readout: segment sums via one-hot(batch) matmuls over PE-transposed
    chunks; segment max via tensor_tensor_scan (mult-mask reset, data>=0)
    plus one-hot(last-node) extraction matmuls. Host combines the per-core
    partials (straddled graphs) and runs the tiny [512,384] classifier.

kernel() accepts the FULL inputs and returns the FULL [512, 2] output.
LAST_EXEC_NS is the on-device NEFF execution time measured via NRT/NTFF
profiling (neuron-profile) when available; wall-clock min otherwise.
"""

import contextlib
import ctypes
import math
import os
import sys
import time

import numpy as np

# ---------------- problem constants (hardcoded per spec) ----------------
N_NODES = 100000
N_EDGES = 500000
D = 128
G_GRAPHS = 512
L_LAYERS = 4
NC = 8                      # cores
NPC = N_NODES // NC         # real nodes per core = 12500
M = 12544                   # padded per-core columns (98 * 128)
NW = M // 128               # 98 narrow windows (pooling)
W5 = 512                    # scatter dst-window width (one PSUM bank)
NW5 = math.ceil(NPC / W5)   # 25 scatter windows
NBUCK = 4
BUCK = NC * M // NBUCK      # 25088 padded rows per src bucket (< 32768)
BN_EPS = 1e-5
WGRP = 4                    # scatter windows per gather-call group
NQ = 4                      # SWDGE queues (ucode max)
CALL_MAX = 8                # tiles per dma_gather call (1024-desc ring)

USE_BF16_X = True           # x_full table + scatter matmuls in bf16
USE_F32R_MLP = True         # MLP matmuls in float32r (1 cyc/row @ >=256)


# ---------------- host preprocessing ----------------

def _pad_id(n):
    """node id -> padded global row id"""
    return (n // NPC) * M + (n % NPC)


def _preprocess(x, edge_index, batch):
    src = np.asarray(edge_index[0], dtype=np.int64)
    dst = np.asarray(edge_index[1], dtype=np.int64)
    batch = np.asarray(batch, dtype=np.int64)

    src_pad = _pad_id(src)
    core = dst // NPC
    dstl = dst - core * NPC            # local dst 0..12499
    win = dstl // W5
    dstrel_all = dstl % W5
    buck = src_pad // BUCK

    # per (core, window, bucket) edge lists
    per = [[[None] * NBUCK for _ in range(NW5)] for _ in range(NC)]
    for c in range(NC):
        m = core == c
        sp, dr, w, b = src_pad[m], dstrel_all[m], win[m], buck[m]
        key = w * NBUCK + b
        order = np.argsort(key, kind="stable")
        sp, dr, key = sp[order], dr[order], key[order]
        bounds = np.searchsorted(key, np.arange(NW5 * NBUCK + 1))
        for wi in range(NW5):
            for bi in range(NBUCK):
                lo, hi = bounds[wi * NBUCK + bi], bounds[wi * NBUCK + bi + 1]
                per[c][wi][bi] = (sp[lo:hi], dr[lo:hi])

    # uniform tiles-per-(window,bucket): max over cores
    T = np.zeros((NW5, NBUCK), np.int64)
    for wi in range(NW5):
        for bi in range(NBUCK):
            cnt = max(len(per[c][wi][bi][0]) for c in range(NC))
            T[wi, bi] = math.ceil(cnt / 128)

    # gather calls: per window-group, per bucket, calls of <=8 tiles
    wgroups = [list(range(g, min(g + WGRP, NW5))) for g in range(0, NW5, WGRP)]
    calls = []          # (bucket, ntiles)
    wg_calls = []       # per wgroup: list of call ids
    tile_slot = {}      # (w, b, k) -> (call_id, pos_in_call)
    ntiles_total = 0
    for grp in wgroups:
        ids = []
        for bi in range(NBUCK):
            pend = []
            for wi in grp:
                for k in range(int(T[wi, bi])):
                    pend.append((wi, k))
            for c0 in range(0, len(pend), CALL_MAX):
                chunk = pend[c0:c0 + CALL_MAX]
                cid = len(calls)
                ids.append(cid)
                for pos, (wi, k) in enumerate(chunk):
                    tile_slot[(wi, bi, k)] = (cid, pos)
                calls.append((bi, len(chunk)))
                ntiles_total += len(chunk)
        wg_calls.append(ids)
    call_start = np.cumsum([0] + [c[1] for c in calls])

    # per-core uploads: idx16 (wrapped [16, NT*8], replicated to 128 rows)
    # + dstrel (f32) in slot order
    idx16 = np.zeros((NC, 128, ntiles_total * 8), np.int16)
    dstrel_up = np.full((NC, 128, ntiles_total), -1.0, np.float32)
    for c in range(NC):
        idx_flat = np.zeros(ntiles_total * 128, np.int16)
        dr_flat = np.full(ntiles_total * 128, -1.0, np.float32)
        for wi in range(NW5):
            for bi in range(NBUCK):
                sp, dr = per[c][wi][bi]
                loc = (sp - bi * BUCK).astype(np.int16)
                for k in range(int(T[wi, bi])):
                    cid, pos = tile_slot[(wi, bi, k)]
                    s0 = (call_start[cid] + pos) * 128
                    seg = slice(k * 128, min((k + 1) * 128, len(sp)))
                    n = seg.stop - seg.start
                    if n > 0:
                        idx_flat[s0:s0 + n] = loc[seg]
                        dr_flat[s0:s0 + n] = dr[seg].astype(np.float32)
                    # pad slots keep idx 0 (valid) and dstrel -1
        idx16[c] = np.tile(idx_flat.reshape(ntiles_total * 8, 16).T, (8, 1))
        dstrel_up[c] = dr_flat.reshape(ntiles_total, 128).T

    # pooling metadata (128-wide windows)
    g_lo = np.zeros(NC, np.int64)
    batchrel = np.full((NC, 128, NW), -1.0, np.float32)
    sel = np.full((NC, 128, NW), -1.0, np.float32)
    mask = np.ones((NC, 128, M), np.float32)
    for c in range(NC):
        bl = batch[c * NPC:(c + 1) * NPC]
        g_lo[c] = bl[0]
        rel = (bl - g_lo[c]).astype(np.float32)
        assert rel.max() < 128, "per-core graph span exceeds 128"
        br = np.full(M, -1.0, np.float32)
        br[:NPC] = rel
        batchrel[c] = br.reshape(NW, 128).T
        last = np.zeros(NPC, bool)
        last[-1] = True
        last[:-1] = bl[1:] != bl[:-1]
        sv = np.full(M, -1.0, np.float32)
        sv[:NPC][last] = rel[last]
        sel[c] = sv.reshape(NW, 128).T
        first = np.zeros(NPC, bool)
        first[0] = True
        first[1:] = bl[1:] != bl[:-1]
        mk = np.ones(M, np.float32)
        mk[:NPC][first] = 0.0
        mask[c] = np.broadcast_to(mk, (128, M))

    meta = dict(calls=calls, call_start=call_start, tile_slot=tile_slot,
                T=T, wgroups=wgroups, wg_calls=wg_calls,
                ntiles_total=ntiles_total, g_lo=g_lo,
                max_calls_grp=max(len(ids) for ids in wg_calls))
    return meta, idx16, dstrel_up, batchrel, sel, mask


# ---------------- device program ----------------

def _build_program(meta):
    import concourse.bass as bass
    import concourse.bacc as bacc
    import concourse.tile as tile
    import concourse.mybir as mybir
    from concourse.masks import make_identity

    F32 = mybir.dt.float32
    F32R = mybir.dt.float32r
    BF16 = mybir.dt.bfloat16
    FP16 = mybir.dt.float16
    I16 = mybir.dt.int16
    AF = mybir.ActivationFunctionType
    OP = mybir.AluOpType

    DT_X = BF16 if USE_BF16_X else F32
    calls = meta["calls"]
    call_start = meta["call_start"]
    tile_slot = meta["tile_slot"]
    T = meta["T"]
    wgroups = meta["wgroups"]
    wg_calls = meta["wg_calls"]
    NT = meta["ntiles_total"]

    nc = bacc.Bacc("TRN2", target_bir_lowering=False, debug=False,
                   num_devices=NC, num_swdge_queues=NQ)

    ein = lambda n, s: nc.dram_tensor(n, s, F32, kind="ExternalInput")
    xT0_d = ein("xT0", [128, M])
    w1_d = ein("w1u", [128, L_LAYERS, 2, 128])
    w2_d = ein("w2u", [128, L_LAYERS, 2, 128])
    b1_d = ein("b1u", [128, L_LAYERS, 2])
    g1_d = ein("g1u", [128, L_LAYERS, 2])
    be1_d = ein("be1u", [128, L_LAYERS, 2])
    b2_d = ein("b2u", [128, L_LAYERS])
    gbn_d = ein("gbnu", [128, L_LAYERS])
    bbn_d = ein("bbnu", [128, L_LAYERS])
    eps1_d = ein("eps1u", [128, L_LAYERS])
    batchrel_d = ein("batchrelu", [128, NW])
    sel_d = ein("selu", [128, NW])
    mask_d = ein("masku", [128, M])
    dstrel_d = ein("dstrelu", [128, NT])
    idx_d = nc.dram_tensor("idx16u", [128, NT * 8], I16, kind="ExternalInput")

    s_out_d = nc.dram_tensor("s_out", [128, 128], F32, kind="ExternalOutput")
    mx_out_d = nc.dram_tensor("mx_out", [128, 128], F32, kind="ExternalOutput")

    def mmdt(ap):
        return ap.bitcast(F32R) if USE_F32R_MLP else ap

    with tile.TileContext(nc) as tc:
        with (
            tc.tile_pool(name="persist", bufs=1) as pp,
            tc.tile_pool(name="dram", bufs=1, space="DRAM") as dpool,
        ):
            # ---- persistent SBUF ----
            xT = pp.tile([128, M], F32, name="bufA")
            hT = pp.tile([128, M], F32, name="bufB")
            w1_sb = pp.tile([128, L_LAYERS, 2, 128], F32)
            w2_sb = pp.tile([128, L_LAYERS, 2, 128], F32)
            b1_sb = pp.tile([128, L_LAYERS, 2], F32)
            g1_sb = pp.tile([128, L_LAYERS, 2], F32)
            be1_sb = pp.tile([128, L_LAYERS, 2], F32)
            b2_sb = pp.tile([128, L_LAYERS], F32)
            gbn_sb = pp.tile([128, L_LAYERS], F32)
            bbn_sb = pp.tile([128, L_LAYERS], F32)
            eps1_sb = pp.tile([128, L_LAYERS], F32)
            dstrel_sb = pp.tile([128, NT], F32)
            idx_sb = pp.tile([128, NT * 8], I16)
            iota5 = pp.tile([128, W5], FP16)
            iota_sb = pp.tile([128, 128], F32)
            ident = pp.tile([128, 128], F32)

            for sb_t, d_t in [(xT, xT0_d), (w1_sb, w1_d), (w2_sb, w2_d),
                              (b1_sb, b1_d), (g1_sb, g1_d), (be1_sb, be1_d),
                              (b2_sb, b2_d), (gbn_sb, gbn_d), (bbn_sb, bbn_d),
                              (eps1_sb, eps1_d), (dstrel_sb, dstrel_d),
                              (idx_sb, idx_d)]:
                nc.sync.dma_start(out=sb_t[:], in_=d_t[:])

            # round weights to f32r once (BIR verifier: f32r matmul inputs
            # must come from rounding producers, DMA loads don't round)
            w1r = pp.tile([128, L_LAYERS, 2, 128], F32)
            nc.vector.tensor_copy(out=mmdt(w1r[:]), in_=w1_sb[:])
            w2r = pp.tile([128, L_LAYERS, 2, 128], F32)
            nc.vector.tensor_copy(out=mmdt(w2r[:]), in_=w2_sb[:])
            epsc = pp.tile([128, 1], F32)
            nc.vector.memset(epsc[:], BN_EPS)
            iota5_i = pp.tile([128, W5], mybir.dt.int32)
            nc.gpsimd.iota(iota5_i[:], pattern=[[1, W5]], base=0,
                           channel_multiplier=0)
            nc.vector.tensor_copy(out=iota5[:], in_=iota5_i[:])
            iota_i = pp.tile([128, 128], mybir.dt.int32)
            nc.gpsimd.iota(iota_i[:], pattern=[[1, 128]], base=0,
                           channel_multiplier=0)
            nc.vector.tensor_copy(out=iota_sb[:], in_=iota_i[:])
            make_identity(nc, ident[:])

            # ---- DRAM scratch ----
            x_fulls = [dpool.tile([NC * M, D], DT_X, addr_space="Shared",
                                  name=f"x_full{i}") for i in range(L_LAYERS)]
            xsh = dpool.tile([M, D], DT_X, name="xsh")
            ar_ins = [dpool.tile([128, 4], F32, name=f"ar_in{i}")
                      for i in range(2 * L_LAYERS)]
            ar_outs = [dpool.tile([128, 4], F32, name=f"ar_out{i}")
                       for i in range(2 * L_LAYERS)]
            ar_count = [0]
            xsh_v = xsh[:].rearrange("(t p) f -> p t f", p=128)

            def share(src_T, x_full):
                """transpose src_T -> row-major DT_X shard -> AllGather."""
                with (
                    tc.tile_pool(name="shps", bufs=2, space="PSUM") as sp_ps,
                    tc.tile_pool(name="shsb", bufs=3) as sp_sb,
                ):
                    for g in range(NW // 7):
                        xrow = sp_sb.tile([128, 7, 128], DT_X, tag="xrow")
                        for i in range(7):
                            ch = g * 7 + i
                            tp = sp_ps.tile([128, 128], F32, tag="tp")
                            nc.tensor.transpose(
                                out=tp[:], in_=src_T[:, ch * 128:(ch + 1) * 128],
                                identity=ident[:])
                            if i % 2 == 0:
                                nc.vector.tensor_copy(out=xrow[:, i, :], in_=tp[:])
                            else:
                                nc.scalar.activation(out=xrow[:, i, :], in_=tp[:],
                                                     func=AF.Copy)
                        nc.sync.dma_start(out=xsh_v[:, g * 7:(g + 1) * 7, :],
                                          in_=xrow[:])
                nc.gpsimd.collective_compute(
                    "AllGather", OP.bypass,
                    replica_groups=[list(range(NC))],
                    ins=[xsh[:].opt()], outs=[x_full[:].opt()])

            def allreduce_stats(stats_sb):
                i = ar_count[0]
                ar_count[0] += 1
                nc.sync.dma_start(out=ar_ins[i][:], in_=stats_sb[:])
                nc.gpsimd.collective_compute(
                    "AllReduce", OP.add, replica_groups=[list(range(NC))],
                    ins=[ar_ins[i][:].opt()], outs=[ar_outs[i][:].opt()])
                glob = stat_pool.tile([128, 4], F32, tag="glob")
                nc.sync.dma_start(out=glob[:], in_=ar_outs[i][:])
                return glob

            with tc.tile_pool(name="stats", bufs=2) as stat_pool:
                share(xT, x_fulls[0])  # x_full <- layer-0 input

                for layer in range(L_LAYERS):
                    # ==== scatter (agg -> hT) + MLP pass A, interleaved ====
                    with (
                        tc.tile_pool(name="gb", bufs=24) as gb_pool,
                        tc.tile_pool(name="oh", bufs=10) as oh_pool,
                        tc.tile_pool(name="aggps", bufs=3, space="PSUM") as agg_ps,
                        tc.tile_pool(name="zps", bufs=2, space="PSUM") as z_ps,
                        tc.tile_pool(name="msb", bufs=3) as m_sb,
                    ):
                        sum_p = stat_pool.tile([128, 2, NW5], F32, tag="sum1")
                        sq_p = stat_pool.tile([128, 2, NW5], F32, tag="sq1")
                        gbufs = {}

                        def emit_calls(gi):
                            for cid in wg_calls[gi]:
                                b_, ntl = calls[cid]
                                gb = gb_pool.tile([128, CALL_MAX, 128], DT_X,
                                                  tag="gb", name="gb")
                                nc.gpsimd.dma_gather(
                                    out_ap=gb[:, :ntl, :],
                                    in_ap=x_fulls[layer][b_ * BUCK:(b_ + 1) * BUCK, :],
                                    idxs_ap=idx_sb[:, call_start[cid] * 8:
                                                   (call_start[cid] + ntl) * 8],
                                    num_idxs=ntl * 128,
                                    num_idxs_reg=ntl * 128,
                                    elem_size=D,
                                    queue_num=cid % NQ)
                                gbufs[cid] = gb

                        emit_calls(0)
                        for gi, grp in enumerate(wgroups):
                            if gi + 1 < len(wgroups):
                                emit_calls(gi + 1)
                            for wi in grp:
                                c0 = wi * W5
                                cw = min(W5, M - c0)
                                ps = agg_ps.tile([128, W5], F32, tag="agg")
                                nmm = 0
                                tot = int(T[wi].sum())
                                for bi in range(NBUCK):
                                    for k in range(int(T[wi, bi])):
                                        cid, pos = tile_slot[(wi, bi, k)]
                                        slot = call_start[cid] + pos
                                        oh = oh_pool.tile([128, W5], DT_X,
                                                          tag="oh")
                                        nc.vector.tensor_scalar(
                                            out=oh[:], in0=iota5[:],
                                            scalar1=dstrel_sb[:, slot:slot + 1],
                                            scalar2=None, op0=OP.is_equal)
                                        nc.tensor.matmul(
                                            out=ps[:],
                                            lhsT=gbufs[cid][:, pos, :],
                                            rhs=oh[:],
                                            start=(nmm == 0),
                                            stop=(nmm == tot - 1))
                                        nmm += 1
                                # h = (1+eps)*x + agg
                                nc.vector.scalar_tensor_tensor(
                                    out=mmdt(hT[:, c0:c0 + cw]),
                                    in0=xT[:, c0:c0 + cw],
                                    scalar=eps1_sb[:, layer:layer + 1],
                                    in1=ps[:, :cw], op0=OP.mult, op1=OP.add)
                                # ---- MLP pass A for this window: BN1 stats
                                for b in range(2):
                                    zp = z_ps.tile([128, W5], F32, tag="z")
                                    nc.tensor.matmul(
                                        out=zp[:, :cw],
                                        lhsT=mmdt(w1r[:, layer, b, :]),
                                        rhs=mmdt(hT[:, c0:c0 + cw]),
                                        start=True, stop=True)
                                    zs = m_sb.tile([128, W5], F32, tag="zs")
                                    nc.scalar.activation(
                                        out=zs[:, :cw], in_=zp[:, :cw],
                                        func=AF.Identity,
                                        bias=b1_sb[:, layer, b:b + 1],
                                        accum_out=sum_p[:, b, wi:wi + 1])
                                    sq = m_sb.tile([128, W5], F32, tag="sq")
                                    nc.vector.scalar_tensor_tensor(
                                        out=sq[:, :cw], in0=zs[:, :cw],
                                        scalar=1.0, in1=zs[:, :cw],
                                        op0=OP.mult, op1=OP.mult,
                                        accum_out=sq_p[:, b, wi:wi + 1])
                        st = stat_pool.tile([128, 4], F32, tag="pack")
                        nc.vector.tensor_reduce(
                            out=st[:, 0:2], in_=sum_p[:], axis=mybir.AxisListType.X,
                            op=OP.add)
                        nc.vector.tensor_reduce(
                            out=st[:, 2:4], in_=sq_p[:], axis=mybir.AxisListType.X,
                            op=OP.add)
                        glob = allreduce_stats(st)
                        # A = g1*rsqrt(var+eps); B = (b1-mean)*A + be1
                        mean = stat_pool.tile([128, 2], F32, tag="mean")
                        nc.vector.tensor_scalar(
                            out=mean[:], in0=glob[:, 0:2], scalar1=1.0 / N_NODES,
                            scalar2=None, op0=OP.mult)
                        var = stat_pool.tile([128, 2], F32, tag="var")
                        nc.vector.tensor_scalar(
                            out=var[:], in0=glob[:, 2:4], scalar1=1.0 / N_NODES,
                            scalar2=None, op0=OP.mult)
                        msq = stat_pool.tile([128, 2], F32, tag="msq")
                        nc.vector.tensor_tensor(
                            out=msq[:], in0=mean[:], in1=mean[:], op=OP.mult)
                        nc.vector.tensor_tensor(
                            out=var[:], in0=var[:], in1=msq[:], op=OP.subtract)
                        sd = stat_pool.tile([128, 2], F32, tag="sd")
                        nc.scalar.activation(out=sd[:], in_=var[:], func=AF.Sqrt,
                                             bias=epsc[:, 0:1])
                        rs = stat_pool.tile([128, 2], F32, tag="rs")
                        nc.vector.reciprocal(out=rs[:], in_=sd[:])
                        A1 = stat_pool.tile([128, 2], F32, tag="A1")
                        nc.vector.tensor_tensor(
                            out=A1[:], in0=rs[:], in1=g1_sb[:, layer, :],
                            op=OP.mult)
                        B1 = stat_pool.tile([128, 2], F32, tag="B1")
                        nc.vector.tensor_tensor(
                            out=B1[:], in0=b1_sb[:, layer, :], in1=mean[:],
                            op=OP.subtract)
                        nc.vector.tensor_tensor(
                            out=B1[:], in0=B1[:], in1=A1[:], op=OP.mult)
                        nc.vector.tensor_tensor(
                            out=B1[:], in0=B1[:], in1=be1_sb[:, layer, :],
                            op=OP.add)

                    # ==== MLP pass B: recompute z1, relu, h2, BN2 stats ====
                    with (
                        tc.tile_pool(name="zps2", bufs=2, space="PSUM") as z_ps,
                        tc.tile_pool(name="h2ps", bufs=2, space="PSUM") as h2_ps,
                        tc.tile_pool(name="msb2", bufs=3) as m_sb,
                    ):
                        sum2 = stat_pool.tile([128, NW5], F32, tag="sum2")
                        sq2 = stat_pool.tile([128, NW5], F32, tag="sq2")
                        for wi in range(NW5):
                            c0 = wi * W5
                            cw = min(W5, M - c0)
                            zr = [None, None]
                            for b in range(2):
                                zp = z_ps.tile([128, W5], F32, tag="z2")
                                nc.tensor.matmul(
                                    out=zp[:, :cw],
                                    lhsT=mmdt(w1r[:, layer, b, :]),
                                    rhs=mmdt(hT[:, c0:c0 + cw]),
                                    start=True, stop=True)
                                zr[b] = m_sb.tile([128, W5], F32,
                                                  tag=f"zr{b}", name=f"zr{b}")
                                nc.scalar.activation(
                                    out=mmdt(zr[b][:, :cw]), in_=zp[:, :cw],
                                    func=AF.Relu,
                                    bias=B1[:, b:b + 1],
                                    scale=A1[:, b:b + 1])
                            hp = h2_ps.tile([128, W5], F32, tag="h2")
                            for b in range(2):
                                nc.tensor.matmul(
                                    out=hp[:, :cw],
                                    lhsT=mmdt(w2r[:, layer, b, :]),
                                    rhs=mmdt(zr[b][:, :cw]),
                                    start=(b == 0), stop=(b == 1))
                            nc.scalar.activation(
                                out=xT[:, c0:c0 + cw],
                                in_=hp[:, :cw], func=AF.Identity,
                                bias=b2_sb[:, layer:layer + 1],
                                accum_out=sum2[:, wi:wi + 1])
                            sq = m_sb.tile([128, W5], F32, tag="sqb")
                            nc.vector.scalar_tensor_tensor(
                                out=sq[:, :cw], in0=xT[:, c0:c0 + cw],
                                scalar=1.0, in1=xT[:, c0:c0 + cw],
                                op0=OP.mult, op1=OP.mult,
                                accum_out=sq2[:, wi:wi + 1])
                        st = stat_pool.tile([128, 4], F32, tag="pack")
                        nc.vector.tensor_reduce(
                            out=st[:, 0:1], in_=sum2[:], axis=mybir.AxisListType.X,
                            op=OP.add)
                        nc.vector.tensor_reduce(
                            out=st[:, 2:3], in_=sq2[:], axis=mybir.AxisListType.X,
                            op=OP.add)
                        nc.vector.memset(st[:, 1:2], 0.0)
                        nc.vector.memset(st[:, 3:4], 0.0)
                        glob = allreduce_stats(st)
                        mean = stat_pool.tile([128, 1], F32, tag="mean2")
                        nc.vector.tensor_scalar(
                            out=mean[:], in0=glob[:, 0:1], scalar1=1.0 / N_NODES,
                            scalar2=None, op0=OP.mult)
                        var = stat_pool.tile([128, 1], F32, tag="var2")
                        nc.vector.tensor_scalar(
                            out=var[:], in0=glob[:, 2:3], scalar1=1.0 / N_NODES,
                            scalar2=None, op0=OP.mult)
                        msq = stat_pool.tile([128, 1], F32, tag="msq2")
                        nc.vector.tensor_tensor(
                            out=msq[:], in0=mean[:], in1=mean[:], op=OP.mult)
                        nc.vector.tensor_tensor(
                            out=var[:], in0=var[:], in1=msq[:], op=OP.subtract)
                        sd = stat_pool.tile([128, 1], F32, tag="sd2")
                        nc.scalar.activation(out=sd[:], in_=var[:], func=AF.Sqrt,
                                             bias=epsc[:, 0:1])
                        rs = stat_pool.tile([128, 1], F32, tag="rs2")
                        nc.vector.reciprocal(out=rs[:], in_=sd[:])
                        A2 = stat_pool.tile([128, 1], F32, tag="A2")
                        nc.vector.tensor_tensor(
                            out=A2[:], in0=rs[:], in1=gbn_sb[:, layer:layer + 1],
                            op=OP.mult)
                        B2 = stat_pool.tile([128, 1], F32, tag="B2")
                        nc.vector.tensor_tensor(
                            out=B2[:], in0=mean[:], in1=A2[:], op=OP.mult)
                        nc.vector.tensor_tensor(
                            out=B2[:], in0=bbn_sb[:, layer:layer + 1], in1=B2[:],
                            op=OP.subtract)
                        # x_new = relu(h2*A2 + B2) in place on xT (h2 lives
                        # there, pre-BN, with b2 added; B2 accounts for it)
                        nc.scalar.activation(
                            out=xT[:, :], in_=xT[:, :], func=AF.Relu,
                            bias=B2[:, 0:1], scale=A2[:, 0:1])
                        nc.vector.memset(xT[:, NPC:M], 0.0)

                    if layer < L_LAYERS - 1:
                        share(xT, x_fulls[layer + 1])

                # ======== pooling ========
                with (
                    tc.tile_pool(name="poolps", bufs=2, space="PSUM") as tp_ps,
                    tc.tile_pool(name="accps", bufs=1, space="PSUM") as acc_ps,
                    tc.tile_pool(name="poolsb", bufs=4) as po_sb,
                ):
                    mask_sb = po_sb.tile([128, M], F32, name="mask_sb", bufs=1)
                    nc.sync.dma_start(out=mask_sb[:], in_=mask_d[:])
                    batchrel_sb = po_sb.tile([128, NW], F32, name="brl", bufs=1)
                    nc.sync.dma_start(out=batchrel_sb[:], in_=batchrel_d[:])
                    sel_sb = po_sb.tile([128, NW], F32, name="slb", bufs=1)
                    nc.sync.dma_start(out=sel_sb[:], in_=sel_d[:])

                    scan = hT  # reuse dead h buffer
                    nc.vector.tensor_tensor_scan(
                        out=mmdt(scan[:]), data0=mask_sb[:], data1=xT[:],
                        initial=0.0, op0=OP.mult, op1=OP.max)

                    s_ps = acc_ps.tile([128, 128], F32, tag="sacc")
                    m_ps = acc_ps.tile([128, 128], F32, tag="macc")
                    for ch in range(NW):
                        cs = ch * 128
                        tp1 = tp_ps.tile([128, 128], F32, tag="tp1")
                        nc.tensor.transpose(out=tp1[:], in_=xT[:, cs:cs + 128],
                                            identity=ident[:])
                        xrow = po_sb.tile([128, 128], F32, tag="xr")
                        nc.vector.tensor_copy(out=xrow[:], in_=tp1[:])
                        tp2 = tp_ps.tile([128, 128], F32, tag="tp2")
                        nc.tensor.transpose(out=tp2[:], in_=scan[:, cs:cs + 128],
                                            identity=ident[:])
                        srow = po_sb.tile([128, 128], F32, tag="sr")
                        nc.scalar.activation(out=srow[:], in_=tp2[:], func=AF.Copy)
                        ohB = po_sb.tile([128, 128], F32, tag="ohB")
                        nc.vector.tensor_scalar(
                            out=ohB[:], in0=iota_sb[:],
                            scalar1=batchrel_sb[:, ch:ch + 1], scalar2=None,
                            op0=OP.is_equal)
                        ohS = po_sb.tile([128, 128], F32, tag="ohS")
                        nc.vector.tensor_scalar(
                            out=ohS[:], in0=iota_sb[:],
                            scalar1=sel_sb[:, ch:ch + 1], scalar2=None,
                            op0=OP.is_equal)
                        nc.tensor.matmul(out=s_ps[:], lhsT=xrow[:],
                                         rhs=ohB[:], start=(ch == 0),
                                         stop=(ch == NW - 1))
                        nc.tensor.matmul(out=m_ps[:], lhsT=srow[:],
                                         rhs=ohS[:], start=(ch == 0),
                                         stop=(ch == NW - 1))
                    s_sb = po_sb.tile([128, 128], F32, tag="sfin")
                    nc.vector.tensor_copy(out=s_sb[:], in_=s_ps[:])
                    m_sb2 = po_sb.tile([128, 128], F32, tag="mfin")
                    nc.vector.tensor_copy(out=m_sb2[:], in_=m_ps[:])
                    nc.sync.dma_start(out=s_out_d[:, :], in_=s_sb[:])
                    nc.sync.dma_start(out=mx_out_d[:, :], in_=m_sb2[:])

    nc.compile()
    return nc


# ---------------- host postprocessing (classifier) ----------------

def _postprocess(results, g_lo, batch, cW1, cb1, cg, cbeta, cW2, cb2, cW3, cb3):
    s_g = np.zeros((G_GRAPHS, D), np.float64)
    mx_g = np.zeros((G_GRAPHS, D), np.float64)
    for c in range(NC):
        s_part = results[c]["s_out"].astype(np.float64).T    # [128 relg, 128 f]
        m_part = results[c]["mx_out"].astype(np.float64).T
        g0 = int(g_lo[c])
        hi = min(128, G_GRAPHS - g0)
        s_g[g0:g0 + hi] += s_part[:hi]
        mx_g[g0:g0 + hi] = np.maximum(mx_g[g0:g0 + hi], m_part[:hi])
    cnt = np.bincount(np.asarray(batch, np.int64), minlength=G_GRAPHS).astype(
        np.float64)
    mean = s_g / np.maximum(cnt, 1.0)[:, None]
    z = np.concatenate([s_g, mean, mx_g], axis=-1).astype(np.float32)

    def bn(v, g, b):
        m = v.mean(0)
        var = v.var(0)
        return (v - m) / np.sqrt(var + BN_EPS) * g + b

    z = np.maximum(bn(z @ cW1 + cb1, cg, cbeta), 0.0)
    z = np.maximum(z @ cW2 + cb2, 0.0)
    return (z @ cW3 + cb3).astype(np.float32)


# ---------------- input packing ----------------

def _pack_inputs(x, batch, W1, b1, g1, be1, W2, b2, gbn, bbn, eps,
                 idx16, dstrel_up, batchrel, sel, mask):
    x = np.asarray(x, np.float32)
    rep = lambda a: np.broadcast_to(np.asarray(a, np.float32), (128,) + np.asarray(a).shape).copy()
    w1u = np.ascontiguousarray(
        np.asarray(W1, np.float32).reshape(L_LAYERS, 128, 2, 128)
        .transpose(1, 0, 2, 3))
    w2u = np.ascontiguousarray(
        np.asarray(W2, np.float32).reshape(L_LAYERS, 2, 128, 128)
        .transpose(2, 0, 1, 3))
    b1u = np.ascontiguousarray(
        np.asarray(b1, np.float32).reshape(L_LAYERS, 2, 128).transpose(2, 0, 1))
    g1u = np.ascontiguousarray(
        np.asarray(g1, np.float32).reshape(L_LAYERS, 2, 128).transpose(2, 0, 1))
    be1u = np.ascontiguousarray(
        np.asarray(be1, np.float32).reshape(L_LAYERS, 2, 128).transpose(2, 0, 1))
    b2u = np.ascontiguousarray(np.asarray(b2, np.float32).T)
    gbnu = np.ascontiguousarray(np.asarray(gbn, np.float32).T)
    bbnu = np.ascontiguousarray(np.asarray(bbn, np.float32).T)
    eps1u = rep(1.0 + np.asarray(eps, np.float32))

    in_maps = []
    for c in range(NC):
        xs = np.zeros((128, M), np.float32)
        xs[:, :NPC] = x[c * NPC:(c + 1) * NPC].T
        in_maps.append({
            "xT0": xs,
            "w1u": w1u, "w2u": w2u, "b1u": b1u, "g1u": g1u, "be1u": be1u,
            "b2u": b2u, "gbnu": gbnu, "bbnu": bbnu, "eps1u": eps1u,
            "dstrelu": dstrel_up[c], "batchrelu": batchrel[c], "selu": sel[c],
            "masku": mask[c], "idx16u": idx16[c],
        })
    return in_maps


# ---------------- execution + timing ----------------

LAST_EXEC_NS = None
LAST_WALL_NS = None
LAST_RESULTS = None


@contextlib.contextmanager
def _ntff_profile(output_dir, device_ids=None):
    lib = ctypes.CDLL('/opt/axon/libaxon_pjrt.so')
    lib.axon_start_nrt_profile.argtypes = [ctypes.POINTER(ctypes.c_int64),
                                           ctypes.c_size_t]
    lib.axon_start_nrt_profile.restype = ctypes.c_int64
    lib.axon_stop_nrt_profile.argtypes = [ctypes.c_char_p]
    lib.axon_stop_nrt_profile.restype = ctypes.c_int64
    import jax
    jax.devices()
    if device_ids:
        ids = (ctypes.c_int64 * len(device_ids))(*device_ids)
        rc = lib.axon_start_nrt_profile(ids, len(device_ids))
    else:
        rc = lib.axon_start_nrt_profile(None, 0)
    if rc != 0:
        raise RuntimeError(f"axon_start_nrt_profile rc={rc}")
    try:
        yield
    finally:
        n = lib.axon_stop_nrt_profile(str(output_dir).encode())
        if n <= 0:
            print(f"ntff profile: {n} files written", file=sys.stderr)


def _ntff_exec_ns(nc, outdir, cores=(0,)):
    """Convert captured NTFFs and return max on-device exec time (ns)."""
    import gauge.profiler
    from concourse._compat import FishPath
    profile = gauge.profiler.Profile(
        profile_path=FishPath(outdir),
        kernel_dev_mode=True,
        profile_on_exit=False,
        bass_kernel=nc.m,
        offline_processing=True,
        fname="*_body*",
    )
    results = profile.to_perfetto(model_index=tuple(cores))
    return max(int(r.exec_time_ns) for r in results)


def _timed_spmd(nc, in_maps, n_reps=3, profile=True):
    """Run the SPMD program via PJRT with device-resident inputs.

    Returns (results, wall_ns, times). Also captures an NTFF (neuron-profile)
    trace of one execution to measure true on-device time when possible.
    """
    import jax
    import numpy as _np
    from jax.sharding import Mesh, PartitionSpec, NamedSharding
    from jax.experimental.shard_map import shard_map
    import concourse.mybir as mybir
    from concourse import bass2jax

    bass2jax.install_neuronx_cc_hook()
    n_cores = len(in_maps)
    partition_name = (nc.partition_id_tensor.name
                      if nc.partition_id_tensor else None)
    in_names, out_names, out_avals, zero_outs = [], [], [], []
    for alloc in nc.m.functions[0].allocations:
        if not isinstance(alloc, mybir.MemoryLocationSet):
            continue
        name = alloc.memorylocations[0].name
        if alloc.kind == "ExternalInput":
            if name != partition_name:
                in_names.append(name)
        elif alloc.kind == "ExternalOutput":
            out_names.append(name)
            shape = tuple(alloc.tensor_shape)
            dtype = mybir.dt.np(alloc.dtype)
            out_avals.append(jax.core.ShapedArray(shape, dtype))
            zero_outs.append(_np.zeros(shape, dtype))
    n_params = len(in_names)
    n_outs = len(out_avals)
    all_in_names = list(in_names) + out_names
    if partition_name is not None:
        all_in_names.append(partition_name)

    def _body(*args):
        operands = list(args)
        if partition_name is not None:
            operands.append(bass2jax.partition_id_tensor())
        outs = bass2jax._bass_exec_p.bind(
            *operands, out_avals=tuple(out_avals),
            in_names=tuple(all_in_names), out_names=tuple(out_names),
            lowering_input_output_aliases=(),
            sim_require_finite=True, sim_require_nnan=True, nc=nc)
        return tuple(outs)

    devices = jax.devices()[:n_cores]
    mesh = Mesh(_np.asarray(devices), ("core",))
    sharded = jax.jit(
        shard_map(_body, mesh=mesh,
                  in_specs=(PartitionSpec("core"),) * (n_params + n_outs),
                  out_specs=(PartitionSpec("core"),) * n_outs,
                  check_rep=False),
        keep_unused=True)
    sh = NamedSharding(mesh, PartitionSpec("core"))
    concat_in = [
        jax.device_put(_np.concatenate(
            [_np.asarray(in_maps[c][nm]) for c in range(n_cores)], axis=0), sh)
        for nm in in_names]

    def one_call():
        zeros = [_np.zeros((n_cores * z.shape[0], *z.shape[1:]), z.dtype)
                 for z in zero_outs]
        t0 = time.perf_counter()
        out_arrs = sharded(*concat_in, *zeros)
        jax.block_until_ready(out_arrs)
        return out_arrs, time.perf_counter() - t0

    times = []
    out_arrs = None
    for _ in range(n_reps):
        out_arrs, dt = one_call()
        times.append(dt)
    wall_ns = int(min(times) * 1e9)

    exec_ns = None
    if profile and os.environ.get("GIN_NO_PROFILE", "0") != "1":
        try:
            import tempfile
            outdir = tempfile.mkdtemp(prefix="gin_ntff_")
            with _ntff_profile(outdir, list(range(n_cores))):
                out_arrs, _ = one_call()
            exec_ns = _ntff_exec_ns(nc, outdir)
        except Exception as e:
            print("ntff profiling failed:", e, file=sys.stderr)

    results = [
        {nm: _np.asarray(out_arrs[i]).reshape(n_cores, *out_avals[i].shape)[c]
         for i, nm in enumerate(out_names)}
        for c in range(n_cores)]
    return results, wall_ns, times, exec_ns


def kernel(x, edge_index, batch, num_graphs, W1, b1, g1, be1, W2, b2, gbn,
           bbn, eps, cW1, cb1, cg, cbeta, cW2, cb2, cW3, cb3):
    x = np.asarray(x)
    meta, idx16, dstrel_up, batchrel, sel, mask = _preprocess(
        x, np.asarray(edge_index), np.asarray(batch))
    in_maps = _pack_inputs(x, batch, W1, b1, g1, be1, W2, b2, gbn, bbn, eps,
                           idx16, dstrel_up, batchrel, sel, mask)
    nc = _build_program(meta)
    global LAST_RESULTS, LAST_EXEC_NS, LAST_WALL_NS
    try:
        results, wall_ns, times, exec_ns = _timed_spmd(nc, in_maps)
        LAST_WALL_NS = wall_ns
        LAST_EXEC_NS = exec_ns if exec_ns is not None else wall_ns
        print("timed calls (s):", [f"{t:.4f}" for t in times])
        if exec_ns is not None:
            print(f"on-device exec (ntff): {exec_ns} ns; "
                  f"wall-clock min: {wall_ns} ns")
        LAST_RESULTS = results
    except Exception as e:
        print("timed runner failed, falling back:", e)
        from concourse.bass_utils import run_bass_kernel_spmd
        res = run_bass_kernel_spmd(nc, in_maps, core_ids=list(range(NC)))
        LAST_RESULTS = res.results
    return _postprocess(LAST_RESULTS, meta["g_lo"], batch,
                        np.asarray(cW1, np.float32), np.asarray(cb1, np.float32),
                        np.asarray(cg, np.float32), np.asarray(cbeta, np.float32),
                        np.asarray(cW2, np.float32), np.asarray(cb2, np.float32),
                        np.asarray(cW3, np.float32), np.asarray(cb3, np.float32))


# revision 14
# speedup vs baseline: 38.3360x; 1.1160x over previous
"""Trainium2 Bass kernel for BaselineGIN (nn_BaselineGIN_42502996361221).

Strategy (8 NeuronCores, SPMD):
  - Nodes sharded 12500/core, padded to 12544 (=98*128) columns per core.
  - Node features live transposed in SBUF: xT [128 feat, 12544 nodes] fp32.
  - Full node table x_full [100352, 128] bf16 in Shared DRAM, rebuilt by
    AllGather before every layer (bf16 halves gather traffic).
  - Edges partitioned by dst owner; dst space split into 512-wide windows
    (one PSUM bank per window), sources into 4 buckets of 25088 rows
    (int16 bucket-relative indices). Edges sorted by (window, bucket),
    packed into 128-row tiles per (window, bucket) cell, gathered with
    dma_gather round-robined over 4 SWDGE queues, then scatter-added into
    [128 feat, 512 dst] PSUM windows via one-hot matmuls (one-hot built
    on DVE as is_equal(iota512, dstrel), fp16 in / bf16 out).
  - GIN MLP interleaved with the scatter at 512-column granularity;
    matmuls in float32r (1 cycle/row at >=256 free dim). BatchNorm batch
    stats via per-core accumulators + tiny AllReduce; BN1 z recomputed
    after the stats AllReduce so Z1 is never materialized full-width.
  - Readout: segment sums via one-hot(batch) matmuls over PE-transposed
    chunks; segment max via tensor_tensor_scan (mult-mask reset, data>=0)
    plus one-hot(last-node) extraction matmuls. Host combines the per-core
    partials (straddled graphs) and runs the tiny [512,384] classifier.

kernel() accepts the FULL inputs and returns the FULL [512, 2] output.
LAST_EXEC_NS is the on-device NEFF execution time measured via NRT/NTFF
profiling (neuron-profile) when available; wall-clock min otherwise.
"""

import contextlib
import ctypes
import math
import os
import sys
import time

import numpy as np

# ---------------- problem constants (hardcoded per spec) ----------------
N_NODES = 100000
N_EDGES = 500000
D = 128
G_GRAPHS = 512
L_LAYERS = 4
NC = 8                      # cores
NPC = N_NODES // NC         # real nodes per core = 12500
M = 12544                   # padded per-core columns (98 * 128)
NW = M // 128               # 98 narrow windows (pooling)
W5 = 512                    # scatter dst-window width (one PSUM bank)
NW5 = math.ceil(NPC / W5)   # 25 scatter windows
NBUCK = 4
BUCK = NC * M // NBUCK      # 25088 padded rows per src bucket (< 32768)
BN_EPS = 1e-5
WGRP = 4                    # scatter windows per gather-call group
NQ = 4                      # SWDGE queues (ucode max)
CALL_MAX = 8                # tiles per dma_gather call (1024-desc ring)

USE_BF16_X = True           # x_full table + scatter matmuls in bf16
USE_F32R_MLP = True         # MLP matmuls in float32r (1 cyc/row @ >=256)


# ---------------- host preprocessing ----------------

def _pad_id(n):
    """node id -> padded global row id"""
    return (n // NPC) * M + (n % NPC)


def _preprocess(x, edge_index, batch):
    src = np.asarray(edge_index[0], dtype=np.int64)
    dst = np.asarray(edge_index[1], dtype=np.int64)
    batch = np.asarray(batch, dtype=np.int64)

    src_pad = _pad_id(src)
    core = dst // NPC
    dstl = dst - core * NPC            # local dst 0..12499
    win = dstl // W5
    dstrel_all = dstl % W5
    buck = src_pad // BUCK

    # per (core, window, bucket) edge lists
    per = [[[None] * NBUCK for _ in range(NW5)] for _ in range(NC)]
    for c in range(NC):
        m = core == c
        sp, dr, w, b = src_pad[m], dstrel_all[m], win[m], buck[m]
        key = w * NBUCK + b
        order = np.argsort(key, kind="stable")
        sp, dr, key = sp[order], dr[order], key[order]
        bounds = np.searchsorted(key, np.arange(NW5 * NBUCK + 1))
        for wi in range(NW5):
            for bi in range(NBUCK):
                lo, hi = bounds[wi * NBUCK + bi], bounds[wi * NBUCK + bi + 1]
                per[c][wi][bi] = (sp[lo:hi], dr[lo:hi])

    # uniform tiles-per-(window,bucket): max over cores
    T = np.zeros((NW5, NBUCK), np.int64)
    for wi in range(NW5):
        for bi in range(NBUCK):
            cnt = max(len(per[c][wi][bi][0]) for c in range(NC))
            T[wi, bi] = math.ceil(cnt / 128)

    # gather calls: per window-group, per bucket, calls of <=8 tiles
    wgroups = [list(range(g, min(g + WGRP, NW5))) for g in range(0, NW5, WGRP)]
    calls = []          # (bucket, ntiles)
    wg_calls = []       # per wgroup: list of call ids
    tile_slot = {}      # (w, b, k) -> (call_id, pos_in_call)
    ntiles_total = 0
    for grp in wgroups:
        ids = []
        for bi in range(NBUCK):
            pend = []
            for wi in grp:
                for k in range(int(T[wi, bi])):
                    pend.append((wi, k))
            for c0 in range(0, len(pend), CALL_MAX):
                chunk = pend[c0:c0 + CALL_MAX]
                cid = len(calls)
                ids.append(cid)
                for pos, (wi, k) in enumerate(chunk):
                    tile_slot[(wi, bi, k)] = (cid, pos)
                calls.append((bi, len(chunk)))
                ntiles_total += len(chunk)
        wg_calls.append(ids)
    call_start = np.cumsum([0] + [c[1] for c in calls])

    # per-core uploads: idx16 (wrapped [16, NT*8], replicated to 128 rows)
    # + dstrel (f32) in slot order
    idx16 = np.zeros((NC, 128, ntiles_total * 8), np.int16)
    dstrel_up = np.full((NC, 128, ntiles_total), -1.0, np.float32)
    for c in range(NC):
        idx_flat = np.zeros(ntiles_total * 128, np.int16)
        dr_flat = np.full(ntiles_total * 128, -1.0, np.float32)
        for wi in range(NW5):
            for bi in range(NBUCK):
                sp, dr = per[c][wi][bi]
                loc = (sp - bi * BUCK).astype(np.int16)
                for k in range(int(T[wi, bi])):
                    cid, pos = tile_slot[(wi, bi, k)]
                    s0 = (call_start[cid] + pos) * 128
                    seg = slice(k * 128, min((k + 1) * 128, len(sp)))
                    n = seg.stop - seg.start
                    if n > 0:
                        idx_flat[s0:s0 + n] = loc[seg]
                        dr_flat[s0:s0 + n] = dr[seg].astype(np.float32)
                    # pad slots keep idx 0 (valid) and dstrel -1
        idx16[c] = np.tile(idx_flat.reshape(ntiles_total * 8, 16).T, (8, 1))
        dstrel_up[c] = dr_flat.reshape(ntiles_total, 128).T

    # pooling metadata (128-wide windows)
    g_lo = np.zeros(NC, np.int64)
    batchrel = np.full((NC, 128, NW), -1.0, np.float32)
    sel = np.full((NC, 128, NW), -1.0, np.float32)
    mask = np.ones((NC, 128, M), np.float32)
    for c in range(NC):
        bl = batch[c * NPC:(c + 1) * NPC]
        g_lo[c] = bl[0]
        rel = (bl - g_lo[c]).astype(np.float32)
        assert rel.max() < 128, "per-core graph span exceeds 128"
        br = np.full(M, -1.0, np.float32)
        br[:NPC] = rel
        batchrel[c] = br.reshape(NW, 128).T
        last = np.zeros(NPC, bool)
        last[-1] = True
        last[:-1] = bl[1:] != bl[:-1]
        sv = np.full(M, -1.0, np.float32)
        sv[:NPC][last] = rel[last]
        sel[c] = sv.reshape(NW, 128).T
        first = np.zeros(NPC, bool)
        first[0] = True
        first[1:] = bl[1:] != bl[:-1]
        mk = np.ones(M, np.float32)
        mk[:NPC][first] = 0.0
        mask[c] = np.broadcast_to(mk, (128, M))

    meta = dict(calls=calls, call_start=call_start, tile_slot=tile_slot,
                T=T, wgroups=wgroups, wg_calls=wg_calls,
                ntiles_total=ntiles_total, g_lo=g_lo,
                max_calls_grp=max(len(ids) for ids in wg_calls))
    return meta, idx16, dstrel_up, batchrel, sel, mask


# ---------------- device program ----------------

def _build_program(meta):
    import concourse.bass as bass
    import concourse.bacc as bacc
    import concourse.tile as tile
    import concourse.mybir as mybir
    from concourse.masks import make_identity

    F32 = mybir.dt.float32
    F32R = mybir.dt.float32r
    BF16 = mybir.dt.bfloat16
    FP16 = mybir.dt.float16
    I16 = mybir.dt.int16
    AF = mybir.ActivationFunctionType
    OP = mybir.AluOpType

    DT_X = BF16 if USE_BF16_X else F32
    calls = meta["calls"]
    call_start = meta["call_start"]
    tile_slot = meta["tile_slot"]
    T = meta["T"]
    wgroups = meta["wgroups"]
    wg_calls = meta["wg_calls"]
    NT = meta["ntiles_total"]

    nc = bacc.Bacc("TRN2", target_bir_lowering=False, debug=False,
                   num_devices=NC, num_swdge_queues=NQ)

    ein = lambda n, s: nc.dram_tensor(n, s, F32, kind="ExternalInput")
    xT0_d = ein("xT0", [128, M])
    w1_d = ein("w1u", [128, L_LAYERS, 2, 128])
    w2_d = ein("w2u", [128, L_LAYERS, 2, 128])
    b1_d = ein("b1u", [128, L_LAYERS, 2])
    g1_d = ein("g1u", [128, L_LAYERS, 2])
    be1_d = ein("be1u", [128, L_LAYERS, 2])
    b2_d = ein("b2u", [128, L_LAYERS])
    gbn_d = ein("gbnu", [128, L_LAYERS])
    bbn_d = ein("bbnu", [128, L_LAYERS])
    eps1_d = ein("eps1u", [128, L_LAYERS])
    batchrel_d = ein("batchrelu", [128, NW])
    sel_d = ein("selu", [128, NW])
    mask_d = ein("masku", [128, M])
    dstrel_d = ein("dstrelu", [128, NT])
    idx_d = nc.dram_tensor("idx16u", [128, NT * 8], I16, kind="ExternalInput")

    s_out_d = nc.dram_tensor("s_out", [128, 128], F32, kind="ExternalOutput")
    mx_out_d = nc.dram_tensor("mx_out", [128, 128], F32, kind="ExternalOutput")

    def mmdt(ap):
        return ap.bitcast(F32R) if USE_F32R_MLP else ap

    with tile.TileContext(nc) as tc:
        with (
            tc.tile_pool(name="persist", bufs=1) as pp,
            tc.tile_pool(name="dram", bufs=1, space="DRAM") as dpool,
        ):
            # ---- persistent SBUF ----
            xT = pp.tile([128, M], F32, name="bufA")
            hT = pp.tile([128, M], F32, name="bufB")
            w1_sb = pp.tile([128, L_LAYERS, 2, 128], F32)
            w2_sb = pp.tile([128, L_LAYERS, 2, 128], F32)
            b1_sb = pp.tile([128, L_LAYERS, 2], F32)
            g1_sb = pp.tile([128, L_LAYERS, 2], F32)
            be1_sb = pp.tile([128, L_LAYERS, 2], F32)
            b2_sb = pp.tile([128, L_LAYERS], F32)
            gbn_sb = pp.tile([128, L_LAYERS], F32)
            bbn_sb = pp.tile([128, L_LAYERS], F32)
            eps1_sb = pp.tile([128, L_LAYERS], F32)
            dstrel_sb = pp.tile([128, NT], F32)
            idx_sb = pp.tile([128, NT * 8], I16)
            iota5 = pp.tile([128, W5], FP16)
            iota_sb = pp.tile([128, 128], F32)
            ident = pp.tile([128, 128], F32)

            for sb_t, d_t in [(xT, xT0_d), (w1_sb, w1_d), (w2_sb, w2_d),
                              (b1_sb, b1_d), (g1_sb, g1_d), (be1_sb, be1_d),
                              (b2_sb, b2_d), (gbn_sb, gbn_d), (bbn_sb, bbn_d),
                              (eps1_sb, eps1_d), (dstrel_sb, dstrel_d),
                              (idx_sb, idx_d)]:
                nc.sync.dma_start(out=sb_t[:], in_=d_t[:])

            # round weights to f32r once (BIR verifier: f32r matmul inputs
            # must come from rounding producers, DMA loads don't round)
            w1r = pp.tile([128, L_LAYERS, 2, 128], F32)
            nc.vector.tensor_copy(out=mmdt(w1r[:]), in_=w1_sb[:])
            w2r = pp.tile([128, L_LAYERS, 2, 128], F32)
            nc.vector.tensor_copy(out=mmdt(w2r[:]), in_=w2_sb[:])
            epsc = pp.tile([128, 1], F32)
            nc.vector.memset(epsc[:], BN_EPS)
            iota5_i = pp.tile([128, W5], mybir.dt.int32)
            nc.gpsimd.iota(iota5_i[:], pattern=[[1, W5]], base=0,
                           channel_multiplier=0)
            nc.vector.tensor_copy(out=iota5[:], in_=iota5_i[:])
            iota_i = pp.tile([128, 128], mybir.dt.int32)
            nc.gpsimd.iota(iota_i[:], pattern=[[1, 128]], base=0,
                           channel_multiplier=0)
            nc.vector.tensor_copy(out=iota_sb[:], in_=iota_i[:])
            make_identity(nc, ident[:])

            # ---- DRAM scratch ----
            x_fulls = [dpool.tile([NC * M, D], DT_X, addr_space="Shared",
                                  name=f"x_full{i}") for i in range(L_LAYERS)]
            xsh = dpool.tile([M, D], DT_X, name="xsh")
            ar_ins = [dpool.tile([128, 4], F32, name=f"ar_in{i}")
                      for i in range(2 * L_LAYERS)]
            ar_outs = [dpool.tile([128, 4], F32, name=f"ar_out{i}")
                       for i in range(2 * L_LAYERS)]
            ar_count = [0]
            xsh_v = xsh[:].rearrange("(t p) f -> p t f", p=128)

            def share(src_T, x_full):
                """transpose src_T -> row-major DT_X shard -> AllGather."""
                with (
                    tc.tile_pool(name="shps", bufs=2, space="PSUM") as sp_ps,
                    tc.tile_pool(name="shsb", bufs=3) as sp_sb,
                ):
                    for g in range(NW // 7):
                        xrow = sp_sb.tile([128, 7, 128], DT_X, tag="xrow")
                        for i in range(7):
                            ch = g * 7 + i
                            tp = sp_ps.tile([128, 128], F32, tag="tp")
                            nc.tensor.transpose(
                                out=tp[:], in_=src_T[:, ch * 128:(ch + 1) * 128],
                                identity=ident[:])
                            if i % 2 == 0:
                                nc.vector.tensor_copy(out=xrow[:, i, :], in_=tp[:])
                            else:
                                nc.scalar.activation(out=xrow[:, i, :], in_=tp[:],
                                                     func=AF.Copy)
                        nc.sync.dma_start(out=xsh_v[:, g * 7:(g + 1) * 7, :],
                                          in_=xrow[:])
                nc.gpsimd.collective_compute(
                    "AllGather", OP.bypass,
                    replica_groups=[list(range(NC))],
                    ins=[xsh[:].opt()], outs=[x_full[:].opt()])

            def allreduce_stats(stats_sb):
                i = ar_count[0]
                ar_count[0] += 1
                nc.sync.dma_start(out=ar_ins[i][:], in_=stats_sb[:])
                nc.gpsimd.collective_compute(
                    "AllReduce", OP.add, replica_groups=[list(range(NC))],
                    ins=[ar_ins[i][:].opt()], outs=[ar_outs[i][:].opt()])
                glob = stat_pool.tile([128, 4], F32, tag="glob")
                nc.sync.dma_start(out=glob[:], in_=ar_outs[i][:])
                return glob

            with tc.tile_pool(name="stats", bufs=2) as stat_pool:
                share(xT, x_fulls[0])  # x_full <- layer-0 input

                for layer in range(L_LAYERS):
                    # ==== scatter (agg -> hT) + MLP pass A, interleaved ====
                    with (
                        tc.tile_pool(name="gb", bufs=26) as gb_pool,
                        tc.tile_pool(name="oh", bufs=10) as oh_pool,
                        tc.tile_pool(name="aggps", bufs=3, space="PSUM") as agg_ps,
                        tc.tile_pool(name="zps", bufs=2, space="PSUM") as z_ps,
                        tc.tile_pool(name="msb", bufs=3) as m_sb,
                    ):
                        sum_p = stat_pool.tile([128, 2, NW5], F32, tag="sum1")
                        sq_p = stat_pool.tile([128, 2, NW5], F32, tag="sq1")
                        gbufs = {}

                        def emit_calls(gi):
                            for cid in wg_calls[gi]:
                                b_, ntl = calls[cid]
                                gb = gb_pool.tile([128, CALL_MAX, 128], DT_X,
                                                  tag="gb", name="gb")
                                nc.gpsimd.dma_gather(
                                    out_ap=gb[:, :ntl, :],
                                    in_ap=x_fulls[layer][b_ * BUCK:(b_ + 1) * BUCK, :],
                                    idxs_ap=idx_sb[:, call_start[cid] * 8:
                                                   (call_start[cid] + ntl) * 8],
                                    num_idxs=ntl * 128,
                                    num_idxs_reg=ntl * 128,
                                    elem_size=D,
                                    queue_num=cid % NQ)
                                gbufs[cid] = gb

                        def pass_a(wi):
                            c0 = wi * W5
                            cw = min(W5, M - c0)
                            for b in range(2):
                                zp = z_ps.tile([128, W5], F32, tag="z")
                                nc.tensor.matmul(
                                    out=zp[:, :cw],
                                    lhsT=mmdt(w1r[:, layer, b, :]),
                                    rhs=mmdt(hT[:, c0:c0 + cw]),
                                    start=True, stop=True)
                                zs = m_sb.tile([128, W5], F32, tag="zs")
                                nc.scalar.activation(
                                    out=zs[:, :cw], in_=zp[:, :cw],
                                    func=AF.Identity,
                                    bias=b1_sb[:, layer, b:b + 1],
                                    accum_out=sum_p[:, b, wi:wi + 1])
                                sq = m_sb.tile([128, W5], F32, tag="sq")
                                nc.vector.scalar_tensor_tensor(
                                    out=sq[:, :cw], in0=zs[:, :cw],
                                    scalar=1.0, in1=zs[:, :cw],
                                    op0=OP.mult, op1=OP.mult,
                                    accum_out=sq_p[:, b, wi:wi + 1])

                        emit_calls(0)
                        for gi, grp in enumerate(wgroups):
                            if gi + 1 < len(wgroups):
                                emit_calls(gi + 1)
                            for wi in grp:
                                c0 = wi * W5
                                cw = min(W5, M - c0)
                                ps = agg_ps.tile([128, W5], F32, tag="agg")
                                nmm = 0
                                tot = int(T[wi].sum())
                                for bi in range(NBUCK):
                                    for k in range(int(T[wi, bi])):
                                        cid, pos = tile_slot[(wi, bi, k)]
                                        slot = call_start[cid] + pos
                                        oh = oh_pool.tile([128, W5], DT_X,
                                                          tag="oh")
                                        nc.vector.tensor_scalar(
                                            out=oh[:], in0=iota5[:],
                                            scalar1=dstrel_sb[:, slot:slot + 1],
                                            scalar2=None, op0=OP.is_equal)
                                        nc.tensor.matmul(
                                            out=ps[:],
                                            lhsT=gbufs[cid][:, pos, :],
                                            rhs=oh[:],
                                            start=(nmm == 0),
                                            stop=(nmm == tot - 1))
                                        nmm += 1
                                # h = (1+eps)*x + agg
                                nc.vector.scalar_tensor_tensor(
                                    out=mmdt(hT[:, c0:c0 + cw]),
                                    in0=xT[:, c0:c0 + cw],
                                    scalar=eps1_sb[:, layer:layer + 1],
                                    in1=ps[:, :cw], op0=OP.mult, op1=OP.add)
                                # MLP pass A deferred one window so PE never
                                # stalls on the fresh hT write
                                if wi > 0:
                                    pass_a(wi - 1)
                        pass_a(NW5 - 1)
                        st = stat_pool.tile([128, 4], F32, tag="pack")
                        nc.vector.tensor_reduce(
                            out=st[:, 0:2], in_=sum_p[:], axis=mybir.AxisListType.X,
                            op=OP.add)
                        nc.vector.tensor_reduce(
                            out=st[:, 2:4], in_=sq_p[:], axis=mybir.AxisListType.X,
                            op=OP.add)
                        glob = allreduce_stats(st)
                        # A = g1*rsqrt(var+eps); B = (b1-mean)*A + be1
                        mean = stat_pool.tile([128, 2], F32, tag="mean")
                        nc.vector.tensor_scalar(
                            out=mean[:], in0=glob[:, 0:2], scalar1=1.0 / N_NODES,
                            scalar2=None, op0=OP.mult)
                        var = stat_pool.tile([128, 2], F32, tag="var")
                        nc.vector.tensor_scalar(
                            out=var[:], in0=glob[:, 2:4], scalar1=1.0 / N_NODES,
                            scalar2=None, op0=OP.mult)
                        msq = stat_pool.tile([128, 2], F32, tag="msq")
                        nc.vector.tensor_tensor(
                            out=msq[:], in0=mean[:], in1=mean[:], op=OP.mult)
                        nc.vector.tensor_tensor(
                            out=var[:], in0=var[:], in1=msq[:], op=OP.subtract)
                        sd = stat_pool.tile([128, 2], F32, tag="sd")
                        nc.scalar.activation(out=sd[:], in_=var[:], func=AF.Sqrt,
                                             bias=epsc[:, 0:1])
                        rs = stat_pool.tile([128, 2], F32, tag="rs")
                        nc.vector.reciprocal(out=rs[:], in_=sd[:])
                        A1 = stat_pool.tile([128, 2], F32, tag="A1")
                        nc.vector.tensor_tensor(
                            out=A1[:], in0=rs[:], in1=g1_sb[:, layer, :],
                            op=OP.mult)
                        B1 = stat_pool.tile([128, 2], F32, tag="B1")
                        nc.vector.tensor_tensor(
                            out=B1[:], in0=b1_sb[:, layer, :], in1=mean[:],
                            op=OP.subtract)
                        nc.vector.tensor_tensor(
                            out=B1[:], in0=B1[:], in1=A1[:], op=OP.mult)
                        nc.vector.tensor_tensor(
                            out=B1[:], in0=B1[:], in1=be1_sb[:, layer, :],
                            op=OP.add)

                    # ==== MLP pass B: recompute z1, relu, h2, BN2 stats ====
                    with (
                        tc.tile_pool(name="zps2", bufs=2, space="PSUM") as z_ps,
                        tc.tile_pool(name="h2ps", bufs=2, space="PSUM") as h2_ps,
                        tc.tile_pool(name="msb2", bufs=3) as m_sb,
                    ):
                        sum2 = stat_pool.tile([128, NW5], F32, tag="sum2")
                        sq2 = stat_pool.tile([128, NW5], F32, tag="sq2")
                        for wi in range(NW5):
                            c0 = wi * W5
                            cw = min(W5, M - c0)
                            zr = [None, None]
                            for b in range(2):
                                zp = z_ps.tile([128, W5], F32, tag="z2")
                                nc.tensor.matmul(
                                    out=zp[:, :cw],
                                    lhsT=mmdt(w1r[:, layer, b, :]),
                                    rhs=mmdt(hT[:, c0:c0 + cw]),
                                    start=True, stop=True)
                                zr[b] = m_sb.tile([128, W5], F32,
                                                  tag=f"zr{b}", name=f"zr{b}")
                                nc.scalar.activation(
                                    out=mmdt(zr[b][:, :cw]), in_=zp[:, :cw],
                                    func=AF.Relu,
                                    bias=B1[:, b:b + 1],
                                    scale=A1[:, b:b + 1])
                            hp = h2_ps.tile([128, W5], F32, tag="h2")
                            for b in range(2):
                                nc.tensor.matmul(
                                    out=hp[:, :cw],
                                    lhsT=mmdt(w2r[:, layer, b, :]),
                                    rhs=mmdt(zr[b][:, :cw]),
                                    start=(b == 0), stop=(b == 1))
                            nc.scalar.activation(
                                out=xT[:, c0:c0 + cw],
                                in_=hp[:, :cw], func=AF.Identity,
                                bias=b2_sb[:, layer:layer + 1],
                                accum_out=sum2[:, wi:wi + 1])
                            sq = m_sb.tile([128, W5], F32, tag="sqb")
                            nc.vector.scalar_tensor_tensor(
                                out=sq[:, :cw], in0=xT[:, c0:c0 + cw],
                                scalar=1.0, in1=xT[:, c0:c0 + cw],
                                op0=OP.mult, op1=OP.mult,
                                accum_out=sq2[:, wi:wi + 1])
                        st = stat_pool.tile([128, 4], F32, tag="pack")
                        nc.vector.tensor_reduce(
                            out=st[:, 0:1], in_=sum2[:], axis=mybir.AxisListType.X,
                            op=OP.add)
                        nc.vector.tensor_reduce(
                            out=st[:, 2:3], in_=sq2[:], axis=mybir.AxisListType.X,
                            op=OP.add)
                        nc.vector.memset(st[:, 1:2], 0.0)
                        nc.vector.memset(st[:, 3:4], 0.0)
                        glob = allreduce_stats(st)
                        mean = stat_pool.tile([128, 1], F32, tag="mean2")
                        nc.vector.tensor_scalar(
                            out=mean[:], in0=glob[:, 0:1], scalar1=1.0 / N_NODES,
                            scalar2=None, op0=OP.mult)
                        var = stat_pool.tile([128, 1], F32, tag="var2")
                        nc.vector.tensor_scalar(
                            out=var[:], in0=glob[:, 2:3], scalar1=1.0 / N_NODES,
                            scalar2=None, op0=OP.mult)
                        msq = stat_pool.tile([128, 1], F32, tag="msq2")
                        nc.vector.tensor_tensor(
                            out=msq[:], in0=mean[:], in1=mean[:], op=OP.mult)
                        nc.vector.tensor_tensor(
                            out=var[:], in0=var[:], in1=msq[:], op=OP.subtract)
                        sd = stat_pool.tile([128, 1], F32, tag="sd2")
                        nc.scalar.activation(out=sd[:], in_=var[:], func=AF.Sqrt,
                                             bias=epsc[:, 0:1])
                        rs = stat_pool.tile([128, 1], F32, tag="rs2")
                        nc.vector.reciprocal(out=rs[:], in_=sd[:])
                        A2 = stat_pool.tile([128, 1], F32, tag="A2")
                        nc.vector.tensor_tensor(
                            out=A2[:], in0=rs[:], in1=gbn_sb[:, layer:layer + 1],
                            op=OP.mult)
                        B2 = stat_pool.tile([128, 1], F32, tag="B2")
                        nc.vector.tensor_tensor(
                            out=B2[:], in0=mean[:], in1=A2[:], op=OP.mult)
                        nc.vector.tensor_tensor(
                            out=B2[:], in0=bbn_sb[:, layer:layer + 1], in1=B2[:],
                            op=OP.subtract)
                        # x_new = relu(h2*A2 + B2) in place on xT (h2 lives
                        # there, pre-BN, with b2 added; B2 accounts for it)
                        nc.scalar.activation(
                            out=xT[:, :], in_=xT[:, :], func=AF.Relu,
                            bias=B2[:, 0:1], scale=A2[:, 0:1])
                        nc.vector.memset(xT[:, NPC:M], 0.0)

                    if layer < L_LAYERS - 1:
                        share(xT, x_fulls[layer + 1])

                # ======== pooling ========
                with (
                    tc.tile_pool(name="poolps", bufs=2, space="PSUM") as tp_ps,
                    tc.tile_pool(name="accps", bufs=1, space="PSUM") as acc_ps,
                    tc.tile_pool(name="poolsb", bufs=4) as po_sb,
                ):
                    mask_sb = po_sb.tile([128, M], F32, name="mask_sb", bufs=1)
                    nc.sync.dma_start(out=mask_sb[:], in_=mask_d[:])
                    batchrel_sb = po_sb.tile([128, NW], F32, name="brl", bufs=1)
                    nc.sync.dma_start(out=batchrel_sb[:], in_=batchrel_d[:])
                    sel_sb = po_sb.tile([128, NW], F32, name="slb", bufs=1)
                    nc.sync.dma_start(out=sel_sb[:], in_=sel_d[:])

                    scan = hT  # reuse dead h buffer
                    nc.vector.tensor_tensor_scan(
                        out=mmdt(scan[:]), data0=mask_sb[:], data1=xT[:],
                        initial=0.0, op0=OP.mult, op1=OP.max)

                    s_ps = acc_ps.tile([128, 128], F32, tag="sacc")
                    m_ps = acc_ps.tile([128, 128], F32, tag="macc")
                    for ch in range(NW):
                        cs = ch * 128
                        tp1 = tp_ps.tile([128, 128], F32, tag="tp1")
                        nc.tensor.transpose(out=tp1[:], in_=xT[:, cs:cs + 128],
                                            identity=ident[:])
                        xrow = po_sb.tile([128, 128], F32, tag="xr")
                        nc.vector.tensor_copy(out=xrow[:], in_=tp1[:])
                        tp2 = tp_ps.tile([128, 128], F32, tag="tp2")
                        nc.tensor.transpose(out=tp2[:], in_=scan[:, cs:cs + 128],
                                            identity=ident[:])
                        srow = po_sb.tile([128, 128], F32, tag="sr")
                        nc.scalar.activation(out=srow[:], in_=tp2[:], func=AF.Copy)
                        ohB = po_sb.tile([128, 128], F32, tag="ohB")
                        nc.vector.tensor_scalar(
                            out=ohB[:], in0=iota_sb[:],
                            scalar1=batchrel_sb[:, ch:ch + 1], scalar2=None,
                            op0=OP.is_equal)
                        ohS = po_sb.tile([128, 128], F32, tag="ohS")
                        nc.vector.tensor_scalar(
                            out=ohS[:], in0=iota_sb[:],
                            scalar1=sel_sb[:, ch:ch + 1], scalar2=None,
                            op0=OP.is_equal)
                        nc.tensor.matmul(out=s_ps[:], lhsT=xrow[:],
                                         rhs=ohB[:], start=(ch == 0),
                                         stop=(ch == NW - 1))
                        nc.tensor.matmul(out=m_ps[:], lhsT=srow[:],
                                         rhs=ohS[:], start=(ch == 0),
                                         stop=(ch == NW - 1))
                    s_sb = po_sb.tile([128, 128], F32, tag="sfin")
                    nc.vector.tensor_copy(out=s_sb[:], in_=s_ps[:])
                    m_sb2 = po_sb.tile([128, 128], F32, tag="mfin")
                    nc.vector.tensor_copy(out=m_sb2[:], in_=m_ps[:])
                    nc.sync.dma_start(out=s_out_d[:, :], in_=s_sb[:])
                    nc.sync.dma_start(out=mx_out_d[:, :], in_=m_sb2[:])

    nc.compile()
    return nc


# ---------------- host postprocessing (classifier) ----------------

def _postprocess(results, g_lo, batch, cW1, cb1, cg, cbeta, cW2, cb2, cW3, cb3):
    s_g = np.zeros((G_GRAPHS, D), np.float64)
    mx_g = np.zeros((G_GRAPHS, D), np.float64)
    for c in range(NC):
        s_part = results[c]["s_out"].astype(np.float64).T    # [128 relg, 128 f]
        m_part = results[c]["mx_out"].astype(np.float64).T
        g0 = int(g_lo[c])
        hi = min(128, G_GRAPHS - g0)
        s_g[g0:g0 + hi] += s_part[:hi]
        mx_g[g0:g0 + hi] = np.maximum(mx_g[g0:g0 + hi], m_part[:hi])
    cnt = np.bincount(np.asarray(batch, np.int64), minlength=G_GRAPHS).astype(
        np.float64)
    mean = s_g / np.maximum(cnt, 1.0)[:, None]
    z = np.concatenate([s_g, mean, mx_g], axis=-1).astype(np.float32)

    def bn(v, g, b):
        m = v.mean(0)
        var = v.var(0)
        return (v - m) / np.sqrt(var + BN_EPS) * g + b

    z = np.maximum(bn(z @ cW1 + cb1, cg, cbeta), 0.0)
    z = np.maximum(z @ cW2 + cb2, 0.0)
    return (z @ cW3 + cb3).astype(np.float32)


# ---------------- input packing ----------------

def _pack_inputs(x, batch, W1, b1, g1, be1, W2, b2, gbn, bbn, eps,
                 idx16, dstrel_up, batchrel, sel, mask):
    x = np.asarray(x, np.float32)
    rep = lambda a: np.broadcast_to(np.asarray(a, np.float32), (128,) + np.asarray(a).shape).copy()
    w1u = np.ascontiguousarray(
        np.asarray(W1, np.float32).reshape(L_LAYERS, 128, 2, 128)
        .transpose(1, 0, 2, 3))
    w2u = np.ascontiguousarray(
        np.asarray(W2, np.float32).reshape(L_LAYERS, 2, 128, 128)
        .transpose(2, 0, 1, 3))
    b1u = np.ascontiguousarray(
        np.asarray(b1, np.float32).reshape(L_LAYERS, 2, 128).transpose(2, 0, 1))
    g1u = np.ascontiguousarray(
        np.asarray(g1, np.float32).reshape(L_LAYERS, 2, 128).transpose(2, 0, 1))
    be1u = np.ascontiguousarray(
        np.asarray(be1, np.float32).reshape(L_LAYERS, 2, 128).transpose(2, 0, 1))
    b2u = np.ascontiguousarray(np.asarray(b2, np.float32).T)
    gbnu = np.ascontiguousarray(np.asarray(gbn, np.float32).T)
    bbnu = np.ascontiguousarray(np.asarray(bbn, np.float32).T)
    eps1u = rep(1.0 + np.asarray(eps, np.float32))

    in_maps = []
    for c in range(NC):
        xs = np.zeros((128, M), np.float32)
        xs[:, :NPC] = x[c * NPC:(c + 1) * NPC].T
        in_maps.append({
            "xT0": xs,
            "w1u": w1u, "w2u": w2u, "b1u": b1u, "g1u": g1u, "be1u": be1u,
            "b2u": b2u, "gbnu": gbnu, "bbnu": bbnu, "eps1u": eps1u,
            "dstrelu": dstrel_up[c], "batchrelu": batchrel[c], "selu": sel[c],
            "masku": mask[c], "idx16u": idx16[c],
        })
    return in_maps


# ---------------- execution + timing ----------------

LAST_EXEC_NS = None
LAST_WALL_NS = None
LAST_RESULTS = None


@contextlib.contextmanager
def _ntff_profile(output_dir, device_ids=None):
    lib = ctypes.CDLL('/opt/axon/libaxon_pjrt.so')
    lib.axon_start_nrt_profile.argtypes = [ctypes.POINTER(ctypes.c_int64),
                                           ctypes.c_size_t]
    lib.axon_start_nrt_profile.restype = ctypes.c_int64
    lib.axon_stop_nrt_profile.argtypes = [ctypes.c_char_p]
    lib.axon_stop_nrt_profile.restype = ctypes.c_int64
    import jax
    jax.devices()
    if device_ids:
        ids = (ctypes.c_int64 * len(device_ids))(*device_ids)
        rc = lib.axon_start_nrt_profile(ids, len(device_ids))
    else:
        rc = lib.axon_start_nrt_profile(None, 0)
    if rc != 0:
        raise RuntimeError(f"axon_start_nrt_profile rc={rc}")
    try:
        yield
    finally:
        n = lib.axon_stop_nrt_profile(str(output_dir).encode())
        if n <= 0:
            print(f"ntff profile: {n} files written", file=sys.stderr)


def _ntff_exec_ns(nc, outdir, cores=(0,)):
    """Convert captured NTFFs and return max on-device exec time (ns)."""
    import gauge.profiler
    from concourse._compat import FishPath
    profile = gauge.profiler.Profile(
        profile_path=FishPath(outdir),
        kernel_dev_mode=True,
        profile_on_exit=False,
        bass_kernel=nc.m,
        offline_processing=True,
        fname="*_body*",
    )
    results = profile.to_perfetto(model_index=tuple(cores))
    return max(int(r.exec_time_ns) for r in results)


def _timed_spmd(nc, in_maps, n_reps=3, profile=True):
    """Run the SPMD program via PJRT with device-resident inputs.

    Returns (results, wall_ns, times). Also captures an NTFF (neuron-profile)
    trace of one execution to measure true on-device time when possible.
    """
    import jax
    import numpy as _np
    from jax.sharding import Mesh, PartitionSpec, NamedSharding
    from jax.experimental.shard_map import shard_map
    import concourse.mybir as mybir
    from concourse import bass2jax

    bass2jax.install_neuronx_cc_hook()
    n_cores = len(in_maps)
    partition_name = (nc.partition_id_tensor.name
                      if nc.partition_id_tensor else None)
    in_names, out_names, out_avals, zero_outs = [], [], [], []
    for alloc in nc.m.functions[0].allocations:
        if not isinstance(alloc, mybir.MemoryLocationSet):
            continue
        name = alloc.memorylocations[0].name
        if alloc.kind == "ExternalInput":
            if name != partition_name:
                in_names.append(name)
        elif alloc.kind == "ExternalOutput":
            out_names.append(name)
            shape = tuple(alloc.tensor_shape)
            dtype = mybir.dt.np(alloc.dtype)
            out_avals.append(jax.core.ShapedArray(shape, dtype))
            zero_outs.append(_np.zeros(shape, dtype))
    n_params = len(in_names)
    n_outs = len(out_avals)
    all_in_names = list(in_names) + out_names
    if partition_name is not None:
        all_in_names.append(partition_name)

    def _body(*args):
        operands = list(args)
        if partition_name is not None:
            operands.append(bass2jax.partition_id_tensor())
        outs = bass2jax._bass_exec_p.bind(
            *operands, out_avals=tuple(out_avals),
            in_names=tuple(all_in_names), out_names=tuple(out_names),
            lowering_input_output_aliases=(),
            sim_require_finite=True, sim_require_nnan=True, nc=nc)
        return tuple(outs)

    devices = jax.devices()[:n_cores]
    mesh = Mesh(_np.asarray(devices), ("core",))
    sharded = jax.jit(
        shard_map(_body, mesh=mesh,
                  in_specs=(PartitionSpec("core"),) * (n_params + n_outs),
                  out_specs=(PartitionSpec("core"),) * n_outs,
                  check_rep=False),
        keep_unused=True)
    sh = NamedSharding(mesh, PartitionSpec("core"))
    concat_in = [
        jax.device_put(_np.concatenate(
            [_np.asarray(in_maps[c][nm]) for c in range(n_cores)], axis=0), sh)
        for nm in in_names]

    def one_call():
        zeros = [_np.zeros((n_cores * z.shape[0], *z.shape[1:]), z.dtype)
                 for z in zero_outs]
        t0 = time.perf_counter()
        out_arrs = sharded(*concat_in, *zeros)
        jax.block_until_ready(out_arrs)
        return out_arrs, time.perf_counter() - t0

    times = []
    out_arrs = None
    for _ in range(n_reps):
        out_arrs, dt = one_call()
        times.append(dt)
    wall_ns = int(min(times) * 1e9)

    exec_ns = None
    if profile and os.environ.get("GIN_NO_PROFILE", "0") != "1":
        try:
            import tempfile
            outdir = tempfile.mkdtemp(prefix="gin_ntff_")
            with _ntff_profile(outdir, list(range(n_cores))):
                out_arrs, _ = one_call()
            exec_ns = _ntff_exec_ns(nc, outdir)
        except Exception as e:
            print("ntff profiling failed:", e, file=sys.stderr)

    results = [
        {nm: _np.asarray(out_arrs[i]).reshape(n_cores, *out_avals[i].shape)[c]
         for i, nm in enumerate(out_names)}
        for c in range(n_cores)]
    return results, wall_ns, times, exec_ns


def kernel(x, edge_index, batch, num_graphs, W1, b1, g1, be1, W2, b2, gbn,
           bbn, eps, cW1, cb1, cg, cbeta, cW2, cb2, cW3, cb3):
    x = np.asarray(x)
    meta, idx16, dstrel_up, batchrel, sel, mask = _preprocess(
        x, np.asarray(edge_index), np.asarray(batch))
    in_maps = _pack_inputs(x, batch, W1, b1, g1, be1, W2, b2, gbn, bbn, eps,
                           idx16, dstrel_up, batchrel, sel, mask)
    nc = _build_program(meta)
    global LAST_RESULTS, LAST_EXEC_NS, LAST_WALL_NS
    try:
        results, wall_ns, times, exec_ns = _timed_spmd(nc, in_maps)
        LAST_WALL_NS = wall_ns
        LAST_EXEC_NS = exec_ns if exec_ns is not None else wall_ns
        print("timed calls (s):", [f"{t:.4f}" for t in times])
        if exec_ns is not None:
            print(f"on-device exec (ntff): {exec_ns} ns; "
                  f"wall-clock min: {wall_ns} ns")
        LAST_RESULTS = results
    except Exception as e:
        print("timed runner failed, falling back:", e)
        from concourse.bass_utils import run_bass_kernel_spmd
        res = run_bass_kernel_spmd(nc, in_maps, core_ids=list(range(NC)))
        LAST_RESULTS = res.results
    return _postprocess(LAST_RESULTS, meta["g_lo"], batch,
                        np.asarray(cW1, np.float32), np.asarray(cb1, np.float32),
                        np.asarray(cg, np.float32), np.asarray(cbeta, np.float32),
                        np.asarray(cW2, np.float32), np.asarray(cb2, np.float32),
                        np.asarray(cW3, np.float32), np.asarray(cb3, np.float32))
